# revision 1
# baseline (speedup 1.0000x reference)
"""Trainium2 Bass kernel for EnhancedMLPDenoisingVAE.

Strategy: pure data parallel over 8 NeuronCores (4096 rows each).
Per core, activations are kept batch-major ([128 batch rows on
partitions, features on free dim]) for LayerNorm/softmax/elementwise;
each matmul consumes a PE-transposed copy of its input activation
(features on partitions) as the stationary operand, so every layer is

    psum = xT.T @ W    (K-tiled fp32r matmuls, fp32 accumulate)
    y    = psum + bias (DVE, PSUM->SBUF)
    LN stats (bn_stats/bn_aggr), xhat = (y - mu) * rstd
    xT'  = PE-transpose(xhat) -> ACT Prelu(xhat*g + b, 0.2) per chunk

The LN affine (gamma/beta) and leaky-relu are fused into the ScalarE
pass that copies each transposed chunk out of PSUM, where gamma/beta
are per-partition scalars.

Weights live in SBUF in three sequential phases (encoder / di+dec1 /
dec2+dec3) to fit; activations stage through DRAM between phases.
Matmuls run in float32r (~2e-4 rel err end to end); set MM_DTYPE to
mybir.dt.float32 for full fp32 at 4x the PE cost.
"""

from contextlib import ExitStack

import numpy as np

import concourse.bass as bass
import concourse.tile as tile
from concourse import bacc, mybir
from concourse.bass_utils import run_bass_kernel_spmd
from concourse.masks import make_identity

F32 = mybir.dt.float32
F32R = mybir.dt.float32r

B, D, H, L, M = 32768, 768, 1024, 256, 32
N_CORES = 8
B_LOC = B // N_CORES  # 4096
P = 128
NT = B_LOC // P  # 32 row tiles per core
LN_EPS = 1e-5
ALPHA = 0.2

MM_DTYPE = F32R  # dtype for the big matmuls


def _chunks(nf):
    return [(s, min(512, nf - s)) for s in range(0, nf, 512)]


def _build():
    nc = bacc.Bacc(
        "TRN2", target_bir_lowering=False, debug=False, num_devices=N_CORES
    )

    dram = lambda name, shape, dt=F32, kind="Internal": nc.dram_tensor(
        name, shape, dt, kind=kind
    )
    x_d = dram("x", [B_LOC, D], kind="ExternalInput")
    eps_d = dram("eps", [B_LOC, L], kind="ExternalInput")
    w1_d = dram("w1t", [D, H], kind="ExternalInput")
    b1_d = dram("b1", [H], kind="ExternalInput")
    g1_d = dram("g1", [H], kind="ExternalInput")
    be1_d = dram("be1", [H], kind="ExternalInput")
    w2_d = dram("w2t", [H, H], kind="ExternalInput")
    b2_d = dram("b2", [H], kind="ExternalInput")
    g2_d = dram("g2", [H], kind="ExternalInput")
    be2_d = dram("be2", [H], kind="ExternalInput")
    wmv_d = dram("wmvt", [H, 2 * L], kind="ExternalInput")
    bmv_d = dram("bmv", [2 * L], kind="ExternalInput")
    ctxT_d = dram("ctxT", [L, M], kind="ExternalInput")
    ctx_d = dram("ctx", [M, L], kind="ExternalInput")
    wdi_d = dram("wdit", [L, H], kind="ExternalInput")
    dib_d = dram("dib", [H], kind="ExternalInput")
    wd1_d = dram("wd1t", [H, H], kind="ExternalInput")
    db1_d = dram("db1", [H], kind="ExternalInput")
    dg1_d = dram("dg1", [H], kind="ExternalInput")
    dbe1_d = dram("dbe1", [H], kind="ExternalInput")
    wd2_d = dram("wd2t", [H, 2 * H], kind="ExternalInput")
    db2_d = dram("db2", [2 * H], kind="ExternalInput")
    dg2_d = dram("dg2", [2 * H], kind="ExternalInput")
    dbe2_d = dram("dbe2", [2 * H], kind="ExternalInput")
    wd3_d = dram("wd3t", [2 * H, D], kind="ExternalInput")
    db3_d = dram("db3", [D], kind="ExternalInput")

    recon_d = dram("recon", [B_LOC, D], kind="ExternalOutput")
    mu_d = dram("mu", [B_LOC, L], kind="ExternalOutput")
    lv_d = dram("lv", [B_LOC, L], kind="ExternalOutput")

    with tile.TileContext(nc) as tc, ExitStack() as glob:
        const = glob.enter_context(tc.tile_pool(name="const", bufs=1))
        dstash = glob.enter_context(
            tc.tile_pool(name="dstash", bufs=1, space="DRAM")
        )
        ident = const.tile([P, P], F32)
        make_identity(nc, ident)
        epsln = const.tile([P, 1], F32)
        nc.vector.memset(epsln, LN_EPS)

        zenh_s = dstash.tile([NT, P, L], F32)
        d2t_s = dstash.tile([NT, P, 8, P], MM_DTYPE)

        # ---------- shared helpers ----------
        def load_w(pool, dram_t, kt, nf, name):
            t = pool.tile([P, kt, nf], MM_DTYPE, name=name)
            nc.gpsimd.dma_start(
                t[:], dram_t.ap().rearrange("(kt p) n -> p kt n", p=P)
            )
            return t

        def load_bcast(pool, dram_t, nf, name):
            t = pool.tile([P, nf], F32, name=name)
            src = bass.AP(
                tensor=dram_t.ap().tensor, offset=0, ap=[[0, P], [1, nf]]
            )
            nc.gpsimd.dma_start(t[:], src)
            return t

        def load_packed(pool, dram_t, kt, name):
            t = pool.tile([P, kt], F32, name=name)
            nc.sync.dma_start(t[:], dram_t.ap().rearrange("(c p) -> p c", p=P))
            return t

        def mm_to(ps_pool, xT, w_sb, kt, y_out, bias_bc, tag):
            """y_out[:, n0:n0+nsz] = xT.T @ W chunk (+ bias)."""
            nf = w_sb.shape[2]
            for ci, (n0, nsz) in enumerate(_chunks(nf)):
                ps = ps_pool.tile(
                    [P, 512], F32, name=f"ps_{tag}_{ci}", tag="mmps"
                )
                for k in range(kt):
                    nc.tensor.matmul(
                        ps[:, :nsz],
                        xT[:, k, :],
                        w_sb[:, k, n0 : n0 + nsz],
                        start=(k == 0),
                        stop=(k == kt - 1),
                    )
                if bias_bc is not None:
                    nc.vector.tensor_tensor(
                        out=y_out[:, n0 : n0 + nsz],
                        in0=ps[:, :nsz],
                        in1=bias_bc[:, n0 : n0 + nsz],
                        op=mybir.AluOpType.add,
                    )
                else:
                    nc.vector.tensor_copy(y_out[:, n0 : n0 + nsz], ps[:, :nsz])

        def layernorm(work, y, nf, tag):
            """Returns xhat tile [P, nf] (pre-affine normalized)."""
            nsub = nf // 512
            stats = work.tile([P, nsub, 6], F32, name=f"stats_{tag}", tag="stats")
            for s in range(nsub):
                nc.vector.bn_stats(
                    out=stats[:, s, :], in_=y[:, s * 512 : (s + 1) * 512]
                )
            mv = work.tile([P, 2], F32, name=f"mv_{tag}", tag="mv")
            nc.vector.bn_aggr(out=mv[:], in_=stats[:])
            sd = work.tile([P, 1], F32, name=f"sd_{tag}", tag="sd")
            nc.scalar.activation(
                out=sd[:],
                in_=mv[:, 1:2],
                func=mybir.ActivationFunctionType.Sqrt,
                bias=epsln[:],
                scale=1.0,
            )
            rs = work.tile([P, 1], F32, name=f"rs_{tag}", tag="rs")
            nc.vector.reciprocal(out=rs[:], in_=sd[:])
            xh = work.tile([P, nf], F32, name=f"xh_{tag}", tag=f"xh{nf}")
            nc.vector.tensor_scalar(
                out=xh[:],
                in0=y[:],
                scalar1=mv[:, 0:1],
                scalar2=rs[:],
                op0=mybir.AluOpType.subtract,
                op1=mybir.AluOpType.mult,
            )
            return xh

        def transpose_to(work, ps_pool, src, kt, out_dt, tag, gb=None):
            """src [P, kt*128] -> out [P, kt, 128] transposed chunks.
            gb=(g_packed, b_packed) applies Prelu(x*g + b, 0.2) on ScalarE;
            gb='bias_only:(b,)' via g=None. None -> plain DVE copy."""
            out = work.tile([P, kt, P], out_dt, name=f"t_{tag}", tag=f"t_{tag}")
            for k in range(kt):
                tr = ps_pool.tile(
                    [P, P], F32, name=f"tr_{tag}_{k}", tag="trps"
                )
                nc.tensor.transpose(
                    tr[:], src[:, k * P : (k + 1) * P], ident[:]
                )
                if gb is not None:
                    g_p, b_p = gb
                    nc.scalar.activation(
                        out=out[:, k, :],
                        in_=tr[:],
                        func=mybir.ActivationFunctionType.Prelu,
                        bias=b_p[:, k : k + 1] if b_p is not None else 0.0,
                        scale=g_p[:, k : k + 1] if g_p is not None else 1.0,
                        alpha=ALPHA,
                    )
                else:
                    nc.vector.tensor_copy(out[:, k, :], tr[:])
            return out

        # ================= PHASE A: encoder =================
        with ExitStack() as ph:
            wp = ph.enter_context(tc.tile_pool(name="wA", bufs=1))
            work = ph.enter_context(tc.tile_pool(name="workA", bufs=2))
            ps_mm = ph.enter_context(
                tc.tile_pool(name="psA", bufs=4, space="PSUM")
            )
            ps_tr = ph.enter_context(
                tc.tile_pool(name="psAt", bufs=4, space="PSUM")
            )
            w1 = load_w(wp, w1_d, D // P, H, "w1")
            w2 = load_w(wp, w2_d, H // P, H, "w2")
            wmv = load_w(wp, wmv_d, H // P, 2 * L, "wmv")
            ctxT = wp.tile([P, 2, M], F32, name="ctxT")
            nc.sync.dma_start(
                ctxT[:], ctxT_d.ap().rearrange("(kt p) n -> p kt n", p=P)
            )
            ctxm = wp.tile([M, L], F32, name="ctxm")
            nc.sync.dma_start(ctxm[:], ctx_d.ap())
            b1c = load_bcast(wp, b1_d, H, "b1c")
            b2c = load_bcast(wp, b2_d, H, "b2c")
            bmvc = load_bcast(wp, bmv_d, 2 * L, "bmvc")
            g1p = load_packed(wp, g1_d, H // P, "g1p")
            be1p = load_packed(wp, be1_d, H // P, "be1p")
            g2p = load_packed(wp, g2_d, H // P, "g2p")
            be2p = load_packed(wp, be2_d, H // P, "be2p")

            for i in range(NT):
                r0 = i * P
                x_sb = work.tile([P, D], F32, name=f"x_{i}", tag="x")
                nc.sync.dma_start(x_sb[:], x_d[r0 : r0 + P, :])
                xT = transpose_to(work, ps_tr, x_sb, D // P, MM_DTYPE, "xT")

                y1 = work.tile([P, H], F32, name=f"y1_{i}", tag="y1")
                mm_to(ps_mm, xT, w1, D // P, y1, b1c, "l1")
                xh1 = layernorm(work, y1, H, "ln1")
                h1T = transpose_to(
                    work, ps_tr, xh1, H // P, MM_DTYPE, "h1T", (g1p, be1p)
                )

                y2 = work.tile([P, H], F32, name=f"y2_{i}", tag="y2")
                mm_to(ps_mm, h1T, w2, H // P, y2, b2c, "l2")
                xh2 = layernorm(work, y2, H, "ln2")
                h2T = transpose_to(
                    work, ps_tr, xh2, H // P, MM_DTYPE, "h2T", (g2p, be2p)
                )

                smv = work.tile([P, 2 * L], F32, name=f"smv_{i}", tag="smv")
                mm_to(ps_mm, h2T, wmv, H // P, smv, bmvc, "mv")
                nc.sync.dma_start(mu_d[r0 : r0 + P, :], smv[:, :L])
                nc.sync.dma_start(lv_d[r0 : r0 + P, :], smv[:, L:])

                elv = work.tile([P, L], F32, name=f"elv_{i}", tag="elv")
                nc.scalar.activation(
                    out=elv[:],
                    in_=smv[:, L:],
                    func=mybir.ActivationFunctionType.Exp,
                    bias=0.0,
                    scale=0.5,
                )
                eps_sb = work.tile([P, L], F32, name=f"eps_{i}", tag="eps")
                nc.sync.dma_start(eps_sb[:], eps_d[r0 : r0 + P, :])
                z_sb = work.tile([P, L], F32, name=f"z_{i}", tag="z")
                nc.vector.tensor_tensor(
                    out=z_sb[:], in0=elv[:], in1=eps_sb[:],
                    op=mybir.AluOpType.mult,
                )
                nc.vector.tensor_tensor(
                    out=z_sb[:], in0=z_sb[:], in1=smv[:, :L],
                    op=mybir.AluOpType.add,
                )

                # context attention (fp32)
                zT = transpose_to(work, ps_tr, z_sb, L // P, F32, "zT")
                s_ps = ps_mm.tile([P, 512], F32, name=f"sps_{i}", tag="mmps")
                for k in range(L // P):
                    nc.tensor.matmul(
                        s_ps[:, :M],
                        zT[:, k, :],
                        ctxT[:, k, :],
                        start=(k == 0),
                        stop=(k == L // P - 1),
                    )
                negmx = work.tile([P, 1], F32, name=f"negmx_{i}", tag="negmx")
                nc.vector.tensor_reduce(
                    out=negmx[:],
                    in_=s_ps[:, :M],
                    axis=mybir.AxisListType.X,
                    op=mybir.AluOpType.max,
                    negate=True,
                )
                e_sb = work.tile([P, M], F32, name=f"e_{i}", tag="e")
                se = work.tile([P, 1], F32, name=f"se_{i}", tag="se")
                nc.scalar.activation(
                    out=e_sb[:],
                    in_=s_ps[:, :M],
                    func=mybir.ActivationFunctionType.Exp,
                    bias=negmx[:],
                    scale=1.0,
                    accum_out=se[:],
                )
                rs01 = work.tile([P, 1], F32, name=f"rs01_{i}", tag="rs01")
                nc.vector.reciprocal(out=rs01[:], in_=se[:])
                nc.vector.tensor_scalar_mul(
                    out=rs01[:], in0=rs01[:], scalar1=0.1
                )
                trE = ps_tr.tile([P, P], F32, name=f"trE_{i}", tag="trps")
                nc.tensor.transpose(trE[:M, :], e_sb[:], ident[:])
                eT = work.tile([M, P], F32, name=f"eT_{i}", tag="eT")
                nc.vector.tensor_copy(eT[:], trE[:M, :])
                ze_ps = ps_mm.tile([P, 512], F32, name=f"zeps_{i}", tag="mmps")
                nc.tensor.matmul(
                    ze_ps[:, :L], eT[:], ctxm[:], start=True, stop=True
                )
                ze = work.tile([P, L], F32, name=f"ze_{i}", tag="ze")
                nc.vector.tensor_scalar(
                    out=ze[:],
                    in0=ze_ps[:, :L],
                    scalar1=rs01[:],
                    scalar2=None,
                    op0=mybir.AluOpType.mult,
                )
                nc.vector.tensor_tensor(
                    out=ze[:], in0=ze[:], in1=z_sb[:], op=mybir.AluOpType.add
                )
                nc.sync.dma_start(zenh_s[i], ze[:])

        # ================= PHASE B: di + dec1 =================
        with ExitStack() as ph:
            wp = ph.enter_context(tc.tile_pool(name="wB", bufs=1))
            work = ph.enter_context(tc.tile_pool(name="workB", bufs=2))
            ps_mm = ph.enter_context(
                tc.tile_pool(name="psB", bufs=4, space="PSUM")
            )
            ps_tr = ph.enter_context(
                tc.tile_pool(name="psBt", bufs=4, space="PSUM")
            )
            wdi = load_w(wp, wdi_d, L // P, H, "wdi")
            wd1 = load_w(wp, wd1_d, H // P, H, "wd1")
            dibp = load_packed(wp, dib_d, H // P, "dibp")
            db1c = load_bcast(wp, db1_d, H, "db1c")
            dg1p = load_packed(wp, dg1_d, H // P, "dg1p")
            dbe1p = load_packed(wp, dbe1_d, H // P, "dbe1p")

            for i in range(NT):
                ze_sb = work.tile([P, L], F32, name=f"zeB_{i}", tag="zeB")
                nc.sync.dma_start(ze_sb[:], zenh_s[i])
                zeT = transpose_to(work, ps_tr, ze_sb, L // P, MM_DTYPE, "zeT")

                ydi = work.tile([P, H], F32, name=f"ydi_{i}", tag="ydi")
                mm_to(ps_mm, zeT, wdi, L // P, ydi, None, "di")
                d1T = transpose_to(
                    work, ps_tr, ydi, H // P, MM_DTYPE, "d1T", (None, dibp)
                )

                y3 = work.tile([P, H], F32, name=f"y3_{i}", tag="y3")
                mm_to(ps_mm, d1T, wd1, H // P, y3, db1c, "d1")
                xh3 = layernorm(work, y3, H, "ln3")
                d2T = transpose_to(
                    work, ps_tr, xh3, H // P, MM_DTYPE, "d2T", (dg1p, dbe1p)
                )
                nc.sync.dma_start(d2t_s[i], d2T[:])

        # ================= PHASE C: dec2 + dec3 =================
        with ExitStack() as ph:
            wp = ph.enter_context(tc.tile_pool(name="wC", bufs=1))
            work = ph.enter_context(tc.tile_pool(name="workC", bufs=2))
            ps_mm = ph.enter_context(
                tc.tile_pool(name="psC", bufs=4, space="PSUM")
            )
            ps_tr = ph.enter_context(
                tc.tile_pool(name="psCt", bufs=4, space="PSUM")
            )
            wd2 = load_w(wp, wd2_d, H // P, 2 * H, "wd2")
            wd3 = load_w(wp, wd3_d, 2 * H // P, D, "wd3")
            db2c = load_bcast(wp, db2_d, 2 * H, "db2c")
            dg2p = load_packed(wp, dg2_d, 2 * H // P, "dg2p")
            dbe2p = load_packed(wp, dbe2_d, 2 * H // P, "dbe2p")
            db3c = load_bcast(wp, db3_d, D, "db3c")

            for i in range(NT):
                r0 = i * P
                d2T = work.tile([P, 8, P], MM_DTYPE, name=f"d2TC_{i}", tag="d2TC")
                nc.sync.dma_start(d2T[:], d2t_s[i])

                y4 = work.tile([P, 2 * H], F32, name=f"y4_{i}", tag="y4")
                mm_to(ps_mm, d2T, wd2, H // P, y4, db2c, "d2")
                xh4 = layernorm(work, y4, 2 * H, "ln4")
                d3T = transpose_to(
                    work, ps_tr, xh4, 2 * H // P, MM_DTYPE, "d3T",
                    (dg2p, dbe2p),
                )

                recon_sb = work.tile([P, D], F32, name=f"rec_{i}", tag="rec")
                mm_to(ps_mm, d3T, wd3, 2 * H // P, recon_sb, db3c, "d3")
                nc.sync.dma_start(recon_d[r0 : r0 + P, :], recon_sb[:])

    nc.finalize()
    return nc


_NC_CACHE = {}


def _get_nc():
    if "nc" not in _NC_CACHE:
        _NC_CACHE["nc"] = _build()
    return _NC_CACHE["nc"]


def kernel(**inputs):
    i = {k: np.ascontiguousarray(np.asarray(v, dtype=np.float32)) for k, v in inputs.items()}
    nc = _get_nc()

    shared = {
        "w1t": np.ascontiguousarray(i["enc_w1"].T),
        "b1": i["enc_b1"],
        "g1": i["ln1_g"],
        "be1": i["ln1_b"],
        "w2t": np.ascontiguousarray(i["enc_w2"].T),
        "b2": i["enc_b2"],
        "g2": i["ln2_g"],
        "be2": i["ln2_b"],
        "wmvt": np.ascontiguousarray(
            np.concatenate([i["mu_w"].T, i["lv_w"].T], axis=1)
        ),
        "bmv": np.concatenate([i["mu_b"], i["lv_b"]]),
        "ctxT": np.ascontiguousarray(i["ctx_mem"].T),
        "ctx": i["ctx_mem"],
        "wdit": np.ascontiguousarray(i["di_w"].T),
        "dib": i["di_b"],
        "wd1t": np.ascontiguousarray(i["dec_w1"].T),
        "db1": i["dec_b1"],
        "dg1": i["dln1_g"],
        "dbe1": i["dln1_b"],
        "wd2t": np.ascontiguousarray(i["dec_w2"].T),
        "db2": i["dec_b2"],
        "dg2": i["dln2_g"],
        "dbe2": i["dln2_b"],
        "wd3t": np.ascontiguousarray(i["dec_w3"].T),
        "db3": i["dec_b3"],
    }
    in_maps = []
    for c in range(N_CORES):
        m = dict(shared)
        m["x"] = i["x"][c * B_LOC : (c + 1) * B_LOC]
        m["eps"] = i["eps"][c * B_LOC : (c + 1) * B_LOC]
        in_maps.append(m)

    res = run_bass_kernel_spmd(nc, in_maps, core_ids=list(range(N_CORES)))
    recon = np.concatenate([r["recon"] for r in res.results], axis=0)
    mu = np.concatenate([r["mu"] for r in res.results], axis=0)
    lv = np.concatenate([r["lv"] for r in res.results], axis=0)
    return recon, mu, lv


# revision 15
# speedup vs baseline: 1.5180x; 1.5180x over previous
"""Trainium2 Bass kernel for EnhancedMLPDenoisingVAE.

Pure data parallel over 8 NeuronCores (4096 rows each). Activations are
batch-major ([128 batch rows on partitions, features free]); each matmul
consumes a PE-transposed copy of its input (features on partitions) as
the stationary operand:

    psum = xT.T @ W            K-tiled fp32r matmuls, fp32 accumulate
    LN stats from PSUM         bn_stats/bn_aggr on DVE
    h = Prelu(psum*rs - mu*rs) one wide ScalarE op per 512-chunk
                               (LN normalize + leaky-relu fused, 0.2)
    hT = PE-transpose(h)       fp32r transposes into shared 512-wide
                               PSUM groups, wide DVE copies out

When LN gamma/beta are not (1, 0) or a layer bias is nonzero (never the
case for this model's setup_inputs), per-layer fallbacks reproduce the
general math: bias is added via a broadcast tile on DVE, and gamma/beta
are applied per transposed chunk on ScalarE where they are per-partition
scalars.

Weights live in SBUF in three sequential phases (encoder / di+dec1 /
dec2+dec3); activations stage through DRAM between phases. fp32r
matmuls give ~4e-4 max rel err end to end; MM_DTYPE=float32 is the
full-precision fallback at 4x PE cost.
"""

from contextlib import ExitStack

import numpy as np

import concourse.bass as bass
import concourse.tile as tile
from concourse import bacc, mybir
from concourse.bass_utils import run_bass_kernel_spmd
from concourse.masks import make_identity

F32 = mybir.dt.float32
F32R = mybir.dt.float32r
AF = mybir.ActivationFunctionType
ALU = mybir.AluOpType

B, D, H, L, M = 32768, 768, 1024, 256, 32
N_CORES = 8
B_LOC = B // N_CORES  # 4096
P = 128
NT = B_LOC // P  # 32 row tiles per core
LN_EPS = 1e-5
ALPHA = 0.2

MM_DTYPE = F32R


def _chunks(nf, sz=512):
    return [(s, min(sz, nf - s)) for s in range(0, nf, sz)]


def _build(simple=True, mm_dtype=None, wbufs=(2, 2, 2), psbufs=4, trbufs=3,
           sbufs=4, ORDER_A=None, ORDER_B=None, ORDER_C=None):
    ORDER_A = ORDER_A or [0, 1, 2, 3, 4]
    ORDER_B = ORDER_B or [0, 1, 2]
    ORDER_C = ORDER_C or [0, 1, 2]
    """simple=True assumes all biases zero and LN gamma=1/beta=0 (true for
    this model's setup_inputs); simple=False emits the general math."""
    if isinstance(wbufs, int):
        wbufs = (wbufs, wbufs, wbufs)
    mmdt = MM_DTYPE if mm_dtype is None else mm_dtype
    nc = bacc.Bacc(
        "TRN2", target_bir_lowering=False, debug=False, num_devices=N_CORES
    )

    dram = lambda name, shape, dt=F32, kind="ExternalInput": nc.dram_tensor(
        name, shape, dt, kind=kind
    )
    x_d = dram("x", [B_LOC, D])
    eps_d = dram("eps", [B_LOC, L])
    w1_d = dram("w1t", [D, H])
    b1_d = dram("b1", [H])
    g1_d = dram("g1", [H])
    be1_d = dram("be1", [H])
    w2_d = dram("w2t", [H, H])
    b2_d = dram("b2", [H])
    g2_d = dram("g2", [H])
    be2_d = dram("be2", [H])
    wmv_d = dram("wmvt", [H, 2 * L])
    bmv_d = dram("bmv", [2 * L])
    ctxT_d = dram("ctxT", [L, M])
    ctx_d = dram("ctx", [M, L])
    wdi_d = dram("wdit", [L, H])
    dib_d = dram("dib", [H])
    wd1_d = dram("wd1t", [H, H])
    db1_d = dram("db1", [H])
    dg1_d = dram("dg1", [H])
    dbe1_d = dram("dbe1", [H])
    wd2_d = dram("wd2t", [H, 2 * H])
    db2_d = dram("db2", [2 * H])
    dg2_d = dram("dg2", [2 * H])
    dbe2_d = dram("dbe2", [2 * H])
    wd3_d = dram("wd3t", [2 * H, D])
    db3_d = dram("db3", [D])

    recon_d = dram("recon", [B_LOC, D], kind="ExternalOutput")
    mu_d = dram("mu", [B_LOC, L], kind="ExternalOutput")
    lv_d = dram("lv", [B_LOC, L], kind="ExternalOutput")

    with tile.TileContext(nc, pool_alloc_mode="queue") as tc, ExitStack() as glob:
        const = glob.enter_context(tc.tile_pool(name="const", bufs=1))
        dstash = glob.enter_context(
            tc.tile_pool(name="dstash", bufs=1, space="DRAM")
        )
        ident = const.tile([P, P], F32)
        make_identity(nc, ident)
        identr = const.tile([P, P], F32R)
        nc.vector.tensor_copy(identr[:], ident[:])
        epsln = const.tile([P, 1], F32)
        nc.vector.memset(epsln, LN_EPS)

        zenh_s = dstash.tile([NT, P, L], F32)
        d2t_s = dstash.tile([NT, P, 8, P], mmdt)
        wpB = glob.enter_context(tc.tile_pool(name="wB", bufs=1))
        prefetch = {}

        # ---------- helpers ----------
        def load_w(pool, dram_t, kt, nf, name):
            t = pool.tile([P, kt, nf], mmdt, name=name)
            nc.gpsimd.dma_start(
                t[:], dram_t.ap().rearrange("(kt p) n -> p kt n", p=P)
            )
            return t

        def load_bcast(pool, dram_t, nf, name):
            t = pool.tile([P, nf], F32, name=name)
            src = bass.AP(
                tensor=dram_t.ap().tensor, offset=0, ap=[[0, P], [1, nf]]
            )
            nc.gpsimd.dma_start(t[:], src)
            return t

        def load_packed(pool, dram_t, kt, name):
            t = pool.tile([P, kt], F32, name=name)
            nc.sync.dma_start(t[:], dram_t.ap().rearrange("(c p) -> p c", p=P))
            return t

        def mm_chunks(ps_pool, xT, w_sb, kt, tag):
            """Returns [(psum_tile, n0, nsz)] for all 512-chunks."""
            nf = w_sb.shape[2]
            out = []
            for ci, (n0, nsz) in enumerate(_chunks(nf)):
                ps = ps_pool.tile(
                    [P, 512], F32, name=f"ps_{tag}_{ci}", tag="mmps",
                    bufs=psbufs,
                )
                for k in range(kt):
                    nc.tensor.matmul(
                        ps[:, :nsz],
                        xT[:, k, :],
                        w_sb[:, k, n0 : n0 + nsz],
                        start=(k == 0),
                        stop=(k == kt - 1),
                    )
                out.append((ps, n0, nsz))
            return out

        def ln_stats(work, srcs, tag):
            """bn stats over chunk aps -> (negmu [P,1], mv [P,2])."""
            nsub = len(srcs)
            stats = work.tile(
                [P, nsub, 6], F32, name=f"st_{tag}", tag="stats", bufs=sbufs
            )
            for s, (src, n0, nsz) in enumerate(srcs):
                nc.vector.bn_stats(out=stats[:, s, :], in_=src[:, :nsz])
            mv = work.tile([P, 2], F32, name=f"mv_{tag}", tag="mv", bufs=sbufs)
            nc.vector.bn_aggr(out=mv[:], in_=stats[:])
            nmu = work.tile(
                [P, 1], F32, name=f"nmu_{tag}", tag="nmu", bufs=sbufs
            )
            nc.vector.tensor_scalar_mul(out=nmu[:], in0=mv[:, 0:1],
                                        scalar1=-1.0)
            return nmu, mv

        def rsqrt_dve(work, mv, tag):
            """rs = 1/sqrt(var + eps) via bit-trick + 2 Newton iters (DVE
            only -- keeps Sqrt off ScalarE so its LUT set never swaps)."""
            I32 = mybir.dt.int32
            v1 = work.tile([P, 1], F32, name=f"v1_{tag}", tag="v1",
                           bufs=sbufs)
            nc.vector.tensor_scalar_add(out=v1[:], in0=mv[:, 1:2],
                                        scalar1=LN_EPS)
            ti = work.tile([P, 1], I32, name=f"ti_{tag}", tag="ti",
                           bufs=sbufs)
            nc.vector.tensor_scalar(
                out=ti[:], in0=v1[:].bitcast(I32), scalar1=1, scalar2=None,
                op0=ALU.logical_shift_right,
            )
            nc.vector.tensor_scalar(
                out=ti[:], in0=ti[:], scalar1=-1, scalar2=0x5F3759DF,
                op0=ALU.mult, op1=ALU.add,
            )
            y = work.tile([P, 1], F32, name=f"yq_{tag}", tag="yq",
                          bufs=sbufs)
            nc.vector.tensor_copy(y[:], ti[:].bitcast(F32))
            hv = work.tile([P, 1], F32, name=f"hv_{tag}", tag="hv",
                           bufs=sbufs)
            nc.vector.tensor_scalar_mul(out=hv[:], in0=v1[:], scalar1=0.5)
            tq = work.tile([P, 1], F32, name=f"tq_{tag}", tag="tq",
                           bufs=sbufs)
            for _ in range(2):
                nc.vector.tensor_tensor(out=tq[:], in0=y[:], in1=y[:],
                                        op=ALU.mult)
                nc.vector.tensor_tensor(out=tq[:], in0=tq[:], in1=hv[:],
                                        op=ALU.mult)
                nc.vector.tensor_scalar(
                    out=tq[:], in0=tq[:], scalar1=-1.0, scalar2=1.5,
                    op0=ALU.mult, op1=ALU.add,
                )
                nc.vector.tensor_tensor(out=y[:], in0=y[:], in1=tq[:],
                                        op=ALU.mult)
            return y

        def transpose_in(work, ps_pool, src_sb, kt, out_dt, idt, tag,
                         grp=None):
            grp = grp or tag
            """src [P, kt*128] -> [P, kt, 128] via PE transposes grouped
            into 512-wide PSUM tiles + wide DVE copies."""
            out = work.tile(
                [P, kt, P], out_dt, name=f"t_{tag}", tag=f"t_{grp}",
                bufs=(sbufs if kt <= 8 else 2),
            )
            for g0 in range(0, kt, 4):
                gn = min(4, kt - g0)
                pw = ps_pool.tile(
                    [P, 512], F32, name=f"tw_{tag}_{g0}", tag="trps",
                    bufs=trbufs,
                )
                for j in range(gn):
                    dst = pw[:, j * P : (j + 1) * P]
                    src_c = src_sb[:, (g0 + j) * P : (g0 + j + 1) * P]
                    if src_c.dtype == F32R:
                        dst = dst.bitcast(F32R)
                    nc.tensor.transpose(dst, src_c, idt)
                nc.vector.tensor_copy(
                    out[:, g0 : g0 + gn, :].rearrange("p k c -> p (k c)"),
                    pw[:, : gn * P],
                )
            return out

        def dense_fast(work, ps_mm, ps_tr, xT, w_sb, kt, tag, *,
                       ln, lrelu, out_kt, need_rs=False, grp=None):
            grp = grp or tag
            """simple-path layer: matmul -> (LN shift) -> lrelu ->
            transposed fp32r copy. The LN 1/std factor is NOT applied
            here: leaky-relu is positively homogeneous and LN is
            scale-invariant per sample, so the factor cancels through
            the next LN; layers feeding non-LN consumers get it back
            via need_rs (folded into the consumer's PSUM copy)."""
            srcs = mm_chunks(ps_mm, xT, w_sb, kt, tag)
            nf = w_sb.shape[2]
            h = work.tile(
                [P, nf], mmdt, name=f"h_{tag}", tag=f"h{nf}", bufs=2
            )
            rs = None
            if ln:
                nmu, mv = ln_stats(work, srcs, tag)
                if need_rs:
                    rs = rsqrt_dve(work, mv, tag)
                for src, n0, nsz in srcs:
                    nc.scalar.activation(
                        out=h[:, n0 : n0 + nsz], in_=src[:, :nsz],
                        func=AF.Prelu, bias=nmu[:], scale=1.0, alpha=ALPHA,
                    )
            else:
                assert lrelu
                for src, n0, nsz in srcs:
                    nc.scalar.activation(
                        out=h[:, n0 : n0 + nsz], in_=src[:, :nsz],
                        func=AF.Prelu, bias=0.0, scale=1.0, alpha=ALPHA,
                    )
            tT = transpose_in(
                work, ps_tr, h, out_kt, mmdt,
                identr if mmdt == F32R else ident, f"{tag}T",
                grp=f"{grp}T",
            )
            return (tT, rs) if need_rs else tT

        def dense_general(work, ps_mm, ps_tr, xT, w_sb, kt, tag, *,
                          ln, lrelu, out_kt, bias_bc, gp, bep, grp=None):
            grp = grp or tag
            """general-path layer (nonzero bias / non-unit gamma):
            y = psum + bias; xhat = (y-mu)*rs; transpose; per-chunk
            ScalarE Prelu(xhat*g + beta)."""
            srcs = mm_chunks(ps_mm, xT, w_sb, kt, tag)
            nf = w_sb.shape[2]
            y = work.tile([P, nf], F32, name=f"y_{tag}", tag=f"y{nf}", bufs=2)
            for src, n0, nsz in srcs:
                if bias_bc is not None:
                    nc.vector.tensor_tensor(
                        out=y[:, n0 : n0 + nsz], in0=src[:, :nsz],
                        in1=bias_bc[:, n0 : n0 + nsz], op=ALU.add,
                    )
                else:
                    nc.vector.tensor_copy(y[:, n0 : n0 + nsz], src[:, :nsz])
            xh = y
            if ln:
                nmu, mv = ln_stats(
                    work,
                    [(y[:, n0 : n0 + nsz], n0, nsz) for _, n0, nsz in srcs],
                    tag,
                )
                rs = rsqrt_dve(work, mv, tag)
                xh = work.tile(
                    [P, nf], F32, name=f"xh_{tag}", tag=f"xh{nf}", bufs=2
                )
                nc.vector.tensor_scalar(
                    out=xh[:], in0=y[:], scalar1=nmu[:], scalar2=rs[:],
                    op0=ALU.add, op1=ALU.mult,
                )
            out = work.tile(
                [P, out_kt, P], mmdt, name=f"t_{tag}", tag=f"t_{grp}",
                bufs=2,
            )
            for g0 in range(0, out_kt, 4):
                gn = min(4, out_kt - g0)
                pw = ps_tr.tile(
                    [P, 512], F32, name=f"tw_{tag}_{g0}", tag="trps",
                    bufs=trbufs,
                )
                for j in range(gn):
                    nc.tensor.transpose(
                        pw[:, j * P : (j + 1) * P],
                        xh[:, (g0 + j) * P : (g0 + j + 1) * P],
                        ident,
                    )
                for j in range(gn):
                    k = g0 + j
                    nc.scalar.activation(
                        out=out[:, k, :], in_=pw[:, j * P : (j + 1) * P],
                        func=AF.Prelu if (ln or lrelu) else AF.Identity,
                        bias=bep[:, k : k + 1] if bep is not None else 0.0,
                        scale=gp[:, k : k + 1] if gp is not None else 1.0,
                        alpha=ALPHA,
                    )
            return out

        def raw_out(work, srcs, nf, tag, bias_bc=None, row_scale=None,
                    grp=None):
            o = work.tile([P, nf], F32, name=f"o_{tag}", tag=f"o_{grp or tag}",
                          bufs=2)
            for src, n0, nsz in srcs:
                if bias_bc is not None:
                    nc.vector.tensor_tensor(
                        out=o[:, n0 : n0 + nsz], in0=src[:, :nsz],
                        in1=bias_bc[:, n0 : n0 + nsz], op=ALU.add,
                    )
                elif row_scale is not None:
                    nc.vector.tensor_scalar(
                        out=o[:, n0 : n0 + nsz], in0=src[:, :nsz],
                        scalar1=row_scale[:], scalar2=None, op0=ALU.mult,
                    )
                else:
                    nc.vector.tensor_copy(o[:, n0 : n0 + nsz], src[:, :nsz])
            return o

        def sw_pipeline(stage_fns, n, order=None):
            """Software-pipelined emission: the Tile scheduler is a
            priority-list scheduler, so per-engine execution order tracks
            emission order -- interleaving stages of neighboring row-tiles
            here is what lets PE run tile j+1 matmuls while tile j's
            LN/softmax chain is on DVE/ScalarE. `order` sets the
            intra-tick stage emission order (default deepest-first)."""
            S = len(stage_fns)
            if order is None:
                order = list(range(S - 1, -1, -1))
            states = [dict() for _ in range(n)]
            for t in range(n + S - 1):
                for s in order:
                    j = t - s
                    if 0 <= j < n:
                        stage_fns[s](j, states[j])

        # ================= PHASE A: encoder =================
        with ExitStack() as ph:
            wp = ph.enter_context(tc.tile_pool(name="wA", bufs=1))
            work = ph.enter_context(tc.tile_pool(name="workA", bufs=wbufs[0]))
            ps_mm = ph.enter_context(
                tc.tile_pool(name="psA", bufs=1, space="PSUM")
            )
            ps_tr = ph.enter_context(
                tc.tile_pool(name="psAt", bufs=1, space="PSUM")
            )
            w1 = load_w(wp, w1_d, D // P, H, "w1")
            w2 = load_w(wp, w2_d, H // P, H, "w2")
            wmv = load_w(wp, wmv_d, H // P, 2 * L, "wmv")
            ctxT = wp.tile([P, 2, M], F32, name="ctxT")
            nc.sync.dma_start(
                ctxT[:], ctxT_d.ap().rearrange("(kt p) n -> p kt n", p=P)
            )
            ctxm = wp.tile([M, L], F32, name="ctxm")
            nc.sync.dma_start(ctxm[:], ctx_d.ap())
            if not simple:
                b1c = load_bcast(wp, b1_d, H, "b1c")
                b2c = load_bcast(wp, b2_d, H, "b2c")
                bmvc = load_bcast(wp, bmv_d, 2 * L, "bmvc")
                g1p = load_packed(wp, g1_d, H // P, "g1p")
                be1p = load_packed(wp, be1_d, H // P, "be1p")
                g2p = load_packed(wp, g2_d, H // P, "g2p")
                be2p = load_packed(wp, be2_d, H // P, "be2p")

            def sA0(i, st):
                if i == 2:
                    prefetch["wdi"] = load_w(wpB, wdi_d, L // P, H, "wdi")
                    prefetch["wd1"] = load_w(wpB, wd1_d, H // P, H, "wd1")
                r0 = i * P
                x_sb = work.tile(
                    [P, D], F32, name=f"x_{i}", tag="x", bufs=sbufs
                )
                nc.sync.dma_start(x_sb[:], x_d[r0 : r0 + P, :])
                st["xT"] = transpose_in(
                    work, ps_tr, x_sb, D // P, mmdt, ident, f"xT{i}",
                    grp="xT",
                )

            def sA1(i, st):
                if simple:
                    st["h1T"] = dense_fast(
                        work, ps_mm, ps_tr, st["xT"], w1, D // P, f"l1_{i}",
                        ln=True, lrelu=True, out_kt=H // P, grp="l1",
                    )
                else:
                    st["h1T"] = dense_general(
                        work, ps_mm, ps_tr, st["xT"], w1, D // P, f"l1_{i}",
                        ln=True, lrelu=True, out_kt=H // P,
                        bias_bc=b1c, gp=g1p, bep=be1p, grp="l1",
                    )

            def sA2(i, st):
                if simple:
                    st["h2T"], st["rs2"] = dense_fast(
                        work, ps_mm, ps_tr, st["h1T"], w2, H // P, f"l2_{i}",
                        ln=True, lrelu=True, out_kt=H // P, need_rs=True,
                        grp="l2",
                    )
                else:
                    st["h2T"] = dense_general(
                        work, ps_mm, ps_tr, st["h1T"], w2, H // P, f"l2_{i}",
                        ln=True, lrelu=True, out_kt=H // P,
                        bias_bc=b2c, gp=g2p, bep=be2p, grp="l2",
                    )
                    st["rs2"] = None

            def sA3(i, st):
                r0 = i * P
                if simple:
                    smv = raw_out(
                        work, mm_chunks(ps_mm, st["h2T"], wmv, H // P,
                                        f"mv_{i}"),
                        2 * L, f"mv_{i}", row_scale=st["rs2"], grp="mv",
                    )
                else:
                    smv = raw_out(
                        work, mm_chunks(ps_mm, st["h2T"], wmv, H // P,
                                        f"mv_{i}"),
                        2 * L, f"mv_{i}", bias_bc=bmvc, grp="mv",
                    )
                nc.sync.dma_start(mu_d[r0 : r0 + P, :], smv[:, :L])
                nc.sync.dma_start(lv_d[r0 : r0 + P, :], smv[:, L:])

                elv = work.tile([P, L], F32, name=f"elv_{i}", tag="elv",
                                bufs=sbufs)
                nc.scalar.activation(
                    out=elv[:], in_=smv[:, L:], func=AF.Exp, bias=0.0,
                    scale=0.5,
                )
                eps_sb = work.tile([P, L], F32, name=f"eps_{i}", tag="eps",
                                   bufs=sbufs)
                nc.sync.dma_start(eps_sb[:], eps_d[r0 : r0 + P, :])
                z_sb = work.tile([P, L], F32, name=f"z_{i}", tag="z",
                                 bufs=sbufs)
                nc.vector.tensor_tensor(
                    out=z_sb[:], in0=elv[:], in1=eps_sb[:], op=ALU.mult
                )
                nc.vector.tensor_tensor(
                    out=z_sb[:], in0=z_sb[:], in1=smv[:, :L], op=ALU.add
                )
                st["z"] = z_sb

            def sA4(i, st):
                z_sb = st["z"]
                zT = transpose_in(work, ps_tr, z_sb, L // P, F32, ident,
                                  f"zT{i}", grp="zT")
                s_ps = ps_mm.tile([P, 512], F32, name=f"sps_{i}", tag="mmps",
                                  bufs=psbufs)
                for k in range(L // P):
                    nc.tensor.matmul(
                        s_ps[:, :M], zT[:, k, :], ctxT[:, k, :],
                        start=(k == 0), stop=(k == L // P - 1),
                    )
                negmx = work.tile([P, 1], F32, name=f"nmx_{i}", tag="nmx",
                                  bufs=sbufs)
                nc.vector.tensor_reduce(
                    out=negmx[:], in_=s_ps[:, :M],
                    axis=mybir.AxisListType.X, op=ALU.max, negate=True,
                )
                e_sb = work.tile([P, M], F32, name=f"e_{i}", tag="e",
                                 bufs=sbufs)
                se = work.tile([P, 1], F32, name=f"se_{i}", tag="se",
                               bufs=sbufs)
                nc.scalar.activation(
                    out=e_sb[:], in_=s_ps[:, :M], func=AF.Exp,
                    bias=negmx[:], scale=1.0, accum_out=se[:],
                )
                rs01 = work.tile([P, 1], F32, name=f"r01_{i}", tag="r01",
                                 bufs=sbufs)
                nc.vector.reciprocal(out=rs01[:], in_=se[:])
                nc.vector.tensor_scalar_mul(
                    out=rs01[:], in0=rs01[:], scalar1=0.1
                )
                trE = ps_tr.tile([P, 512], F32, name=f"trE_{i}", tag="trps",
                                 bufs=trbufs)
                nc.tensor.transpose(trE[:M, :P], e_sb[:], ident[:])
                eT = work.tile([M, P], F32, name=f"eT_{i}", tag="eT",
                               bufs=sbufs)
                nc.vector.tensor_copy(eT[:], trE[:M, :P])
                ze_ps = ps_mm.tile([P, 512], F32, name=f"zeps_{i}",
                                   tag="mmps", bufs=psbufs)
                nc.tensor.matmul(
                    ze_ps[:, :L], eT[:], ctxm[:], start=True, stop=True
                )
                ze = work.tile([P, L], F32, name=f"ze_{i}", tag="ze",
                               bufs=sbufs)
                nc.vector.tensor_scalar(
                    out=ze[:], in0=ze_ps[:, :L], scalar1=rs01[:],
                    scalar2=None, op0=ALU.mult,
                )
                nc.vector.tensor_tensor(
                    out=ze[:], in0=ze[:], in1=z_sb[:], op=ALU.add
                )
                nc.sync.dma_start(zenh_s[i], ze[:])
                st.clear()

            sw_pipeline([sA0, sA1, sA2, sA3, sA4], NT, order=ORDER_A)

        # ================= PHASE B: di + dec1 =================
        wpC = glob.enter_context(tc.tile_pool(name="wC", bufs=1))
        with ExitStack() as ph:
            work = ph.enter_context(tc.tile_pool(name="workB", bufs=wbufs[1]))
            ps_mm = ph.enter_context(
                tc.tile_pool(name="psB", bufs=1, space="PSUM")
            )
            ps_tr = ph.enter_context(
                tc.tile_pool(name="psBt", bufs=1, space="PSUM")
            )
            wdi = prefetch["wdi"]
            wd1 = prefetch["wd1"]
            if not simple:
                dibp = load_packed(wpB, dib_d, H // P, "dibp")
                db1c = load_bcast(wpB, db1_d, H, "db1c")
                dg1p = load_packed(wpB, dg1_d, H // P, "dg1p")
                dbe1p = load_packed(wpB, dbe1_d, H // P, "dbe1p")

            def sB0(i, st):
                if i == 2:
                    prefetch["wd2"] = load_w(wpC, wd2_d, H // P, 2 * H, "wd2")
                    prefetch["wd3"] = load_w(wpC, wd3_d, 2 * H // P, D, "wd3")
                ze_sb = work.tile([P, L], F32, name=f"zeB_{i}", tag="zeB",
                                  bufs=sbufs)
                nc.sync.dma_start(ze_sb[:], zenh_s[i])
                st["zeT"] = transpose_in(
                    work, ps_tr, ze_sb, L // P, mmdt, ident, f"zeT{i}",
                    grp="zeT",
                )

            def sB1(i, st):
                if simple:
                    st["d1T"] = dense_fast(
                        work, ps_mm, ps_tr, st["zeT"], wdi, L // P, f"di_{i}",
                        ln=False, lrelu=True, out_kt=H // P, grp="di",
                    )
                else:
                    st["d1T"] = dense_general(
                        work, ps_mm, ps_tr, st["zeT"], wdi, L // P, f"di_{i}",
                        ln=False, lrelu=True, out_kt=H // P,
                        bias_bc=None, gp=None, bep=dibp, grp="di",
                    )

            def sB2(i, st):
                if simple:
                    d2T = dense_fast(
                        work, ps_mm, ps_tr, st["d1T"], wd1, H // P, f"d1_{i}",
                        ln=True, lrelu=True, out_kt=H // P, grp="d1",
                    )
                else:
                    d2T = dense_general(
                        work, ps_mm, ps_tr, st["d1T"], wd1, H // P, f"d1_{i}",
                        ln=True, lrelu=True, out_kt=H // P,
                        bias_bc=db1c, gp=dg1p, bep=dbe1p, grp="d1",
                    )
                nc.sync.dma_start(d2t_s[i], d2T[:])
                st.clear()

            sw_pipeline([sB0, sB1, sB2], NT, order=ORDER_B)

        # ================= PHASE C: dec2 + dec3 =================
        with ExitStack() as ph:
            work = ph.enter_context(tc.tile_pool(name="workC", bufs=wbufs[2]))
            ps_mm = ph.enter_context(
                tc.tile_pool(name="psC", bufs=1, space="PSUM")
            )
            ps_tr = ph.enter_context(
                tc.tile_pool(name="psCt", bufs=1, space="PSUM")
            )
            wd2 = prefetch["wd2"]
            wd3 = prefetch["wd3"]
            if not simple:
                db2c = load_bcast(wpC, db2_d, 2 * H, "db2c")
                dg2p = load_packed(wpC, dg2_d, 2 * H // P, "dg2p")
                dbe2p = load_packed(wpC, dbe2_d, 2 * H // P, "dbe2p")
                db3c = load_bcast(wpC, db3_d, D, "db3c")

            def sC0(i, st):
                d2T = work.tile([P, 8, P], mmdt, name=f"d2C_{i}", tag="d2C",
                                bufs=3)
                nc.sync.dma_start(d2T[:], d2t_s[i])
                st["d2T"] = d2T

            def sC1(i, st):
                if simple:
                    st["d3T"], st["rs4"] = dense_fast(
                        work, ps_mm, ps_tr, st["d2T"], wd2, H // P, f"d2_{i}",
                        ln=True, lrelu=True, out_kt=2 * H // P, need_rs=True,
                        grp="d2",
                    )
                else:
                    st["d3T"] = dense_general(
                        work, ps_mm, ps_tr, st["d2T"], wd2, H // P, f"d2_{i}",
                        ln=True, lrelu=True, out_kt=2 * H // P,
                        bias_bc=db2c, gp=dg2p, bep=dbe2p, grp="d2",
                    )
                    st["rs4"] = None

            def sC2(i, st):
                r0 = i * P
                if simple:
                    recon_sb = raw_out(
                        work,
                        mm_chunks(ps_mm, st["d3T"], wd3, 2 * H // P,
                                  f"d3_{i}"),
                        D, f"d3_{i}", row_scale=st["rs4"], grp="d3",
                    )
                else:
                    recon_sb = raw_out(
                        work,
                        mm_chunks(ps_mm, st["d3T"], wd3, 2 * H // P,
                                  f"d3_{i}"),
                        D, f"d3_{i}", bias_bc=db3c, grp="d3",
                    )
                nc.sync.dma_start(recon_d[r0 : r0 + P, :], recon_sb[:])
                st.clear()

            sw_pipeline([sC0, sC1, sC2], NT, order=ORDER_C)

    nc.finalize()
    return nc


_NC_CACHE = {}


def _get_nc(simple=True):
    key = ("simple" if simple else "general", str(MM_DTYPE))
    if key not in _NC_CACHE:
        _NC_CACHE[key] = _build(simple=simple)
    return _NC_CACHE[key]


def kernel(**inputs):
    i = {
        k: np.ascontiguousarray(np.asarray(v, dtype=np.float32))
        for k, v in inputs.items()
    }
    zeros = all(
        not np.any(i[k])
        for k in (
            "enc_b1", "enc_b2", "mu_b", "lv_b", "di_b", "dec_b1", "dec_b2",
            "dec_b3", "ln1_b", "ln2_b", "dln1_b", "dln2_b",
        )
    )
    units = all(
        np.all(i[k] == 1.0) for k in ("ln1_g", "ln2_g", "dln1_g", "dln2_g")
    )
    nc = _get_nc(simple=(zeros and units))

    shared = {
        "w1t": np.ascontiguousarray(i["enc_w1"].T),
        "b1": i["enc_b1"],
        "g1": i["ln1_g"],
        "be1": i["ln1_b"],
        "w2t": np.ascontiguousarray(i["enc_w2"].T),
        "b2": i["enc_b2"],
        "g2": i["ln2_g"],
        "be2": i["ln2_b"],
        "wmvt": np.ascontiguousarray(
            np.concatenate([i["mu_w"].T, i["lv_w"].T], axis=1)
        ),
        "bmv": np.concatenate([i["mu_b"], i["lv_b"]]),
        "ctxT": np.ascontiguousarray(i["ctx_mem"].T),
        "ctx": i["ctx_mem"],
        "wdit": np.ascontiguousarray(i["di_w"].T),
        "dib": i["di_b"],
        "wd1t": np.ascontiguousarray(i["dec_w1"].T),
        "db1": i["dec_b1"],
        "dg1": i["dln1_g"],
        "dbe1": i["dln1_b"],
        "wd2t": np.ascontiguousarray(i["dec_w2"].T),
        "db2": i["dec_b2"],
        "dg2": i["dln2_g"],
        "dbe2": i["dln2_b"],
        "wd3t": np.ascontiguousarray(i["dec_w3"].T),
        "db3": i["dec_b3"],
    }
    in_maps = []
    for c in range(N_CORES):
        m = dict(shared)
        m["x"] = i["x"][c * B_LOC : (c + 1) * B_LOC]
        m["eps"] = i["eps"][c * B_LOC : (c + 1) * B_LOC]
        in_maps.append(m)

    res = run_bass_kernel_spmd(nc, in_maps, core_ids=list(range(N_CORES)))
    recon = np.concatenate([r["recon"] for r in res.results], axis=0)
    mu = np.concatenate([r["mu"] for r in res.results], axis=0)
    lv = np.concatenate([r["lv"] for r in res.results], axis=0)
    return recon, mu, lv


# revision 35
# speedup vs baseline: 1.8496x; 1.2185x over previous
"""Trainium2 Bass kernel for EnhancedMLPDenoisingVAE.

Pure data parallel over 8 NeuronCores (4096 rows each). Activations are
batch-major ([128 batch rows on partitions, features free]); each matmul
consumes a PE-transposed copy of its input (features on partitions) as
the stationary operand:

    psum = xT.T @ W            K-tiled fp32r matmuls, fp32 accumulate
    LN stats from PSUM         bn_stats/bn_aggr on DVE
    h = Prelu(psum*rs - mu*rs) one wide ScalarE op per 512-chunk
                               (LN normalize + leaky-relu fused, 0.2)
    hT = PE-transpose(h)       fp32r transposes into shared 512-wide
                               PSUM groups, wide DVE copies out

When LN gamma/beta are not (1, 0) or a layer bias is nonzero (never the
case for this model's setup_inputs), per-layer fallbacks reproduce the
general math: bias is added via a broadcast tile on DVE, and gamma/beta
are applied per transposed chunk on ScalarE where they are per-partition
scalars.

Weights live in SBUF in three sequential phases (encoder / di+dec1 /
dec2+dec3); activations stage through DRAM between phases. fp32r
matmuls give ~4e-4 max rel err end to end; MM_DTYPE=float32 is the
full-precision fallback at 4x PE cost.
"""

from contextlib import ExitStack

import numpy as np

import concourse.bass as bass
import concourse.tile as tile
from concourse import bacc, mybir
from concourse.bass_utils import run_bass_kernel_spmd
from concourse.masks import make_identity

F32 = mybir.dt.float32
F32R = mybir.dt.float32r
AF = mybir.ActivationFunctionType
ALU = mybir.AluOpType

B, D, H, L, M = 32768, 768, 1024, 256, 32
N_CORES = 8
B_LOC = B // N_CORES  # 4096
P = 128
NT = B_LOC // P  # 32 row tiles per core
LN_EPS = 1e-5
ALPHA = 0.2

MM_DTYPE = F32R


def _chunks(nf, sz=512):
    return [(s, min(sz, nf - s)) for s in range(0, nf, sz)]


def _build_v2(simple=True, mm_dtype=None, wbufs=(2, 2, 2), psbufs=6, trbufs=2,
           sbufs=4, ORDER_A=None, ORDER_B=None, ORDER_C=None):
    ORDER_A = ORDER_A or [0, 1, 2, 3, 4, 5, 6]
    ORDER_B = ORDER_B or [4, 3, 2, 1, 0]
    ORDER_C = ORDER_C or [0, 1, 2, 3]
    """simple=True assumes all biases zero and LN gamma=1/beta=0 (true for
    this model's setup_inputs); simple=False emits the general math."""
    if isinstance(wbufs, int):
        wbufs = (wbufs, wbufs, wbufs)
    mmdt = MM_DTYPE if mm_dtype is None else mm_dtype
    nc = bacc.Bacc(
        "TRN2", target_bir_lowering=False, debug=False, num_devices=N_CORES
    )

    dram = lambda name, shape, dt=F32, kind="ExternalInput": nc.dram_tensor(
        name, shape, dt, kind=kind
    )
    x_d = dram("x", [D, B_LOC])
    eps_d = dram("eps", [B_LOC, L])
    w1_d = dram("w1t", [D, H])
    b1_d = dram("b1", [H])
    g1_d = dram("g1", [H])
    be1_d = dram("be1", [H])
    w2_d = dram("w2t", [H, H])
    b2_d = dram("b2", [H])
    g2_d = dram("g2", [H])
    be2_d = dram("be2", [H])
    wmv_d = dram("wmvt", [H, 2 * L])
    bmv_d = dram("bmv", [2 * L])
    ctxT_d = dram("ctxT", [L, M])
    ctx_d = dram("ctx", [M, L])
    wdi_d = dram("wdit", [L, H])
    dib_d = dram("dib", [H])
    wd1_d = dram("wd1t", [H, H])
    db1_d = dram("db1", [H])
    dg1_d = dram("dg1", [H])
    dbe1_d = dram("dbe1", [H])
    wd2_d = dram("wd2t", [H, 2 * H])
    db2_d = dram("db2", [2 * H])
    dg2_d = dram("dg2", [2 * H])
    dbe2_d = dram("dbe2", [2 * H])
    wd3_d = dram("wd3t", [2 * H, D])
    db3_d = dram("db3", [D])

    recon_d = dram("recon", [B_LOC, D], kind="ExternalOutput")
    mu_d = dram("mu", [B_LOC, L], kind="ExternalOutput")
    lv_d = dram("lv", [B_LOC, L], kind="ExternalOutput")

    with tile.TileContext(nc, pool_alloc_mode="queue") as tc, ExitStack() as glob:
        const = glob.enter_context(tc.tile_pool(name="const", bufs=1))
        dstash = glob.enter_context(
            tc.tile_pool(name="dstash", bufs=1, space="DRAM")
        )
        ident = const.tile([P, P], F32)
        make_identity(nc, ident)
        identr = const.tile([P, P], F32R)
        nc.vector.tensor_copy(identr[:], ident[:])
        epsln = const.tile([P, 1], F32)
        nc.vector.memset(epsln, LN_EPS)

        zenh_s = dstash.tile([NT, P, L // P, P], mmdt)
        d2t_s = dstash.tile([NT, P, 8, P], mmdt)
        wpB = glob.enter_context(tc.tile_pool(name="wB", bufs=1))
        prefetch = {}

        # ---------- helpers ----------
        def load_w(pool, dram_t, kt, nf, name):
            t = pool.tile([P, kt, nf], mmdt, name=name)
            nc.gpsimd.dma_start(
                t[:], dram_t.ap().rearrange("(kt p) n -> p kt n", p=P)
            )
            return t

        def load_bcast(pool, dram_t, nf, name):
            t = pool.tile([P, nf], F32, name=name)
            src = bass.AP(
                tensor=dram_t.ap().tensor, offset=0, ap=[[0, P], [1, nf]]
            )
            nc.gpsimd.dma_start(t[:], src)
            return t

        def load_packed(pool, dram_t, kt, name):
            t = pool.tile([P, kt], F32, name=name)
            nc.sync.dma_start(t[:], dram_t.ap().rearrange("(c p) -> p c", p=P))
            return t

        def mm_chunks(ps_pool, xT, w_sb, kt, tag):
            """Returns [(psum_tile, n0, nsz)] for all 512-chunks."""
            nf = w_sb.shape[2]
            out = []
            for ci, (n0, nsz) in enumerate(_chunks(nf)):
                ps = ps_pool.tile(
                    [P, 512], F32, name=f"ps_{tag}_{ci}", tag="mmps",
                    bufs=psbufs,
                )
                for k in range(kt):
                    nc.tensor.matmul(
                        ps[:, :nsz],
                        xT[:, k, :],
                        w_sb[:, k, n0 : n0 + nsz],
                        start=(k == 0),
                        stop=(k == kt - 1),
                    )
                out.append((ps, n0, nsz))
            return out

        def ln_stats(work, srcs, tag):
            """bn stats over chunk aps -> (negmu [P,1], mv [P,2])."""
            nsub = len(srcs)
            stats = work.tile(
                [P, nsub, 6], F32, name=f"st_{tag}", tag="stats", bufs=sbufs
            )
            for s, (src, n0, nsz) in enumerate(srcs):
                nc.vector.bn_stats(out=stats[:, s, :], in_=src[:, :nsz])
            mv = work.tile([P, 2], F32, name=f"mv_{tag}", tag="mv", bufs=sbufs)
            nc.vector.bn_aggr(out=mv[:], in_=stats[:])
            nmu = work.tile(
                [P, 1], F32, name=f"nmu_{tag}", tag="nmu", bufs=sbufs
            )
            nc.vector.tensor_scalar_mul(out=nmu[:], in0=mv[:, 0:1],
                                        scalar1=-1.0)
            return nmu, mv

        def rsqrt_dve(work, mv, tag):
            """rs = 1/sqrt(var + eps) via bit-trick + 2 Newton iters (DVE
            only -- keeps Sqrt off ScalarE so its LUT set never swaps)."""
            I32 = mybir.dt.int32
            v1 = work.tile([P, 1], F32, name=f"v1_{tag}", tag="v1",
                           bufs=sbufs)
            nc.vector.tensor_scalar_add(out=v1[:], in0=mv[:, 1:2],
                                        scalar1=LN_EPS)
            ti = work.tile([P, 1], I32, name=f"ti_{tag}", tag="ti",
                           bufs=sbufs)
            nc.vector.tensor_scalar(
                out=ti[:], in0=v1[:].bitcast(I32), scalar1=1, scalar2=None,
                op0=ALU.logical_shift_right,
            )
            nc.vector.tensor_scalar(
                out=ti[:], in0=ti[:], scalar1=-1, scalar2=0x5F3759DF,
                op0=ALU.mult, op1=ALU.add,
            )
            y = work.tile([P, 1], F32, name=f"yq_{tag}", tag="yq",
                          bufs=sbufs)
            nc.vector.tensor_copy(y[:], ti[:].bitcast(F32))
            hv = work.tile([P, 1], F32, name=f"hv_{tag}", tag="hv",
                           bufs=sbufs)
            nc.vector.tensor_scalar_mul(out=hv[:], in0=v1[:], scalar1=0.5)
            tq = work.tile([P, 1], F32, name=f"tq_{tag}", tag="tq",
                           bufs=sbufs)
            for _ in range(2):
                nc.vector.tensor_tensor(out=tq[:], in0=y[:], in1=y[:],
                                        op=ALU.mult)
                nc.vector.tensor_tensor(out=tq[:], in0=tq[:], in1=hv[:],
                                        op=ALU.mult)
                nc.vector.tensor_scalar(
                    out=tq[:], in0=tq[:], scalar1=-1.0, scalar2=1.5,
                    op0=ALU.mult, op1=ALU.add,
                )
                nc.vector.tensor_tensor(out=y[:], in0=y[:], in1=tq[:],
                                        op=ALU.mult)
            return y

        def transpose_in(work, ps_pool, src_sb, kt, out_dt, idt, tag,
                         grp=None):
            grp = grp or tag
            """src [P, kt*128] -> [P, kt, 128] via PE transposes grouped
            into 512-wide PSUM tiles + wide DVE copies."""
            out = work.tile(
                [P, kt, P], out_dt, name=f"t_{tag}", tag=f"t_{grp}",
                bufs=(sbufs if kt <= 8 else 2),
            )
            for g0 in range(0, kt, 4):
                gn = min(4, kt - g0)
                pw = ps_pool.tile(
                    [P, 512], F32, name=f"tw_{tag}_{g0}", tag="trps",
                    bufs=trbufs,
                )
                for j in range(gn):
                    dst = pw[:, j * P : (j + 1) * P]
                    src_c = src_sb[:, (g0 + j) * P : (g0 + j + 1) * P]
                    if src_c.dtype == F32R:
                        dst = dst.bitcast(F32R)
                    nc.tensor.transpose(dst, src_c, idt)
                nc.vector.tensor_copy(
                    out[:, g0 : g0 + gn, :].rearrange("p k c -> p (k c)"),
                    pw[:, : gn * P],
                )
            return out

        def dense_act(work, ps_tr, srcs, nf, tag, grp, *, ln, need_rs,
                      out_kt):
            """activate (LN shift + lrelu) + transpose half of a layer."""
            h = work.tile(
                [P, nf], mmdt, name=f"h_{tag}", tag=f"h{nf}", bufs=2
            )
            rs = None
            if ln:
                nmu, mv = ln_stats(work, srcs, tag)
                if need_rs:
                    rs = rsqrt_dve(work, mv, tag)
                for src, n0, nsz in srcs:
                    nc.scalar.activation(
                        out=h[:, n0 : n0 + nsz], in_=src[:, :nsz],
                        func=AF.Prelu, bias=nmu[:], scale=1.0, alpha=ALPHA,
                    )
            else:
                for src, n0, nsz in srcs:
                    nc.scalar.activation(
                        out=h[:, n0 : n0 + nsz], in_=src[:, :nsz],
                        func=AF.Prelu, bias=0.0, scale=1.0, alpha=ALPHA,
                    )
            tT = transpose_in(
                work, ps_tr, h, out_kt, mmdt,
                identr if mmdt == F32R else ident, f"{tag}T",
                grp=f"{grp}T",
            )
            return (tT, rs) if need_rs else tT

        def dense_fast(work, ps_mm, ps_tr, xT, w_sb, kt, tag, *,
                       ln, lrelu, out_kt, need_rs=False, grp=None):
            grp = grp or tag
            """simple-path layer: matmul -> (LN shift) -> lrelu ->
            transposed fp32r copy. The LN 1/std factor is NOT applied
            here: leaky-relu is positively homogeneous and LN is
            scale-invariant per sample, so the factor cancels through
            the next LN; layers feeding non-LN consumers get it back
            via need_rs (folded into the consumer's PSUM copy)."""
            srcs = mm_chunks(ps_mm, xT, w_sb, kt, tag)
            nf = w_sb.shape[2]
            h = work.tile(
                [P, nf], mmdt, name=f"h_{tag}", tag=f"h{nf}", bufs=2
            )
            rs = None
            if ln:
                nmu, mv = ln_stats(work, srcs, tag)
                if need_rs:
                    rs = rsqrt_dve(work, mv, tag)
                for src, n0, nsz in srcs:
                    nc.scalar.activation(
                        out=h[:, n0 : n0 + nsz], in_=src[:, :nsz],
                        func=AF.Prelu, bias=nmu[:], scale=1.0, alpha=ALPHA,
                    )
            else:
                assert lrelu
                for src, n0, nsz in srcs:
                    nc.scalar.activation(
                        out=h[:, n0 : n0 + nsz], in_=src[:, :nsz],
                        func=AF.Prelu, bias=0.0, scale=1.0, alpha=ALPHA,
                    )
            tT = transpose_in(
                work, ps_tr, h, out_kt, mmdt,
                identr if mmdt == F32R else ident, f"{tag}T",
                grp=f"{grp}T",
            )
            return (tT, rs) if need_rs else tT

        def dense_general(work, ps_mm, ps_tr, xT, w_sb, kt, tag, *,
                          ln, lrelu, out_kt, bias_bc, gp, bep, grp=None):
            grp = grp or tag
            """general-path layer (nonzero bias / non-unit gamma):
            y = psum + bias; xhat = (y-mu)*rs; transpose; per-chunk
            ScalarE Prelu(xhat*g + beta)."""
            srcs = mm_chunks(ps_mm, xT, w_sb, kt, tag)
            nf = w_sb.shape[2]
            y = work.tile([P, nf], F32, name=f"y_{tag}", tag=f"y{nf}", bufs=2)
            for src, n0, nsz in srcs:
                if bias_bc is not None:
                    nc.vector.tensor_tensor(
                        out=y[:, n0 : n0 + nsz], in0=src[:, :nsz],
                        in1=bias_bc[:, n0 : n0 + nsz], op=ALU.add,
                    )
                else:
                    nc.vector.tensor_copy(y[:, n0 : n0 + nsz], src[:, :nsz])
            xh = y
            if ln:
                nmu, mv = ln_stats(
                    work,
                    [(y[:, n0 : n0 + nsz], n0, nsz) for _, n0, nsz in srcs],
                    tag,
                )
                rs = rsqrt_dve(work, mv, tag)
                xh = work.tile(
                    [P, nf], F32, name=f"xh_{tag}", tag=f"xh{nf}", bufs=2
                )
                nc.vector.tensor_scalar(
                    out=xh[:], in0=y[:], scalar1=nmu[:], scalar2=rs[:],
                    op0=ALU.add, op1=ALU.mult,
                )
            out = work.tile(
                [P, out_kt, P], mmdt, name=f"t_{tag}", tag=f"t_{grp}",
                bufs=2,
            )
            for g0 in range(0, out_kt, 4):
                gn = min(4, out_kt - g0)
                pw = ps_tr.tile(
                    [P, 512], F32, name=f"tw_{tag}_{g0}", tag="trps",
                    bufs=trbufs,
                )
                for j in range(gn):
                    nc.tensor.transpose(
                        pw[:, j * P : (j + 1) * P],
                        xh[:, (g0 + j) * P : (g0 + j + 1) * P],
                        ident,
                    )
                for j in range(gn):
                    k = g0 + j
                    nc.scalar.activation(
                        out=out[:, k, :], in_=pw[:, j * P : (j + 1) * P],
                        func=AF.Prelu if (ln or lrelu) else AF.Identity,
                        bias=bep[:, k : k + 1] if bep is not None else 0.0,
                        scale=gp[:, k : k + 1] if gp is not None else 1.0,
                        alpha=ALPHA,
                    )
            return out

        def raw_out(work, srcs, nf, tag, bias_bc=None, row_scale=None,
                    grp=None):
            o = work.tile([P, nf], F32, name=f"o_{tag}", tag=f"o_{grp or tag}",
                          bufs=2)
            for src, n0, nsz in srcs:
                if bias_bc is not None:
                    nc.vector.tensor_tensor(
                        out=o[:, n0 : n0 + nsz], in0=src[:, :nsz],
                        in1=bias_bc[:, n0 : n0 + nsz], op=ALU.add,
                    )
                elif row_scale is not None:
                    nc.vector.tensor_scalar(
                        out=o[:, n0 : n0 + nsz], in0=src[:, :nsz],
                        scalar1=row_scale[:], scalar2=None, op0=ALU.mult,
                    )
                else:
                    nc.vector.tensor_copy(o[:, n0 : n0 + nsz], src[:, :nsz])
            return o

        def sw_pipeline(stage_fns, n, order=None):
            """Software-pipelined emission: the Tile scheduler is a
            priority-list scheduler, so per-engine execution order tracks
            emission order -- interleaving stages of neighboring row-tiles
            here is what lets PE run tile j+1 matmuls while tile j's
            LN/softmax chain is on DVE/ScalarE. `order` sets the
            intra-tick stage emission order (default deepest-first)."""
            S = len(stage_fns)
            if order is None:
                order = list(range(S - 1, -1, -1))
            states = [dict() for _ in range(n)]
            for t in range(n + S - 1):
                for s in order:
                    j = t - s
                    if 0 <= j < n:
                        stage_fns[s](j, states[j])

        # ================= PHASE A: encoder =================
        with ExitStack() as ph:
            wp = ph.enter_context(tc.tile_pool(name="wA", bufs=1))
            work = ph.enter_context(tc.tile_pool(name="workA", bufs=wbufs[0]))
            ps_mm = ph.enter_context(
                tc.tile_pool(name="psA", bufs=1, space="PSUM")
            )
            ps_tr = ph.enter_context(
                tc.tile_pool(name="psAt", bufs=1, space="PSUM")
            )
            w1 = load_w(wp, w1_d, D // P, H, "w1")
            w2 = load_w(wp, w2_d, H // P, H, "w2")
            wmv = load_w(wp, wmv_d, H // P, 2 * L, "wmv")
            ctxT = wp.tile([P, 2, M], F32, name="ctxT")
            nc.sync.dma_start(
                ctxT[:], ctxT_d.ap().rearrange("(kt p) n -> p kt n", p=P)
            )
            ctxm = wp.tile([M, L], mmdt, name="ctxm")
            nc.gpsimd.dma_start(ctxm[:], ctx_d.ap())
            if not simple:
                b1c = load_bcast(wp, b1_d, H, "b1c")
                b2c = load_bcast(wp, b2_d, H, "b2c")
                bmvc = load_bcast(wp, bmv_d, 2 * L, "bmvc")
                g1p = load_packed(wp, g1_d, H // P, "g1p")
                be1p = load_packed(wp, be1_d, H // P, "be1p")
                g2p = load_packed(wp, g2_d, H // P, "g2p")
                be2p = load_packed(wp, be2_d, H // P, "be2p")

            def sA0(i, st):
                if i == 2:
                    prefetch["wdi"] = load_w(wpB, wdi_d, L // P, H, "wdi")
                    prefetch["wd1"] = load_w(wpB, wd1_d, H // P, H, "wd1")
                r0 = i * P
                xT = work.tile([P, D // P, P], mmdt, name=f"xT_{i}",
                               tag="xT", bufs=sbufs)
                nc.gpsimd.dma_start(
                    xT[:],
                    x_d.ap().rearrange("(kt p) (nt c) -> p kt nt c", p=P,
                                       c=P)[:, :, i, :],
                )
                st["xT"] = xT

            def sA1a(i, st):
                if simple:
                    st["l1s"] = mm_chunks(ps_mm, st["xT"], w1, D // P,
                                          f"l1_{i}")
                else:
                    st["h1T"] = dense_general(
                        work, ps_mm, ps_tr, st["xT"], w1, D // P, f"l1_{i}",
                        ln=True, lrelu=True, out_kt=H // P,
                        bias_bc=b1c, gp=g1p, bep=be1p, grp="l1",
                    )

            def sA1b(i, st):
                if simple:
                    st["h1T"] = dense_act(
                        work, ps_tr, st.pop("l1s"), H, f"l1_{i}", "l1",
                        ln=True, need_rs=False, out_kt=H // P,
                    )

            def sA2a(i, st):
                if simple:
                    st["l2s"] = mm_chunks(ps_mm, st["h1T"], w2, H // P,
                                          f"l2_{i}")
                else:
                    st["h2T"] = dense_general(
                        work, ps_mm, ps_tr, st["h1T"], w2, H // P, f"l2_{i}",
                        ln=True, lrelu=True, out_kt=H // P,
                        bias_bc=b2c, gp=g2p, bep=be2p, grp="l2",
                    )
                    st["rs2"] = None

            def sA2b(i, st):
                if simple:
                    st["h2T"], st["rs2"] = dense_act(
                        work, ps_tr, st.pop("l2s"), H, f"l2_{i}", "l2",
                        ln=True, need_rs=True, out_kt=H // P,
                    )

            def sA3(i, st):
                r0 = i * P
                if simple:
                    smv = raw_out(
                        work, mm_chunks(ps_mm, st["h2T"], wmv, H // P,
                                        f"mv_{i}"),
                        2 * L, f"mv_{i}", row_scale=st["rs2"], grp="mv",
                    )
                else:
                    smv = raw_out(
                        work, mm_chunks(ps_mm, st["h2T"], wmv, H // P,
                                        f"mv_{i}"),
                        2 * L, f"mv_{i}", bias_bc=bmvc, grp="mv",
                    )
                nc.sync.dma_start(mu_d[r0 : r0 + P, :], smv[:, :L])
                nc.sync.dma_start(lv_d[r0 : r0 + P, :], smv[:, L:])

                elv = work.tile([P, L], F32, name=f"elv_{i}", tag="elv",
                                bufs=sbufs)
                nc.scalar.activation(
                    out=elv[:], in_=smv[:, L:], func=AF.Exp, bias=0.0,
                    scale=0.5,
                )
                eps_sb = work.tile([P, L], F32, name=f"eps_{i}", tag="eps",
                                   bufs=sbufs)
                nc.sync.dma_start(eps_sb[:], eps_d[r0 : r0 + P, :])
                z_sb = work.tile([P, L], F32, name=f"z_{i}", tag="z",
                                 bufs=sbufs)
                nc.vector.tensor_tensor(
                    out=z_sb[:], in0=elv[:], in1=eps_sb[:], op=ALU.mult
                )
                nc.vector.tensor_tensor(
                    out=z_sb[:], in0=z_sb[:], in1=smv[:, :L], op=ALU.add
                )
                st["z"] = z_sb

            def sA4(i, st):
                z_sb = st["z"]
                zT = transpose_in(work, ps_tr, z_sb, L // P, F32, ident,
                                  f"zT{i}", grp="zT")
                s_ps = ps_mm.tile([P, 512], F32, name=f"sps_{i}", tag="mmps",
                                  bufs=psbufs)
                for k in range(L // P):
                    nc.tensor.matmul(
                        s_ps[:, :M], zT[:, k, :], ctxT[:, k, :],
                        start=(k == 0), stop=(k == L // P - 1),
                    )
                negmx = work.tile([P, 1], F32, name=f"nmx_{i}", tag="nmx",
                                  bufs=sbufs)
                nc.vector.tensor_reduce(
                    out=negmx[:], in_=s_ps[:, :M],
                    axis=mybir.AxisListType.X, op=ALU.max, negate=True,
                )
                e_sb = work.tile([P, M], F32, name=f"e_{i}", tag="e",
                                 bufs=sbufs)
                se = work.tile([P, 1], F32, name=f"se_{i}", tag="se",
                               bufs=sbufs)
                nc.scalar.activation(
                    out=e_sb[:], in_=s_ps[:, :M], func=AF.Exp,
                    bias=negmx[:], scale=1.0, accum_out=se[:],
                )
                rs01 = work.tile([P, 1], F32, name=f"r01_{i}", tag="r01",
                                 bufs=sbufs)
                nc.vector.reciprocal(out=rs01[:], in_=se[:])
                nc.vector.tensor_scalar_mul(
                    out=rs01[:], in0=rs01[:], scalar1=0.1
                )
                # e_n = e * (0.1 / sum): fold attn normalization here so the
                # transposed context matmul needs no per-column scale
                nc.vector.tensor_scalar(
                    out=e_sb[:], in0=e_sb[:], scalar1=rs01[:], scalar2=None,
                    op0=ALU.mult,
                )
                trE = ps_tr.tile([P, 512], F32, name=f"trE_{i}", tag="trps",
                                 bufs=trbufs)
                nc.tensor.transpose(trE[:M, :P], e_sb[:], ident[:])
                eT = work.tile([M, P], mmdt, name=f"eT_{i}", tag="eT",
                               bufs=sbufs)
                nc.vector.tensor_copy(eT[:], trE[:M, :P])
                # z_addT[l_chunk, b] = ctx[:, l_chunk].T @ e_n.T  (fp32r)
                za_ps = ps_tr.tile([P, 512], F32, name=f"zaps_{i}",
                                   tag="trps", bufs=trbufs)
                for c in range(L // P):
                    nc.tensor.matmul(
                        za_ps[:, c * P : (c + 1) * P],
                        ctxm[:, c * P : (c + 1) * P],
                        eT[:],
                        start=True, stop=True,
                    )
                zeT = work.tile([P, L // P, P], mmdt, name=f"zeT_{i}",
                                tag="zeTA", bufs=sbufs)
                nc.vector.tensor_tensor(
                    out=zeT[:].rearrange("p k c -> p (k c)"),
                    in0=zT[:].rearrange("p k c -> p (k c)"),
                    in1=za_ps[:, :L],
                    op=ALU.add,
                )
                nc.sync.dma_start(zenh_s[i], zeT[:])
                st.clear()

            sw_pipeline([sA0, sA1a, sA1b, sA2a, sA2b, sA3, sA4], NT,
                        order=ORDER_A)

        # ================= PHASE B: di + dec1 =================
        wpC = glob.enter_context(tc.tile_pool(name="wC", bufs=1))
        with ExitStack() as ph:
            work = ph.enter_context(tc.tile_pool(name="workB", bufs=wbufs[1]))
            ps_mm = ph.enter_context(
                tc.tile_pool(name="psB", bufs=1, space="PSUM")
            )
            ps_tr = ph.enter_context(
                tc.tile_pool(name="psBt", bufs=1, space="PSUM")
            )
            wdi = prefetch["wdi"]
            wd1 = prefetch["wd1"]
            if not simple:
                dibp = load_packed(wpB, dib_d, H // P, "dibp")
                db1c = load_bcast(wpB, db1_d, H, "db1c")
                dg1p = load_packed(wpB, dg1_d, H // P, "dg1p")
                dbe1p = load_packed(wpB, dbe1_d, H // P, "dbe1p")

            def sB0(i, st):
                if i == 2:
                    prefetch["wd2"] = load_w(wpC, wd2_d, H // P, 2 * H, "wd2")
                    prefetch["wd3"] = load_w(wpC, wd3_d, 2 * H // P, D, "wd3")
                zeT = work.tile([P, L // P, P], mmdt, name=f"zeB_{i}",
                                tag="zeB", bufs=sbufs)
                nc.sync.dma_start(zeT[:], zenh_s[i])
                st["zeT"] = zeT

            def sB1a(i, st):
                if simple:
                    st["dis"] = mm_chunks(ps_mm, st["zeT"], wdi, L // P,
                                          f"di_{i}")
                else:
                    st["d1T"] = dense_general(
                        work, ps_mm, ps_tr, st["zeT"], wdi, L // P, f"di_{i}",
                        ln=False, lrelu=True, out_kt=H // P,
                        bias_bc=None, gp=None, bep=dibp, grp="di",
                    )

            def sB1b(i, st):
                if simple:
                    st["d1T"] = dense_act(
                        work, ps_tr, st.pop("dis"), H, f"di_{i}", "di",
                        ln=False, need_rs=False, out_kt=H // P,
                    )

            def sB2a(i, st):
                if simple:
                    st["d1s"] = mm_chunks(ps_mm, st["d1T"], wd1, H // P,
                                          f"d1_{i}")
                else:
                    d2T = dense_general(
                        work, ps_mm, ps_tr, st["d1T"], wd1, H // P, f"d1_{i}",
                        ln=True, lrelu=True, out_kt=H // P,
                        bias_bc=db1c, gp=dg1p, bep=dbe1p, grp="d1",
                    )
                    nc.sync.dma_start(d2t_s[i], d2T[:])
                    st.clear()

            def sB2b(i, st):
                if simple:
                    d2T = dense_act(
                        work, ps_tr, st.pop("d1s"), H, f"d1_{i}", "d1",
                        ln=True, need_rs=False, out_kt=H // P,
                    )
                    nc.sync.dma_start(d2t_s[i], d2T[:])
                    st.clear()

            sw_pipeline([sB0, sB1a, sB1b, sB2a, sB2b], NT,
                        order=ORDER_B)

        # ================= PHASE C: dec2 + dec3 =================
        with ExitStack() as ph:
            work = ph.enter_context(tc.tile_pool(name="workC", bufs=wbufs[2]))
            ps_mm = ph.enter_context(
                tc.tile_pool(name="psC", bufs=1, space="PSUM")
            )
            ps_tr = ph.enter_context(
                tc.tile_pool(name="psCt", bufs=1, space="PSUM")
            )
            wd2 = prefetch["wd2"]
            wd3 = prefetch["wd3"]
            if not simple:
                db2c = load_bcast(wpC, db2_d, 2 * H, "db2c")
                dg2p = load_packed(wpC, dg2_d, 2 * H // P, "dg2p")
                dbe2p = load_packed(wpC, dbe2_d, 2 * H // P, "dbe2p")
                db3c = load_bcast(wpC, db3_d, D, "db3c")

            def sC0(i, st):
                d2T = work.tile([P, 8, P], mmdt, name=f"d2C_{i}", tag="d2C",
                                bufs=3)
                nc.sync.dma_start(d2T[:], d2t_s[i])
                st["d2T"] = d2T

            def sC1a(i, st):
                if simple:
                    st["d2s"] = mm_chunks(ps_mm, st["d2T"], wd2, H // P,
                                          f"d2_{i}")
                else:
                    st["d3T"] = dense_general(
                        work, ps_mm, ps_tr, st["d2T"], wd2, H // P, f"d2_{i}",
                        ln=True, lrelu=True, out_kt=2 * H // P,
                        bias_bc=db2c, gp=dg2p, bep=dbe2p, grp="d2",
                    )
                    st["rs4"] = None

            def sC1b(i, st):
                if simple:
                    st["d3T"], st["rs4"] = dense_act(
                        work, ps_tr, st.pop("d2s"), 2 * H, f"d2_{i}", "d2",
                        ln=True, need_rs=True, out_kt=2 * H // P,
                    )

            def sC2(i, st):
                r0 = i * P
                if simple:
                    recon_sb = raw_out(
                        work,
                        mm_chunks(ps_mm, st["d3T"], wd3, 2 * H // P,
                                  f"d3_{i}"),
                        D, f"d3_{i}", row_scale=st["rs4"], grp="d3",
                    )
                else:
                    recon_sb = raw_out(
                        work,
                        mm_chunks(ps_mm, st["d3T"], wd3, 2 * H // P,
                                  f"d3_{i}"),
                        D, f"d3_{i}", bias_bc=db3c, grp="d3",
                    )
                nc.sync.dma_start(recon_d[r0 : r0 + P, :], recon_sb[:])
                st.clear()

            sw_pipeline([sC0, sC1a, sC1b, sC2], NT, order=ORDER_C)

    nc.finalize()
    return nc


NB = 256  # batch columns per super-tile (moving-dim of fp32r matmuls)
NST = B_LOC // NB  # 16 super-tiles per core


def _build_v3(psbufs=4, trbufs=3, hbufs=2, sq_on_act=True,
              ORDER_A=None, ORDER_B=None, ORDER_C=None):
    """Feature-major dataflow: activations live transposed ([feature
    chunk on partitions, batch free]) end to end, weights are the
    stationary matmul operand, so no PE transposes of activations are
    needed. LN mean-subtraction is folded into host-centered weights
    (W' = W - mean_out(W)); the LN 1/std factor cancels through LN->LN
    chains (leaky-relu is positively homogeneous) and is only computed
    for ln2/dln2 via a ones-matmul over ScalarE-squared chunks, then
    applied per batch column through a PE-broadcast row. Assumes zero
    biases / unit gammas (checked by kernel())."""
    ORDER_A = ORDER_A or [0, 1, 2, 3, 4]
    ORDER_B = ORDER_B or [0, 1, 2]
    ORDER_C = ORDER_C or [0, 1, 2]
    mmdt = F32R
    nc = bacc.Bacc(
        "TRN2", target_bir_lowering=False, debug=False, num_devices=N_CORES
    )
    dram = lambda name, shape, dt=F32, kind="ExternalInput": nc.dram_tensor(
        name, shape, dt, kind=kind
    )
    x_d = dram("x", [D, B_LOC])        # host-transposed
    eps_d = dram("eps", [L, B_LOC])    # host-transposed
    w1_d = dram("w1t", [D, H])         # host-centered
    w2_d = dram("w2t", [H, H])         # host-centered
    wmv_d = dram("wmvt", [H, 2 * L])
    ctxT_d = dram("ctxT", [L, M])
    ctx_d = dram("ctx", [M, L])
    wdi_d = dram("wdit", [L, H])
    wd1_d = dram("wd1t", [H, H])       # host-centered
    wd2_d = dram("wd2t", [H, 2 * H])   # host-centered
    wd3_d = dram("wd3t", [2 * H, D])
    recon_d = dram("recon", [D, B_LOC], kind="ExternalOutput")  # host .T
    mu_d = dram("mu", [L, B_LOC], kind="ExternalOutput")        # host .T
    lv_d = dram("lv", [L, B_LOC], kind="ExternalOutput")        # host .T

    with tile.TileContext(nc) as tc, ExitStack() as glob:
        const = glob.enter_context(tc.tile_pool(name="const", bufs=1))
        dstash = glob.enter_context(
            tc.tile_pool(name="dstash", bufs=1, space="DRAM")
        )
        ident = const.tile([P, P], F32)
        make_identity(nc, ident)
        ones_f = const.tile([P, 1], F32)
        nc.vector.memset(ones_f, 1.0)
        onesr = const.tile([P, 1], F32R)
        nc.vector.tensor_copy(onesr[:], ones_f[:])
        ones_rf = const.tile([1, P], F32)
        nc.vector.memset(ones_rf, 1.0)
        ones_row = const.tile([1, P], F32R)
        nc.vector.tensor_copy(ones_row[:], ones_rf[:])

        zenh_s = dstash.tile([NST, P, L // P, NB], mmdt)
        d2t_s = dstash.tile([NST, P, H // P, NB], mmdt)

        wrapB = glob.enter_context(ExitStack())
        wpB = wrapB.enter_context(
            tc.tile_pool(name="wB", bufs=1, side="right")
        )
        prefetch = {}

        def load_w(pool, dram_t, kt, nf, name):
            # one DMA per K-chunk so matmuls can start before the whole
            # weight tile has landed
            t = pool.tile([P, kt, nf], mmdt, name=name)
            src_ap = dram_t.ap().rearrange("(kt p) n -> p kt n", p=P)
            for k in range(kt):
                nc.gpsimd.dma_start(t[:, k : k + 1, :], src_ap[:, k : k + 1, :])
            return t

        def sw_pipeline(stage_fns, n, order):
            S = len(stage_fns)
            states = [dict() for _ in range(n)]
            for t in range(n + S - 1):
                for s in order:
                    j = t - s
                    if 0 <= j < n:
                        stage_fns[s](j, states[j])

        def mmF(ps_pool, w_sb, xT, kt, out_kt, tag):
            """feature-major layer: psum chunk m = sum_k W[:,k,m].T@xT[:,k].
            Two 256-wide chunks share one 512-wide PSUM tile (bank)."""
            chunks = []
            for mp in range((out_kt + 1) // 2):
                ps = ps_pool.tile([P, 512], F32, name=f"ps_{tag}_{mp}",
                                  tag="mmps", bufs=psbufs)
                for sub in range(min(2, out_kt - 2 * mp)):
                    m = 2 * mp + sub
                    pslice = ps[:, sub * NB : (sub + 1) * NB]
                    for k in range(kt):
                        nc.tensor.matmul(
                            pslice,
                            w_sb[:, k, m * P : (m + 1) * P],
                            xT[:, k, :],
                            start=(k == 0),
                            stop=(k == kt - 1),
                        )
                    chunks.append(pslice)
            return chunks

        def act_lrelu(work, chunks, out_kt, tag, grp):
            h = work.tile([P, out_kt, NB], mmdt, name=f"h_{tag}",
                          tag=f"h_{grp}", bufs=hbufs)
            for m, ps in enumerate(chunks):
                nc.scalar.activation(
                    out=h[:, m, :], in_=ps[:], func=AF.Prelu, bias=0.0,
                    scale=1.0, alpha=ALPHA,
                )
            return h

        def rs_row(work, ps_q, chunks, nf, tag):
            """rs = 1/sqrt(mean(y_c^2)+eps) per batch column, PE-broadcast
            to [P, NB] in SBUF (f32). chunks are this layer's psums."""
            q_ps = ps_q.tile([1, NB], F32, name=f"q_{tag}", tag="qps",
                             bufs=1)
            for m, ps in enumerate(chunks):
                sq = work.tile([P, NB], F32R, name=f"sq_{tag}_{m}", tag="sq",
                               bufs=2)
                if sq_on_act:
                    nc.scalar.activation(out=sq[:], in_=ps[:],
                                         func=AF.Square, bias=0.0, scale=1.0)
                else:
                    nc.vector.tensor_tensor(out=sq[:], in0=ps[:], in1=ps[:],
                                            op=ALU.mult)
                nc.tensor.matmul(
                    q_ps[:], onesr[:], sq[:],
                    start=(m == 0), stop=(m == len(chunks) - 1),
                )
            I32 = mybir.dt.int32
            v1 = work.tile([1, NB], F32, name=f"v1_{tag}", tag="v1", bufs=2)
            nc.vector.tensor_scalar(
                out=v1[:], in0=q_ps[:], scalar1=1.0 / nf, scalar2=LN_EPS,
                op0=ALU.mult, op1=ALU.add,
            )
            ti = work.tile([1, NB], I32, name=f"ti_{tag}", tag="ti", bufs=2)
            nc.vector.tensor_scalar(
                out=ti[:], in0=v1[:].bitcast(I32), scalar1=1, scalar2=None,
                op0=ALU.logical_shift_right,
            )
            nc.vector.tensor_scalar(
                out=ti[:], in0=ti[:], scalar1=-1, scalar2=0x5F3759DF,
                op0=ALU.mult, op1=ALU.add,
            )
            y = work.tile([1, NB], F32, name=f"yq_{tag}", tag="yq", bufs=2)
            nc.vector.tensor_copy(y[:], ti[:].bitcast(F32))
            hv = work.tile([1, NB], F32, name=f"hv_{tag}", tag="hv", bufs=2)
            nc.vector.tensor_scalar_mul(out=hv[:], in0=v1[:], scalar1=0.5)
            tq = work.tile([1, NB], F32, name=f"tq_{tag}", tag="tq", bufs=2)
            for _ in range(2):
                nc.vector.tensor_tensor(out=tq[:], in0=y[:], in1=y[:],
                                        op=ALU.mult)
                nc.vector.tensor_tensor(out=tq[:], in0=tq[:], in1=hv[:],
                                        op=ALU.mult)
                nc.vector.tensor_scalar(
                    out=tq[:], in0=tq[:], scalar1=-1.0, scalar2=1.5,
                    op0=ALU.mult, op1=ALU.add,
                )
                nc.vector.tensor_tensor(out=y[:], in0=y[:], in1=tq[:],
                                        op=ALU.mult)
            yr = work.tile([1, NB], F32R, name=f"yr_{tag}", tag="yr", bufs=2)
            nc.vector.tensor_copy(yr[:], y[:])
            rb_ps = ps_q.tile([P, NB], F32, name=f"rb_{tag}", tag="rbps",
                              bufs=1)
            nc.tensor.matmul(rb_ps[:], ones_row[:], yr[:], start=True,
                             stop=True)
            rsb = work.tile([P, NB], F32, name=f"rsb_{tag}", tag="rsb",
                            bufs=3)
            nc.vector.tensor_copy(rsb[:], rb_ps[:])
            return rsb

        # ================= PHASE A =================
        with ExitStack() as ph:
            wp = ph.enter_context(tc.tile_pool(name="wA", bufs=1))
            work = ph.enter_context(tc.tile_pool(name="workA", bufs=2))
            ps_mm = ph.enter_context(
                tc.tile_pool(name="psA", bufs=1, space="PSUM")
            )
            ps_x = ph.enter_context(
                tc.tile_pool(name="psAx", bufs=1, space="PSUM")
            )
            w1 = load_w(wp, w1_d, D // P, H, "w1")
            w2 = load_w(wp, w2_d, H // P, H, "w2")
            wmv = load_w(wp, wmv_d, H // P, 2 * L, "wmv")
            ctxT = wp.tile([P, 2, M], F32, name="ctxT")
            nc.sync.dma_start(
                ctxT[:], ctxT_d.ap().rearrange("(kt p) n -> p kt n", p=P)
            )
            ctxm = wp.tile([M, L], mmdt, name="ctxm")
            nc.gpsimd.dma_start(ctxm[:], ctx_d.ap())

            def sA0(i, st):
                if i == 1:
                    prefetch["wdi"] = load_w(wpB, wdi_d, L // P, H, "wdi")
                    prefetch["wd1"] = load_w(wpB, wd1_d, H // P, H, "wd1")
                xT = work.tile([P, D // P, NB], mmdt, name=f"xT_{i}",
                               tag="xT", bufs=2)
                nc.gpsimd.dma_start(
                    xT[:],
                    x_d.ap().rearrange("(kt p) (nt c) -> p kt nt c", p=P,
                                       c=NB)[:, :, i, :],
                )
                st["xT"] = xT

            def sA1(i, st):
                st["h1T"] = act_lrelu(
                    work, mmF(ps_mm, w1, st.pop("xT"), D // P, H // P,
                              f"l1_{i}"),
                    H // P, f"l1_{i}", "l1",
                )

            def sA2(i, st):
                chunks = mmF(ps_mm, w2, st.pop("h1T"), H // P, H // P,
                             f"l2_{i}")
                st["h2T"] = act_lrelu(work, chunks, H // P, f"l2_{i}", "l2")
                st["rsb2"] = rs_row(work, ps_x, chunks, H, f"l2_{i}")

            def sA3(i, st):
                chunks = mmF(ps_mm, wmv, st.pop("h2T"), H // P,
                             2 * L // P, f"mv_{i}")
                rsb2 = st.pop("rsb2")
                smv = work.tile([P, 2 * L // P, NB], F32, name=f"smv_{i}",
                                tag="smv", bufs=2)
                for m, ps in enumerate(chunks):
                    nc.vector.tensor_tensor(
                        out=smv[:, m, :], in0=ps[:], in1=rsb2[:],
                        op=ALU.mult,
                    )
                mu_ap = mu_d.ap().rearrange(
                    "(c p) (nt b) -> p c nt b", p=P, b=NB
                )[:, :, i, :]
                lv_ap = lv_d.ap().rearrange(
                    "(c p) (nt b) -> p c nt b", p=P, b=NB
                )[:, :, i, :]
                nc.sync.dma_start(mu_ap, smv[:, 0 : L // P, :])
                nc.sync.dma_start(lv_ap, smv[:, L // P :, :])

                elv = work.tile([P, L // P, NB], F32, name=f"elv_{i}",
                                tag="elv", bufs=1)
                for c in range(L // P):
                    nc.scalar.activation(
                        out=elv[:, c, :], in_=smv[:, L // P + c, :],
                        func=AF.Exp, bias=0.0, scale=0.5,
                    )
                epsT = work.tile([P, L // P, NB], F32, name=f"epsT_{i}",
                                 tag="epsT", bufs=2)
                nc.sync.dma_start(
                    epsT[:],
                    eps_d.ap().rearrange("(kt p) (nt c) -> p kt nt c", p=P,
                                         c=NB)[:, :, i, :],
                )
                zT = work.tile([P, L // P, NB], F32, name=f"zT_{i}",
                               tag="zT", bufs=2)
                nc.vector.tensor_tensor(
                    out=zT[:].rearrange("p k c -> p (k c)"),
                    in0=elv[:].rearrange("p k c -> p (k c)"),
                    in1=epsT[:].rearrange("p k c -> p (k c)"),
                    op=ALU.mult,
                )
                nc.vector.tensor_tensor(
                    out=zT[:].rearrange("p k c -> p (k c)"),
                    in0=zT[:].rearrange("p k c -> p (k c)"),
                    in1=smv[:, 0 : L // P, :].rearrange("p k c -> p (k c)"),
                    op=ALU.add,
                )
                st["zT"] = zT

            def sA4(i, st):
                zT = st.pop("zT")
                eT = work.tile([M, NB], mmdt, name=f"eT_{i}", tag="eT",
                               bufs=2)
                for bc in range(NB // P):
                    s_ps = ps_x.tile([P, 512], F32, name=f"sps_{i}_{bc}",
                                     tag="sps", bufs=2)
                    for k in range(L // P):
                        nc.tensor.matmul(
                            s_ps[:, :M],
                            zT[:, k, bc * P : (bc + 1) * P],
                            ctxT[:, k, :],
                            start=(k == 0), stop=(k == L // P - 1),
                        )
                    negmx = work.tile([P, 1], F32, name=f"nmx_{i}_{bc}",
                                      tag="nmx", bufs=4)
                    nc.vector.tensor_reduce(
                        out=negmx[:], in_=s_ps[:, :M],
                        axis=mybir.AxisListType.X, op=ALU.max, negate=True,
                    )
                    e_sb = work.tile([P, M], F32, name=f"e_{i}_{bc}",
                                     tag="e", bufs=4)
                    se = work.tile([P, 1], F32, name=f"se_{i}_{bc}",
                                   tag="se", bufs=4)
                    nc.scalar.activation(
                        out=e_sb[:], in_=s_ps[:, :M], func=AF.Exp,
                        bias=negmx[:], scale=1.0, accum_out=se[:],
                    )
                    rs01 = work.tile([P, 1], F32, name=f"r01_{i}_{bc}",
                                     tag="r01", bufs=4)
                    nc.vector.reciprocal(out=rs01[:], in_=se[:])
                    nc.vector.tensor_scalar_mul(out=rs01[:], in0=rs01[:],
                                                scalar1=0.1)
                    nc.vector.tensor_scalar(
                        out=e_sb[:], in0=e_sb[:], scalar1=rs01[:],
                        scalar2=None, op0=ALU.mult,
                    )
                    trE = ps_x.tile([P, 512], F32, name=f"trE_{i}_{bc}",
                                    tag="sps", bufs=2)
                    nc.tensor.transpose(trE[:M, :P], e_sb[:], ident[:])
                    nc.vector.tensor_copy(
                        eT[:, bc * P : (bc + 1) * P], trE[:M, :P]
                    )
                za_ps = ps_x.tile([P, 2, NB], F32, name=f"za_{i}",
                                  tag="sps", bufs=2)
                for c in range(L // P):
                    nc.tensor.matmul(
                        za_ps[:, c, :],
                        ctxm[:, c * P : (c + 1) * P],
                        eT[:],
                        start=True, stop=True,
                    )
                zeT = work.tile([P, L // P, NB], mmdt, name=f"zeT_{i}",
                                tag="zeT", bufs=2)
                nc.vector.tensor_tensor(
                    out=zeT[:].rearrange("p k c -> p (k c)"),
                    in0=zT[:].rearrange("p k c -> p (k c)"),
                    in1=za_ps[:].rearrange("p k c -> p (k c)"),
                    op=ALU.add,
                )
                nc.sync.dma_start(zenh_s[i], zeT[:])
                st.clear()

            sw_pipeline([sA0, sA1, sA2, sA3, sA4], NST, ORDER_A)

        # ================= PHASE B =================
        wpC = glob.enter_context(tc.tile_pool(name="wC", bufs=1))
        with ExitStack() as ph:
            work = ph.enter_context(tc.tile_pool(name="workB", bufs=2))
            ps_mm = ph.enter_context(
                tc.tile_pool(name="psB", bufs=1, space="PSUM")
            )
            wdi = prefetch["wdi"]
            wd1 = prefetch["wd1"]

            def sB0(i, st):
                if i == 1:
                    prefetch["wd2"] = load_w(wpC, wd2_d, H // P, 2 * H,
                                             "wd2")
                zeT = work.tile([P, L // P, NB], mmdt, name=f"zeB_{i}",
                                tag="zeB", bufs=3)
                nc.sync.dma_start(zeT[:], zenh_s[i])
                st["zeT"] = zeT

            def sB1(i, st):
                st["d1T"] = act_lrelu(
                    work, mmF(ps_mm, wdi, st.pop("zeT"), L // P, H // P,
                              f"di_{i}"),
                    H // P, f"di_{i}", "di",
                )

            def sB2(i, st):
                d2T = act_lrelu(
                    work, mmF(ps_mm, wd1, st.pop("d1T"), H // P, H // P,
                              f"d1_{i}"),
                    H // P, f"d1_{i}", "d1",
                )
                nc.sync.dma_start(d2t_s[i], d2T[:])
                st.clear()

            sw_pipeline([sB0, sB1, sB2], NST, ORDER_B)
        wrapB.close()  # free the wB weight range before phase C work pool

        # ================= PHASE C =================
        wpC2 = glob.enter_context(tc.tile_pool(name="wC2", bufs=1))
        prefetch["wd3"] = load_w(wpC2, wd3_d, 2 * H // P, D, "wd3")
        with ExitStack() as ph:
            work = ph.enter_context(tc.tile_pool(name="workC", bufs=2))
            ps_mm = ph.enter_context(
                tc.tile_pool(name="psC", bufs=1, space="PSUM")
            )
            ps_x = ph.enter_context(
                tc.tile_pool(name="psCx", bufs=1, space="PSUM")
            )
            wd2 = prefetch["wd2"]
            wd3 = prefetch["wd3"]

            def sC0(i, st):
                d2T = work.tile([P, H // P, NB], mmdt, name=f"d2C_{i}",
                                tag="d2C", bufs=3)
                nc.sync.dma_start(d2T[:], d2t_s[i])
                st["d2T"] = d2T

            def sC1(i, st):
                chunks = mmF(ps_mm, wd2, st.pop("d2T"), H // P,
                             2 * H // P, f"d2_{i}")
                st["d3T"] = act_lrelu(work, chunks, 2 * H // P, f"d2_{i}",
                                      "d2")
                st["rsb4"] = rs_row(work, ps_x, chunks, 2 * H, f"d2_{i}")

            def sC2(i, st):
                chunks = mmF(ps_mm, wd3, st.pop("d3T"), 2 * H // P,
                             D // P, f"d3_{i}")
                rsb4 = st.pop("rsb4")
                recon = work.tile([P, D // P, NB], F32, name=f"rec_{i}",
                                  tag="rec", bufs=2)
                for m, ps in enumerate(chunks):
                    nc.vector.tensor_tensor(
                        out=recon[:, m, :], in0=ps[:], in1=rsb4[:],
                        op=ALU.mult,
                    )
                rec_ap = recon_d.ap().rearrange(
                    "(c p) (nt b) -> p c nt b", p=P, b=NB
                )[:, :, i, :]
                nc.sync.dma_start(rec_ap, recon[:])
                st.clear()

            sw_pipeline([sC0, sC1, sC2], NST, ORDER_C)

    nc.finalize()
    return nc


_NC_CACHE = {}


def _get_nc(simple=True):
    key = ("simple" if simple else "general", str(MM_DTYPE))
    if key not in _NC_CACHE:
        _NC_CACHE[key] = (
            _build_v3() if simple else _build_v2(simple=False)
        )
    return _NC_CACHE[key]


def kernel(**inputs):
    i = {
        k: np.ascontiguousarray(np.asarray(v, dtype=np.float32))
        for k, v in inputs.items()
    }
    zeros = all(
        not np.any(i[k])
        for k in (
            "enc_b1", "enc_b2", "mu_b", "lv_b", "di_b", "dec_b1", "dec_b2",
            "dec_b3", "ln1_b", "ln2_b", "dln1_b", "dln2_b",
        )
    )
    units = all(
        np.all(i[k] == 1.0) for k in ("ln1_g", "ln2_g", "dln1_g", "dln2_g")
    )
    simple = zeros and units
    nc = _get_nc(simple=simple)

    def _ct(w):  # transpose + center over out-features (folds LN mean)
        wt = np.ascontiguousarray(w.T)
        return wt - wt.mean(axis=1, keepdims=True)

    shared = {
        "w1t": _ct(i["enc_w1"]) if simple
        else np.ascontiguousarray(i["enc_w1"].T),
        "b1": i["enc_b1"],
        "g1": i["ln1_g"],
        "be1": i["ln1_b"],
        "w2t": _ct(i["enc_w2"]) if simple
        else np.ascontiguousarray(i["enc_w2"].T),
        "b2": i["enc_b2"],
        "g2": i["ln2_g"],
        "be2": i["ln2_b"],
        "wmvt": np.ascontiguousarray(
            np.concatenate([i["mu_w"].T, i["lv_w"].T], axis=1)
        ),
        "bmv": np.concatenate([i["mu_b"], i["lv_b"]]),
        "ctxT": np.ascontiguousarray(i["ctx_mem"].T),
        "ctx": i["ctx_mem"],
        "wdit": np.ascontiguousarray(i["di_w"].T),
        "dib": i["di_b"],
        "wd1t": _ct(i["dec_w1"]) if simple
        else np.ascontiguousarray(i["dec_w1"].T),
        "db1": i["dec_b1"],
        "dg1": i["dln1_g"],
        "dbe1": i["dln1_b"],
        "wd2t": _ct(i["dec_w2"]) if simple
        else np.ascontiguousarray(i["dec_w2"].T),
        "db2": i["dec_b2"],
        "dg2": i["dln2_g"],
        "dbe2": i["dln2_b"],
        "wd3t": np.ascontiguousarray(i["dec_w3"].T),
        "db3": i["dec_b3"],
    }
    in_names = {
        alloc.memorylocations[0].name
        for alloc in nc.m.functions[0].allocations
        if isinstance(alloc, mybir.MemoryLocationSet)
        and alloc.kind == "ExternalInput"
    }
    shared = {k: v for k, v in shared.items() if k in in_names}
    in_maps = []
    for c in range(N_CORES):
        m = dict(shared)
        xc = i["x"][c * B_LOC : (c + 1) * B_LOC]
        ec = i["eps"][c * B_LOC : (c + 1) * B_LOC]
        m["x"] = np.ascontiguousarray(xc.T)
        m["eps"] = np.ascontiguousarray(ec.T) if simple else ec
        in_maps.append(m)

    res = run_bass_kernel_spmd(nc, in_maps, core_ids=list(range(N_CORES)))
    if simple:
        recon = np.concatenate(
            [r["recon"].T for r in res.results], axis=0
        )
        mu = np.concatenate([r["mu"].T for r in res.results], axis=0)
        lv = np.concatenate([r["lv"].T for r in res.results], axis=0)
    else:
        recon = np.concatenate([r["recon"] for r in res.results], axis=0)
        mu = np.concatenate([r["mu"] for r in res.results], axis=0)
        lv = np.concatenate([r["lv"] for r in res.results], axis=0)
    return recon, mu, lv


# revision 44
# speedup vs baseline: 1.9457x; 1.0519x over previous
"""Trainium2 Bass kernel for EnhancedMLPDenoisingVAE.

Pure data parallel over 8 NeuronCores (4096 rows each). Activations are
batch-major ([128 batch rows on partitions, features free]); each matmul
consumes a PE-transposed copy of its input (features on partitions) as
the stationary operand:

    psum = xT.T @ W            K-tiled fp32r matmuls, fp32 accumulate
    LN stats from PSUM         bn_stats/bn_aggr on DVE
    h = Prelu(psum*rs - mu*rs) one wide ScalarE op per 512-chunk
                               (LN normalize + leaky-relu fused, 0.2)
    hT = PE-transpose(h)       fp32r transposes into shared 512-wide
                               PSUM groups, wide DVE copies out

When LN gamma/beta are not (1, 0) or a layer bias is nonzero (never the
case for this model's setup_inputs), per-layer fallbacks reproduce the
general math: bias is added via a broadcast tile on DVE, and gamma/beta
are applied per transposed chunk on ScalarE where they are per-partition
scalars.

Weights live in SBUF in three sequential phases (encoder / di+dec1 /
dec2+dec3); activations stage through DRAM between phases. fp32r
matmuls give ~4e-4 max rel err end to end; MM_DTYPE=float32 is the
full-precision fallback at 4x PE cost.
"""

from contextlib import ExitStack

import numpy as np

import concourse.bass as bass
import concourse.tile as tile
from concourse import bacc, mybir
from concourse.bass_utils import run_bass_kernel_spmd
from concourse.masks import make_identity

F32 = mybir.dt.float32
F32R = mybir.dt.float32r
AF = mybir.ActivationFunctionType
ALU = mybir.AluOpType

B, D, H, L, M = 32768, 768, 1024, 256, 32
N_CORES = 8
B_LOC = B // N_CORES  # 4096
P = 128
NT = B_LOC // P  # 32 row tiles per core
LN_EPS = 1e-5
ALPHA = 0.2

MM_DTYPE = F32R


def _chunks(nf, sz=512):
    return [(s, min(sz, nf - s)) for s in range(0, nf, sz)]


def _build_v2(simple=True, mm_dtype=None, wbufs=(2, 2, 2), psbufs=6, trbufs=2,
           sbufs=4, ORDER_A=None, ORDER_B=None, ORDER_C=None):
    ORDER_A = ORDER_A or [0, 1, 2, 3, 4, 5, 6]
    ORDER_B = ORDER_B or [4, 3, 2, 1, 0]
    ORDER_C = ORDER_C or [0, 1, 2, 3]
    """simple=True assumes all biases zero and LN gamma=1/beta=0 (true for
    this model's setup_inputs); simple=False emits the general math."""
    if isinstance(wbufs, int):
        wbufs = (wbufs, wbufs, wbufs)
    mmdt = MM_DTYPE if mm_dtype is None else mm_dtype
    nc = bacc.Bacc(
        "TRN2", target_bir_lowering=False, debug=False, num_devices=N_CORES
    )

    dram = lambda name, shape, dt=F32, kind="ExternalInput": nc.dram_tensor(
        name, shape, dt, kind=kind
    )
    x_d = dram("x", [D, B_LOC])
    eps_d = dram("eps", [B_LOC, L])
    w1_d = dram("w1t", [D, H])
    b1_d = dram("b1", [H])
    g1_d = dram("g1", [H])
    be1_d = dram("be1", [H])
    w2_d = dram("w2t", [H, H])
    b2_d = dram("b2", [H])
    g2_d = dram("g2", [H])
    be2_d = dram("be2", [H])
    wmv_d = dram("wmvt", [H, 2 * L])
    bmv_d = dram("bmv", [2 * L])
    ctxT_d = dram("ctxT", [L, M])
    ctx_d = dram("ctx", [M, L])
    wdi_d = dram("wdit", [L, H])
    dib_d = dram("dib", [H])
    wd1_d = dram("wd1t", [H, H])
    db1_d = dram("db1", [H])
    dg1_d = dram("dg1", [H])
    dbe1_d = dram("dbe1", [H])
    wd2_d = dram("wd2t", [H, 2 * H])
    db2_d = dram("db2", [2 * H])
    dg2_d = dram("dg2", [2 * H])
    dbe2_d = dram("dbe2", [2 * H])
    wd3_d = dram("wd3t", [2 * H, D])
    db3_d = dram("db3", [D])

    recon_d = dram("recon", [B_LOC, D], kind="ExternalOutput")
    mu_d = dram("mu", [B_LOC, L], kind="ExternalOutput")
    lv_d = dram("lv", [B_LOC, L], kind="ExternalOutput")

    with tile.TileContext(nc, pool_alloc_mode="queue") as tc, ExitStack() as glob:
        const = glob.enter_context(tc.tile_pool(name="const", bufs=1))
        dstash = glob.enter_context(
            tc.tile_pool(name="dstash", bufs=1, space="DRAM")
        )
        ident = const.tile([P, P], F32)
        make_identity(nc, ident)
        identr = const.tile([P, P], F32R)
        nc.vector.tensor_copy(identr[:], ident[:])
        epsln = const.tile([P, 1], F32)
        nc.vector.memset(epsln, LN_EPS)

        zenh_s = dstash.tile([NT, P, L // P, P], mmdt)
        d2t_s = dstash.tile([NT, P, 8, P], mmdt)
        wpB = glob.enter_context(tc.tile_pool(name="wB", bufs=1))
        prefetch = {}

        # ---------- helpers ----------
        def load_w(pool, dram_t, kt, nf, name):
            t = pool.tile([P, kt, nf], mmdt, name=name)
            nc.gpsimd.dma_start(
                t[:], dram_t.ap().rearrange("(kt p) n -> p kt n", p=P)
            )
            return t

        def load_bcast(pool, dram_t, nf, name):
            t = pool.tile([P, nf], F32, name=name)
            src = bass.AP(
                tensor=dram_t.ap().tensor, offset=0, ap=[[0, P], [1, nf]]
            )
            nc.gpsimd.dma_start(t[:], src)
            return t

        def load_packed(pool, dram_t, kt, name):
            t = pool.tile([P, kt], F32, name=name)
            nc.sync.dma_start(t[:], dram_t.ap().rearrange("(c p) -> p c", p=P))
            return t

        def mm_chunks(ps_pool, xT, w_sb, kt, tag):
            """Returns [(psum_tile, n0, nsz)] for all 512-chunks."""
            nf = w_sb.shape[2]
            out = []
            for ci, (n0, nsz) in enumerate(_chunks(nf)):
                ps = ps_pool.tile(
                    [P, 512], F32, name=f"ps_{tag}_{ci}", tag="mmps",
                    bufs=psbufs,
                )
                for k in range(kt):
                    nc.tensor.matmul(
                        ps[:, :nsz],
                        xT[:, k, :],
                        w_sb[:, k, n0 : n0 + nsz],
                        start=(k == 0),
                        stop=(k == kt - 1),
                    )
                out.append((ps, n0, nsz))
            return out

        def ln_stats(work, srcs, tag):
            """bn stats over chunk aps -> (negmu [P,1], mv [P,2])."""
            nsub = len(srcs)
            stats = work.tile(
                [P, nsub, 6], F32, name=f"st_{tag}", tag="stats", bufs=sbufs
            )
            for s, (src, n0, nsz) in enumerate(srcs):
                nc.vector.bn_stats(out=stats[:, s, :], in_=src[:, :nsz])
            mv = work.tile([P, 2], F32, name=f"mv_{tag}", tag="mv", bufs=sbufs)
            nc.vector.bn_aggr(out=mv[:], in_=stats[:])
            nmu = work.tile(
                [P, 1], F32, name=f"nmu_{tag}", tag="nmu", bufs=sbufs
            )
            nc.vector.tensor_scalar_mul(out=nmu[:], in0=mv[:, 0:1],
                                        scalar1=-1.0)
            return nmu, mv

        def rsqrt_dve(work, mv, tag):
            """rs = 1/sqrt(var + eps) via bit-trick + 2 Newton iters (DVE
            only -- keeps Sqrt off ScalarE so its LUT set never swaps)."""
            I32 = mybir.dt.int32
            v1 = work.tile([P, 1], F32, name=f"v1_{tag}", tag="v1",
                           bufs=sbufs)
            nc.vector.tensor_scalar_add(out=v1[:], in0=mv[:, 1:2],
                                        scalar1=LN_EPS)
            ti = work.tile([P, 1], I32, name=f"ti_{tag}", tag="ti",
                           bufs=sbufs)
            nc.vector.tensor_scalar(
                out=ti[:], in0=v1[:].bitcast(I32), scalar1=1, scalar2=None,
                op0=ALU.logical_shift_right,
            )
            nc.vector.tensor_scalar(
                out=ti[:], in0=ti[:], scalar1=-1, scalar2=0x5F3759DF,
                op0=ALU.mult, op1=ALU.add,
            )
            y = work.tile([P, 1], F32, name=f"yq_{tag}", tag="yq",
                          bufs=sbufs)
            nc.vector.tensor_copy(y[:], ti[:].bitcast(F32))
            hv = work.tile([P, 1], F32, name=f"hv_{tag}", tag="hv",
                           bufs=sbufs)
            nc.vector.tensor_scalar_mul(out=hv[:], in0=v1[:], scalar1=0.5)
            tq = work.tile([P, 1], F32, name=f"tq_{tag}", tag="tq",
                           bufs=sbufs)
            for _ in range(2):
                nc.vector.tensor_tensor(out=tq[:], in0=y[:], in1=y[:],
                                        op=ALU.mult)
                nc.vector.tensor_tensor(out=tq[:], in0=tq[:], in1=hv[:],
                                        op=ALU.mult)
                nc.vector.tensor_scalar(
                    out=tq[:], in0=tq[:], scalar1=-1.0, scalar2=1.5,
                    op0=ALU.mult, op1=ALU.add,
                )
                nc.vector.tensor_tensor(out=y[:], in0=y[:], in1=tq[:],
                                        op=ALU.mult)
            return y

        def transpose_in(work, ps_pool, src_sb, kt, out_dt, idt, tag,
                         grp=None):
            grp = grp or tag
            """src [P, kt*128] -> [P, kt, 128] via PE transposes grouped
            into 512-wide PSUM tiles + wide DVE copies."""
            out = work.tile(
                [P, kt, P], out_dt, name=f"t_{tag}", tag=f"t_{grp}",
                bufs=(sbufs if kt <= 8 else 2),
            )
            for g0 in range(0, kt, 4):
                gn = min(4, kt - g0)
                pw = ps_pool.tile(
                    [P, 512], F32, name=f"tw_{tag}_{g0}", tag="trps",
                    bufs=trbufs,
                )
                for j in range(gn):
                    dst = pw[:, j * P : (j + 1) * P]
                    src_c = src_sb[:, (g0 + j) * P : (g0 + j + 1) * P]
                    if src_c.dtype == F32R:
                        dst = dst.bitcast(F32R)
                    nc.tensor.transpose(dst, src_c, idt)
                nc.vector.tensor_copy(
                    out[:, g0 : g0 + gn, :].rearrange("p k c -> p (k c)"),
                    pw[:, : gn * P],
                )
            return out

        def dense_act(work, ps_tr, srcs, nf, tag, grp, *, ln, need_rs,
                      out_kt):
            """activate (LN shift + lrelu) + transpose half of a layer."""
            h = work.tile(
                [P, nf], mmdt, name=f"h_{tag}", tag=f"h{nf}", bufs=2
            )
            rs = None
            if ln:
                nmu, mv = ln_stats(work, srcs, tag)
                if need_rs:
                    rs = rsqrt_dve(work, mv, tag)
                for src, n0, nsz in srcs:
                    nc.scalar.activation(
                        out=h[:, n0 : n0 + nsz], in_=src[:, :nsz],
                        func=AF.Prelu, bias=nmu[:], scale=1.0, alpha=ALPHA,
                    )
            else:
                for src, n0, nsz in srcs:
                    nc.scalar.activation(
                        out=h[:, n0 : n0 + nsz], in_=src[:, :nsz],
                        func=AF.Prelu, bias=0.0, scale=1.0, alpha=ALPHA,
                    )
            tT = transpose_in(
                work, ps_tr, h, out_kt, mmdt,
                identr if mmdt == F32R else ident, f"{tag}T",
                grp=f"{grp}T",
            )
            return (tT, rs) if need_rs else tT

        def dense_fast(work, ps_mm, ps_tr, xT, w_sb, kt, tag, *,
                       ln, lrelu, out_kt, need_rs=False, grp=None):
            grp = grp or tag
            """simple-path layer: matmul -> (LN shift) -> lrelu ->
            transposed fp32r copy. The LN 1/std factor is NOT applied
            here: leaky-relu is positively homogeneous and LN is
            scale-invariant per sample, so the factor cancels through
            the next LN; layers feeding non-LN consumers get it back
            via need_rs (folded into the consumer's PSUM copy)."""
            srcs = mm_chunks(ps_mm, xT, w_sb, kt, tag)
            nf = w_sb.shape[2]
            h = work.tile(
                [P, nf], mmdt, name=f"h_{tag}", tag=f"h{nf}", bufs=2
            )
            rs = None
            if ln:
                nmu, mv = ln_stats(work, srcs, tag)
                if need_rs:
                    rs = rsqrt_dve(work, mv, tag)
                for src, n0, nsz in srcs:
                    nc.scalar.activation(
                        out=h[:, n0 : n0 + nsz], in_=src[:, :nsz],
                        func=AF.Prelu, bias=nmu[:], scale=1.0, alpha=ALPHA,
                    )
            else:
                assert lrelu
                for src, n0, nsz in srcs:
                    nc.scalar.activation(
                        out=h[:, n0 : n0 + nsz], in_=src[:, :nsz],
                        func=AF.Prelu, bias=0.0, scale=1.0, alpha=ALPHA,
                    )
            tT = transpose_in(
                work, ps_tr, h, out_kt, mmdt,
                identr if mmdt == F32R else ident, f"{tag}T",
                grp=f"{grp}T",
            )
            return (tT, rs) if need_rs else tT

        def dense_general(work, ps_mm, ps_tr, xT, w_sb, kt, tag, *,
                          ln, lrelu, out_kt, bias_bc, gp, bep, grp=None):
            grp = grp or tag
            """general-path layer (nonzero bias / non-unit gamma):
            y = psum + bias; xhat = (y-mu)*rs; transpose; per-chunk
            ScalarE Prelu(xhat*g + beta)."""
            srcs = mm_chunks(ps_mm, xT, w_sb, kt, tag)
            nf = w_sb.shape[2]
            y = work.tile([P, nf], F32, name=f"y_{tag}", tag=f"y{nf}", bufs=2)
            for src, n0, nsz in srcs:
                if bias_bc is not None:
                    nc.vector.tensor_tensor(
                        out=y[:, n0 : n0 + nsz], in0=src[:, :nsz],
                        in1=bias_bc[:, n0 : n0 + nsz], op=ALU.add,
                    )
                else:
                    nc.vector.tensor_copy(y[:, n0 : n0 + nsz], src[:, :nsz])
            xh = y
            if ln:
                nmu, mv = ln_stats(
                    work,
                    [(y[:, n0 : n0 + nsz], n0, nsz) for _, n0, nsz in srcs],
                    tag,
                )
                rs = rsqrt_dve(work, mv, tag)
                xh = work.tile(
                    [P, nf], F32, name=f"xh_{tag}", tag=f"xh{nf}", bufs=2
                )
                nc.vector.tensor_scalar(
                    out=xh[:], in0=y[:], scalar1=nmu[:], scalar2=rs[:],
                    op0=ALU.add, op1=ALU.mult,
                )
            out = work.tile(
                [P, out_kt, P], mmdt, name=f"t_{tag}", tag=f"t_{grp}",
                bufs=2,
            )
            for g0 in range(0, out_kt, 4):
                gn = min(4, out_kt - g0)
                pw = ps_tr.tile(
                    [P, 512], F32, name=f"tw_{tag}_{g0}", tag="trps",
                    bufs=trbufs,
                )
                for j in range(gn):
                    nc.tensor.transpose(
                        pw[:, j * P : (j + 1) * P],
                        xh[:, (g0 + j) * P : (g0 + j + 1) * P],
                        ident,
                    )
                for j in range(gn):
                    k = g0 + j
                    nc.scalar.activation(
                        out=out[:, k, :], in_=pw[:, j * P : (j + 1) * P],
                        func=AF.Prelu if (ln or lrelu) else AF.Identity,
                        bias=bep[:, k : k + 1] if bep is not None else 0.0,
                        scale=gp[:, k : k + 1] if gp is not None else 1.0,
                        alpha=ALPHA,
                    )
            return out

        def raw_out(work, srcs, nf, tag, bias_bc=None, row_scale=None,
                    grp=None):
            o = work.tile([P, nf], F32, name=f"o_{tag}", tag=f"o_{grp or tag}",
                          bufs=2)
            for src, n0, nsz in srcs:
                if bias_bc is not None:
                    nc.vector.tensor_tensor(
                        out=o[:, n0 : n0 + nsz], in0=src[:, :nsz],
                        in1=bias_bc[:, n0 : n0 + nsz], op=ALU.add,
                    )
                elif row_scale is not None:
                    nc.vector.tensor_scalar(
                        out=o[:, n0 : n0 + nsz], in0=src[:, :nsz],
                        scalar1=row_scale[:], scalar2=None, op0=ALU.mult,
                    )
                else:
                    nc.vector.tensor_copy(o[:, n0 : n0 + nsz], src[:, :nsz])
            return o

        def sw_pipeline(stage_fns, n, order=None):
            """Software-pipelined emission: the Tile scheduler is a
            priority-list scheduler, so per-engine execution order tracks
            emission order -- interleaving stages of neighboring row-tiles
            here is what lets PE run tile j+1 matmuls while tile j's
            LN/softmax chain is on DVE/ScalarE. `order` sets the
            intra-tick stage emission order (default deepest-first)."""
            S = len(stage_fns)
            if order is None:
                order = list(range(S - 1, -1, -1))
            states = [dict() for _ in range(n)]
            for t in range(n + S - 1):
                for s in order:
                    j = t - s
                    if 0 <= j < n:
                        stage_fns[s](j, states[j])

        # ================= PHASE A: encoder =================
        with ExitStack() as ph:
            wp = ph.enter_context(tc.tile_pool(name="wA", bufs=1))
            work = ph.enter_context(tc.tile_pool(name="workA", bufs=wbufs[0]))
            ps_mm = ph.enter_context(
                tc.tile_pool(name="psA", bufs=1, space="PSUM")
            )
            ps_tr = ph.enter_context(
                tc.tile_pool(name="psAt", bufs=1, space="PSUM")
            )
            w1 = load_w(wp, w1_d, D // P, H, "w1")
            w2 = load_w(wp, w2_d, H // P, H, "w2")
            wmv = load_w(wp, wmv_d, H // P, 2 * L, "wmv")
            ctxT = wp.tile([P, 2, M], F32, name="ctxT")
            nc.sync.dma_start(
                ctxT[:], ctxT_d.ap().rearrange("(kt p) n -> p kt n", p=P)
            )
            ctxm = wp.tile([M, L], mmdt, name="ctxm")
            nc.gpsimd.dma_start(ctxm[:], ctx_d.ap())
            if not simple:
                b1c = load_bcast(wp, b1_d, H, "b1c")
                b2c = load_bcast(wp, b2_d, H, "b2c")
                bmvc = load_bcast(wp, bmv_d, 2 * L, "bmvc")
                g1p = load_packed(wp, g1_d, H // P, "g1p")
                be1p = load_packed(wp, be1_d, H // P, "be1p")
                g2p = load_packed(wp, g2_d, H // P, "g2p")
                be2p = load_packed(wp, be2_d, H // P, "be2p")

            def sA0(i, st):
                if i == 2:
                    prefetch["wdi"] = load_w(wpB, wdi_d, L // P, H, "wdi")
                    prefetch["wd1"] = load_w(wpB, wd1_d, H // P, H, "wd1")
                r0 = i * P
                xT = work.tile([P, D // P, P], mmdt, name=f"xT_{i}",
                               tag="xT", bufs=sbufs)
                nc.gpsimd.dma_start(
                    xT[:],
                    x_d.ap().rearrange("(kt p) (nt c) -> p kt nt c", p=P,
                                       c=P)[:, :, i, :],
                )
                st["xT"] = xT

            def sA1a(i, st):
                if simple:
                    st["l1s"] = mm_chunks(ps_mm, st["xT"], w1, D // P,
                                          f"l1_{i}")
                else:
                    st["h1T"] = dense_general(
                        work, ps_mm, ps_tr, st["xT"], w1, D // P, f"l1_{i}",
                        ln=True, lrelu=True, out_kt=H // P,
                        bias_bc=b1c, gp=g1p, bep=be1p, grp="l1",
                    )

            def sA1b(i, st):
                if simple:
                    st["h1T"] = dense_act(
                        work, ps_tr, st.pop("l1s"), H, f"l1_{i}", "l1",
                        ln=True, need_rs=False, out_kt=H // P,
                    )

            def sA2a(i, st):
                if simple:
                    st["l2s"] = mm_chunks(ps_mm, st["h1T"], w2, H // P,
                                          f"l2_{i}")
                else:
                    st["h2T"] = dense_general(
                        work, ps_mm, ps_tr, st["h1T"], w2, H // P, f"l2_{i}",
                        ln=True, lrelu=True, out_kt=H // P,
                        bias_bc=b2c, gp=g2p, bep=be2p, grp="l2",
                    )
                    st["rs2"] = None

            def sA2b(i, st):
                if simple:
                    st["h2T"], st["rs2"] = dense_act(
                        work, ps_tr, st.pop("l2s"), H, f"l2_{i}", "l2",
                        ln=True, need_rs=True, out_kt=H // P,
                    )

            def sA3(i, st):
                r0 = i * P
                if simple:
                    smv = raw_out(
                        work, mm_chunks(ps_mm, st["h2T"], wmv, H // P,
                                        f"mv_{i}"),
                        2 * L, f"mv_{i}", row_scale=st["rs2"], grp="mv",
                    )
                else:
                    smv = raw_out(
                        work, mm_chunks(ps_mm, st["h2T"], wmv, H // P,
                                        f"mv_{i}"),
                        2 * L, f"mv_{i}", bias_bc=bmvc, grp="mv",
                    )
                nc.sync.dma_start(mu_d[r0 : r0 + P, :], smv[:, :L])
                nc.sync.dma_start(lv_d[r0 : r0 + P, :], smv[:, L:])

                elv = work.tile([P, L], F32, name=f"elv_{i}", tag="elv",
                                bufs=sbufs)
                nc.scalar.activation(
                    out=elv[:], in_=smv[:, L:], func=AF.Exp, bias=0.0,
                    scale=0.5,
                )
                eps_sb = work.tile([P, L], F32, name=f"eps_{i}", tag="eps",
                                   bufs=sbufs)
                nc.sync.dma_start(eps_sb[:], eps_d[r0 : r0 + P, :])
                z_sb = work.tile([P, L], F32, name=f"z_{i}", tag="z",
                                 bufs=sbufs)
                nc.vector.tensor_tensor(
                    out=z_sb[:], in0=elv[:], in1=eps_sb[:], op=ALU.mult
                )
                nc.vector.tensor_tensor(
                    out=z_sb[:], in0=z_sb[:], in1=smv[:, :L], op=ALU.add
                )
                st["z"] = z_sb

            def sA4(i, st):
                z_sb = st["z"]
                zT = transpose_in(work, ps_tr, z_sb, L // P, F32, ident,
                                  f"zT{i}", grp="zT")
                s_ps = ps_mm.tile([P, 512], F32, name=f"sps_{i}", tag="mmps",
                                  bufs=psbufs)
                for k in range(L // P):
                    nc.tensor.matmul(
                        s_ps[:, :M], zT[:, k, :], ctxT[:, k, :],
                        start=(k == 0), stop=(k == L // P - 1),
                    )
                negmx = work.tile([P, 1], F32, name=f"nmx_{i}", tag="nmx",
                                  bufs=sbufs)
                nc.vector.tensor_reduce(
                    out=negmx[:], in_=s_ps[:, :M],
                    axis=mybir.AxisListType.X, op=ALU.max, negate=True,
                )
                e_sb = work.tile([P, M], F32, name=f"e_{i}", tag="e",
                                 bufs=sbufs)
                se = work.tile([P, 1], F32, name=f"se_{i}", tag="se",
                               bufs=sbufs)
                nc.scalar.activation(
                    out=e_sb[:], in_=s_ps[:, :M], func=AF.Exp,
                    bias=negmx[:], scale=1.0, accum_out=se[:],
                )
                rs01 = work.tile([P, 1], F32, name=f"r01_{i}", tag="r01",
                                 bufs=sbufs)
                nc.vector.reciprocal(out=rs01[:], in_=se[:])
                nc.vector.tensor_scalar_mul(
                    out=rs01[:], in0=rs01[:], scalar1=0.1
                )
                # e_n = e * (0.1 / sum): fold attn normalization here so the
                # transposed context matmul needs no per-column scale
                nc.vector.tensor_scalar(
                    out=e_sb[:], in0=e_sb[:], scalar1=rs01[:], scalar2=None,
                    op0=ALU.mult,
                )
                trE = ps_tr.tile([P, 512], F32, name=f"trE_{i}", tag="trps",
                                 bufs=trbufs)
                nc.tensor.transpose(trE[:M, :P], e_sb[:], ident[:])
                eT = work.tile([M, P], mmdt, name=f"eT_{i}", tag="eT",
                               bufs=sbufs)
                nc.vector.tensor_copy(eT[:], trE[:M, :P])
                # z_addT[l_chunk, b] = ctx[:, l_chunk].T @ e_n.T  (fp32r)
                za_ps = ps_tr.tile([P, 512], F32, name=f"zaps_{i}",
                                   tag="trps", bufs=trbufs)
                for c in range(L // P):
                    nc.tensor.matmul(
                        za_ps[:, c * P : (c + 1) * P],
                        ctxm[:, c * P : (c + 1) * P],
                        eT[:],
                        start=True, stop=True,
                    )
                zeT = work.tile([P, L // P, P], mmdt, name=f"zeT_{i}",
                                tag="zeTA", bufs=sbufs)
                nc.vector.tensor_tensor(
                    out=zeT[:].rearrange("p k c -> p (k c)"),
                    in0=zT[:].rearrange("p k c -> p (k c)"),
                    in1=za_ps[:, :L],
                    op=ALU.add,
                )
                nc.sync.dma_start(zenh_s[i], zeT[:])
                st.clear()

            sw_pipeline([sA0, sA1a, sA1b, sA2a, sA2b, sA3, sA4], NT,
                        order=ORDER_A)

        # ================= PHASE B: di + dec1 =================
        wpC = glob.enter_context(tc.tile_pool(name="wC", bufs=1))
        with ExitStack() as ph:
            work = ph.enter_context(tc.tile_pool(name="workB", bufs=wbufs[1]))
            ps_mm = ph.enter_context(
                tc.tile_pool(name="psB", bufs=1, space="PSUM")
            )
            ps_tr = ph.enter_context(
                tc.tile_pool(name="psBt", bufs=1, space="PSUM")
            )
            wdi = prefetch["wdi"]
            wd1 = prefetch["wd1"]
            if not simple:
                dibp = load_packed(wpB, dib_d, H // P, "dibp")
                db1c = load_bcast(wpB, db1_d, H, "db1c")
                dg1p = load_packed(wpB, dg1_d, H // P, "dg1p")
                dbe1p = load_packed(wpB, dbe1_d, H // P, "dbe1p")

            def sB0(i, st):
                if i == 2:
                    prefetch["wd2"] = load_w(wpC, wd2_d, H // P, 2 * H, "wd2")
                    prefetch["wd3"] = load_w(wpC, wd3_d, 2 * H // P, D, "wd3")
                zeT = work.tile([P, L // P, P], mmdt, name=f"zeB_{i}",
                                tag="zeB", bufs=sbufs)
                nc.sync.dma_start(zeT[:], zenh_s[i])
                st["zeT"] = zeT

            def sB1a(i, st):
                if simple:
                    st["dis"] = mm_chunks(ps_mm, st["zeT"], wdi, L // P,
                                          f"di_{i}")
                else:
                    st["d1T"] = dense_general(
                        work, ps_mm, ps_tr, st["zeT"], wdi, L // P, f"di_{i}",
                        ln=False, lrelu=True, out_kt=H // P,
                        bias_bc=None, gp=None, bep=dibp, grp="di",
                    )

            def sB1b(i, st):
                if simple:
                    st["d1T"] = dense_act(
                        work, ps_tr, st.pop("dis"), H, f"di_{i}", "di",
                        ln=False, need_rs=False, out_kt=H // P,
                    )

            def sB2a(i, st):
                if simple:
                    st["d1s"] = mm_chunks(ps_mm, st["d1T"], wd1, H // P,
                                          f"d1_{i}")
                else:
                    d2T = dense_general(
                        work, ps_mm, ps_tr, st["d1T"], wd1, H // P, f"d1_{i}",
                        ln=True, lrelu=True, out_kt=H // P,
                        bias_bc=db1c, gp=dg1p, bep=dbe1p, grp="d1",
                    )
                    nc.sync.dma_start(d2t_s[i], d2T[:])
                    st.clear()

            def sB2b(i, st):
                if simple:
                    d2T = dense_act(
                        work, ps_tr, st.pop("d1s"), H, f"d1_{i}", "d1",
                        ln=True, need_rs=False, out_kt=H // P,
                    )
                    nc.sync.dma_start(d2t_s[i], d2T[:])
                    st.clear()

            sw_pipeline([sB0, sB1a, sB1b, sB2a, sB2b], NT,
                        order=ORDER_B)

        # ================= PHASE C: dec2 + dec3 =================
        with ExitStack() as ph:
            work = ph.enter_context(tc.tile_pool(name="workC", bufs=wbufs[2]))
            ps_mm = ph.enter_context(
                tc.tile_pool(name="psC", bufs=1, space="PSUM")
            )
            ps_tr = ph.enter_context(
                tc.tile_pool(name="psCt", bufs=1, space="PSUM")
            )
            wd2 = prefetch["wd2"]
            wd3 = prefetch["wd3"]
            if not simple:
                db2c = load_bcast(wpC, db2_d, 2 * H, "db2c")
                dg2p = load_packed(wpC, dg2_d, 2 * H // P, "dg2p")
                dbe2p = load_packed(wpC, dbe2_d, 2 * H // P, "dbe2p")
                db3c = load_bcast(wpC, db3_d, D, "db3c")

            def sC0(i, st):
                d2T = work.tile([P, 8, P], mmdt, name=f"d2C_{i}", tag="d2C",
                                bufs=3)
                nc.sync.dma_start(d2T[:], d2t_s[i])
                st["d2T"] = d2T

            def sC1a(i, st):
                if simple:
                    st["d2s"] = mm_chunks(ps_mm, st["d2T"], wd2, H // P,
                                          f"d2_{i}")
                else:
                    st["d3T"] = dense_general(
                        work, ps_mm, ps_tr, st["d2T"], wd2, H // P, f"d2_{i}",
                        ln=True, lrelu=True, out_kt=2 * H // P,
                        bias_bc=db2c, gp=dg2p, bep=dbe2p, grp="d2",
                    )
                    st["rs4"] = None

            def sC1b(i, st):
                if simple:
                    st["d3T"], st["rs4"] = dense_act(
                        work, ps_tr, st.pop("d2s"), 2 * H, f"d2_{i}", "d2",
                        ln=True, need_rs=True, out_kt=2 * H // P,
                    )

            def sC2(i, st):
                r0 = i * P
                if simple:
                    recon_sb = raw_out(
                        work,
                        mm_chunks(ps_mm, st["d3T"], wd3, 2 * H // P,
                                  f"d3_{i}"),
                        D, f"d3_{i}", row_scale=st["rs4"], grp="d3",
                    )
                else:
                    recon_sb = raw_out(
                        work,
                        mm_chunks(ps_mm, st["d3T"], wd3, 2 * H // P,
                                  f"d3_{i}"),
                        D, f"d3_{i}", bias_bc=db3c, grp="d3",
                    )
                nc.sync.dma_start(recon_d[r0 : r0 + P, :], recon_sb[:])
                st.clear()

            sw_pipeline([sC0, sC1a, sC1b, sC2], NT, order=ORDER_C)

    nc.finalize()
    return nc


NB = 256  # batch columns per super-tile (moving-dim of fp32r matmuls)
NST = B_LOC // NB  # 16 super-tiles per core


def _build_v3(psbufs=4, trbufs=3, hbufs=2, sq_on_act=True,
              ORDER_A=None, ORDER_B=None, ORDER_C=None):
    """Feature-major dataflow: activations live transposed ([feature
    chunk on partitions, batch free]) end to end, weights are the
    stationary matmul operand, so no PE transposes of activations are
    needed. LN mean-subtraction is folded into host-centered weights
    (W' = W - mean_out(W)); the LN 1/std factor cancels through LN->LN
    chains (leaky-relu is positively homogeneous) and is only computed
    for ln2/dln2 via a ones-matmul over ScalarE-squared chunks, then
    applied per batch column through a PE-broadcast row. Assumes zero
    biases / unit gammas (checked by kernel())."""
    ORDER_A = ORDER_A or [0, 1, 2, 3, 4]
    ORDER_B = ORDER_B or [0, 1, 2]
    ORDER_C = ORDER_C or [0, 1, 2]
    mmdt = F32R
    nc = bacc.Bacc(
        "TRN2", target_bir_lowering=False, debug=False, num_devices=N_CORES
    )
    dram = lambda name, shape, dt=F32, kind="ExternalInput": nc.dram_tensor(
        name, shape, dt, kind=kind
    )
    x_d = dram("x", [D, B_LOC])        # host-transposed
    eps_d = dram("eps", [L, B_LOC])    # host-transposed
    w1_d = dram("w1t", [D, H])         # host-centered
    w2_d = dram("w2t", [H, H])         # host-centered
    wmv_d = dram("wmvt", [H, 2 * L])
    ctxT_d = dram("ctxT", [L, M])
    ctx_d = dram("ctx", [M, L])
    wdi_d = dram("wdit", [L, H])
    wd1_d = dram("wd1t", [H, H])       # host-centered
    wd2_d = dram("wd2t", [H, 2 * H])   # host-centered
    wd3_d = dram("wd3t", [2 * H, D])
    recon_d = dram("recon", [D, B_LOC], kind="ExternalOutput")  # host .T
    mu_d = dram("mu", [L, B_LOC], kind="ExternalOutput")        # host .T
    lv_d = dram("lv", [L, B_LOC], kind="ExternalOutput")        # host .T

    with tile.TileContext(nc) as tc, ExitStack() as glob:
        const = glob.enter_context(tc.tile_pool(name="const", bufs=1))
        dstash = glob.enter_context(
            tc.tile_pool(name="dstash", bufs=1, space="DRAM")
        )
        ident = const.tile([P, P], F32)
        make_identity(nc, ident)
        ones_f = const.tile([P, 1], F32)
        nc.vector.memset(ones_f, 1.0)
        onesr = const.tile([P, 1], F32R)
        nc.vector.tensor_copy(onesr[:], ones_f[:])
        ones_rf = const.tile([1, P], F32)
        nc.vector.memset(ones_rf, 1.0)
        ones_row = const.tile([1, P], F32R)
        nc.vector.tensor_copy(ones_row[:], ones_rf[:])

        zenh_s = dstash.tile([NST, P, L // P, NB], mmdt)
        d2t_s = dstash.tile([NST, P, H // P, NB], mmdt)

        wrapB = glob.enter_context(ExitStack())
        wpB = wrapB.enter_context(
            tc.tile_pool(name="wB", bufs=1, side="right")
        )
        prefetch = {}

        def load_w(pool, dram_t, kt, nf, name):
            # one DMA per K-chunk so matmuls can start before the whole
            # weight tile has landed
            t = pool.tile([P, kt, nf], mmdt, name=name)
            src_ap = dram_t.ap().rearrange("(kt p) n -> p kt n", p=P)
            for k in range(kt):
                nc.gpsimd.dma_start(t[:, k : k + 1, :], src_ap[:, k : k + 1, :])
            return t

        def sw_pipeline(stage_fns, n, order):
            S = len(stage_fns)
            states = [dict() for _ in range(n)]
            for t in range(n + S - 1):
                for s in order:
                    j = t - s
                    if 0 <= j < n:
                        stage_fns[s](j, states[j])

        def mmF(ps_pool, w_sb, xT, kt, out_kt, tag):
            """feature-major layer: psum chunk m = sum_k W[:,k,m].T@xT[:,k].
            Two 256-wide chunks share one 512-wide PSUM tile (bank)."""
            chunks = []
            for mp in range((out_kt + 1) // 2):
                ps = ps_pool.tile([P, 512], F32, name=f"ps_{tag}_{mp}",
                                  tag="mmps", bufs=psbufs)
                msz = min(2, out_kt - 2 * mp)
                for sub in range(msz):
                    m = 2 * mp + sub
                    pslice = ps[:, sub * NB : (sub + 1) * NB]
                    for k in range(kt):
                        nc.tensor.matmul(
                            pslice,
                            w_sb[:, k, m * P : (m + 1) * P],
                            xT[:, k, :],
                            start=(k == 0),
                            stop=(k == kt - 1),
                        )
                chunks.append((ps, 2 * mp, msz))
            return chunks

        def act_lrelu(work, chunks, out_kt, tag, grp):
            h = work.tile([P, out_kt, NB], mmdt, name=f"h_{tag}",
                          tag=f"h_{grp}", bufs=hbufs)
            for ps, m0, msz in chunks:
                nc.scalar.activation(
                    out=h[:, m0 : m0 + msz, :].rearrange(
                        "p k c -> p (k c)"
                    ),
                    in_=ps[:, : msz * NB],
                    func=AF.Prelu, bias=0.0, scale=1.0, alpha=ALPHA,
                )
            return h

        def rs_row(work, ps_q, chunks, nf, tag):
            """rs = 1/sqrt(mean(y_c^2)+eps) per batch column, PE-broadcast
            to [P, NB] in SBUF (f32). chunks are this layer's psums."""
            q_ps = ps_q.tile([1, 2, NB], F32, name=f"q_{tag}", tag="qps",
                             bufs=1)
            for pi, (ps, m0, msz) in enumerate(chunks):
                sq = work.tile([P, 512], F32R, name=f"sq_{tag}_{m0}",
                               tag="sq", bufs=2)
                nc.scalar.activation(out=sq[:, : msz * NB],
                                     in_=ps[:, : msz * NB],
                                     func=AF.Square, bias=0.0, scale=1.0)
                nc.tensor.matmul(
                    q_ps[:].rearrange("o k c -> o (k c)")[:, : msz * NB],
                    onesr[:],
                    sq[:, : msz * NB],
                    start=(pi == 0), stop=(pi == len(chunks) - 1),
                )
            I32 = mybir.dt.int32
            q_sb = work.tile([1, 2, NB], F32, name=f"qs_{tag}", tag="qs",
                             bufs=1)
            nc.vector.tensor_copy(
                q_sb[:].rearrange("o k c -> o (k c)"),
                q_ps[:].rearrange("o k c -> o (k c)"),
            )
            qt = work.tile([1, NB], F32, name=f"qt_{tag}", tag="qt", bufs=2)
            nc.vector.tensor_tensor(out=qt[:], in0=q_sb[:, 0, :],
                                    in1=q_sb[:, 1, :], op=ALU.add)
            v1 = work.tile([1, NB], F32, name=f"v1_{tag}", tag="v1", bufs=1)
            nc.vector.tensor_scalar(
                out=v1[:], in0=qt[:], scalar1=1.0 / nf, scalar2=LN_EPS,
                op0=ALU.mult, op1=ALU.add,
            )
            ti = work.tile([1, NB], I32, name=f"ti_{tag}", tag="ti", bufs=1)
            nc.vector.tensor_scalar(
                out=ti[:], in0=v1[:].bitcast(I32), scalar1=1, scalar2=None,
                op0=ALU.logical_shift_right,
            )
            nc.vector.tensor_scalar(
                out=ti[:], in0=ti[:], scalar1=-1, scalar2=0x5F3759DF,
                op0=ALU.mult, op1=ALU.add,
            )
            y = work.tile([1, NB], F32, name=f"yq_{tag}", tag="yq", bufs=1)
            nc.vector.tensor_copy(y[:], ti[:].bitcast(F32))
            hv = work.tile([1, NB], F32, name=f"hv_{tag}", tag="hv", bufs=1)
            nc.vector.tensor_scalar_mul(out=hv[:], in0=v1[:], scalar1=0.5)
            tq = work.tile([1, NB], F32, name=f"tq_{tag}", tag="tq", bufs=1)
            for _ in range(2):
                nc.vector.tensor_tensor(out=tq[:], in0=y[:], in1=y[:],
                                        op=ALU.mult)
                nc.vector.tensor_tensor(out=tq[:], in0=tq[:], in1=hv[:],
                                        op=ALU.mult)
                nc.vector.tensor_scalar(
                    out=tq[:], in0=tq[:], scalar1=-1.0, scalar2=1.5,
                    op0=ALU.mult, op1=ALU.add,
                )
                nc.vector.tensor_tensor(out=y[:], in0=y[:], in1=tq[:],
                                        op=ALU.mult)
            yr = work.tile([1, NB], F32R, name=f"yr_{tag}", tag="yr", bufs=2)
            nc.vector.tensor_copy(yr[:], y[:])
            yr_b = bass.AP(
                tensor=yr.tensor, offset=yr.offset,
                ap=[list(yr.ap[0]), [0, 2], [1, NB]],
            )
            rb_ps = ps_q.tile([P, 2, NB], F32, name=f"rb_{tag}", tag="rbps",
                              bufs=1)
            nc.tensor.matmul(
                rb_ps[:].rearrange("p k c -> p (k c)"),
                ones_row[:],
                yr_b,
                start=True, stop=True,
            )
            rsb = work.tile([P, 2, NB], F32, name=f"rsb_{tag}", tag="rsb",
                            bufs=2)
            nc.vector.tensor_copy(
                rsb[:].rearrange("p k c -> p (k c)"),
                rb_ps[:].rearrange("p k c -> p (k c)"),
            )
            return rsb

        # ================= PHASE A =================
        with ExitStack() as ph:
            wp = ph.enter_context(tc.tile_pool(name="wA", bufs=1))
            work = ph.enter_context(tc.tile_pool(name="workA", bufs=2))
            ps_mm = ph.enter_context(
                tc.tile_pool(name="psA", bufs=1, space="PSUM")
            )
            ps_x = ph.enter_context(
                tc.tile_pool(name="psAx", bufs=1, space="PSUM")
            )
            weights = {}

            def sA0(i, st):
                xT = work.tile([P, D // P, NB], mmdt, name=f"xT_{i}",
                               tag="xT", bufs=2)
                nc.gpsimd.dma_start(
                    xT[:],
                    x_d.ap().rearrange("(kt p) (nt c) -> p kt nt c", p=P,
                                       c=NB)[:, :, i, :],
                )
                st["xT"] = xT
                if i == 0:
                    # weight loads go after the first input tile on the
                    # gpsimd queue so PE can start as soon as w1 lands
                    weights["w1"] = load_w(wp, w1_d, D // P, H, "w1")
                    weights["w2"] = load_w(wp, w2_d, H // P, H, "w2")
                    weights["wmv"] = load_w(wp, wmv_d, H // P, 2 * L, "wmv")
                    ctxT = wp.tile([P, 2, M], F32, name="ctxT")
                    nc.sync.dma_start(
                        ctxT[:],
                        ctxT_d.ap().rearrange("(kt p) n -> p kt n", p=P),
                    )
                    weights["ctxT"] = ctxT
                    ctxm = wp.tile([M, L], mmdt, name="ctxm")
                    nc.gpsimd.dma_start(ctxm[:], ctx_d.ap())
                    weights["ctxm"] = ctxm
                if i == 1:
                    prefetch["wdi"] = load_w(wpB, wdi_d, L // P, H, "wdi")
                    prefetch["wd1"] = load_w(wpB, wd1_d, H // P, H, "wd1")

            def sA1(i, st):
                st["h1T"] = act_lrelu(
                    work, mmF(ps_mm, weights["w1"], st.pop("xT"), D // P, H // P,
                              f"l1_{i}"),
                    H // P, f"l1_{i}", "l1",
                )

            def sA2(i, st):
                chunks = mmF(ps_mm, weights["w2"], st.pop("h1T"), H // P, H // P,
                             f"l2_{i}")
                st["h2T"] = act_lrelu(work, chunks, H // P, f"l2_{i}", "l2")
                st["rsb2"] = rs_row(work, ps_x, chunks, H, f"l2_{i}")

            def sA3(i, st):
                chunks = mmF(ps_mm, weights["wmv"], st.pop("h2T"), H // P,
                             2 * L // P, f"mv_{i}")
                rsb2 = st.pop("rsb2")
                smv = work.tile([P, 2 * L // P, NB], F32, name=f"smv_{i}",
                                tag="smv", bufs=2)
                for ps, m0, msz in chunks:
                    nc.vector.tensor_tensor(
                        out=smv[:, m0 : m0 + msz, :].rearrange(
                            "p k c -> p (k c)"
                        ),
                        in0=ps[:, : msz * NB],
                        in1=rsb2[:].rearrange("p k c -> p (k c)")[
                            :, : msz * NB
                        ],
                        op=ALU.mult,
                    )
                mu_ap = mu_d.ap().rearrange(
                    "(c p) (nt b) -> p c nt b", p=P, b=NB
                )[:, :, i, :]
                lv_ap = lv_d.ap().rearrange(
                    "(c p) (nt b) -> p c nt b", p=P, b=NB
                )[:, :, i, :]
                nc.sync.dma_start(mu_ap, smv[:, 0 : L // P, :])
                nc.sync.dma_start(lv_ap, smv[:, L // P :, :])

                elv = work.tile([P, L // P, NB], F32, name=f"elv_{i}",
                                tag="elv", bufs=1)
                for c in range(L // P):
                    nc.scalar.activation(
                        out=elv[:, c, :], in_=smv[:, L // P + c, :],
                        func=AF.Exp, bias=0.0, scale=0.5,
                    )
                epsT = work.tile([P, L // P, NB], F32, name=f"epsT_{i}",
                                 tag="epsT", bufs=2)
                nc.sync.dma_start(
                    epsT[:],
                    eps_d.ap().rearrange("(kt p) (nt c) -> p kt nt c", p=P,
                                         c=NB)[:, :, i, :],
                )
                zT = work.tile([P, L // P, NB], F32, name=f"zT_{i}",
                               tag="zT", bufs=2)
                nc.vector.tensor_tensor(
                    out=zT[:].rearrange("p k c -> p (k c)"),
                    in0=elv[:].rearrange("p k c -> p (k c)"),
                    in1=epsT[:].rearrange("p k c -> p (k c)"),
                    op=ALU.mult,
                )
                nc.vector.tensor_tensor(
                    out=zT[:].rearrange("p k c -> p (k c)"),
                    in0=zT[:].rearrange("p k c -> p (k c)"),
                    in1=smv[:, 0 : L // P, :].rearrange("p k c -> p (k c)"),
                    op=ALU.add,
                )
                st["zT"] = zT

            def sA4(i, st):
                zT = st.pop("zT")
                eT = work.tile([M, NB], mmdt, name=f"eT_{i}", tag="eT",
                               bufs=2)
                for bc in range(NB // P):
                    s_ps = ps_x.tile([P, 512], F32, name=f"sps_{i}_{bc}",
                                     tag="sps", bufs=2)
                    for k in range(L // P):
                        nc.tensor.matmul(
                            s_ps[:, :M],
                            zT[:, k, bc * P : (bc + 1) * P],
                            weights["ctxT"][:, k, :],
                            start=(k == 0), stop=(k == L // P - 1),
                        )
                    negmx = work.tile([P, 1], F32, name=f"nmx_{i}_{bc}",
                                      tag="nmx", bufs=4)
                    nc.vector.tensor_reduce(
                        out=negmx[:], in_=s_ps[:, :M],
                        axis=mybir.AxisListType.X, op=ALU.max, negate=True,
                    )
                    e_sb = work.tile([P, M], F32, name=f"e_{i}_{bc}",
                                     tag="e", bufs=4)
                    se = work.tile([P, 1], F32, name=f"se_{i}_{bc}",
                                   tag="se", bufs=4)
                    nc.scalar.activation(
                        out=e_sb[:], in_=s_ps[:, :M], func=AF.Exp,
                        bias=negmx[:], scale=1.0, accum_out=se[:],
                    )
                    rs01 = work.tile([P, 1], F32, name=f"r01_{i}_{bc}",
                                     tag="r01", bufs=4)
                    nc.vector.reciprocal(out=rs01[:], in_=se[:])
                    nc.vector.tensor_scalar_mul(out=rs01[:], in0=rs01[:],
                                                scalar1=0.1)
                    nc.vector.tensor_scalar(
                        out=e_sb[:], in0=e_sb[:], scalar1=rs01[:],
                        scalar2=None, op0=ALU.mult,
                    )
                    trE = ps_x.tile([P, 512], F32, name=f"trE_{i}_{bc}",
                                    tag="sps", bufs=2)
                    nc.tensor.transpose(trE[:M, :P], e_sb[:], ident[:])
                    nc.vector.tensor_copy(
                        eT[:, bc * P : (bc + 1) * P], trE[:M, :P]
                    )
                za_ps = ps_x.tile([P, 2, NB], F32, name=f"za_{i}",
                                  tag="sps", bufs=2)
                for c in range(L // P):
                    nc.tensor.matmul(
                        za_ps[:, c, :],
                        weights["ctxm"][:, c * P : (c + 1) * P],
                        eT[:],
                        start=True, stop=True,
                    )
                zeT = work.tile([P, L // P, NB], mmdt, name=f"zeT_{i}",
                                tag="zeT", bufs=2)
                nc.vector.tensor_tensor(
                    out=zeT[:].rearrange("p k c -> p (k c)"),
                    in0=zT[:].rearrange("p k c -> p (k c)"),
                    in1=za_ps[:].rearrange("p k c -> p (k c)"),
                    op=ALU.add,
                )
                nc.sync.dma_start(zenh_s[i], zeT[:])
                st.clear()

            sw_pipeline([sA0, sA1, sA2, sA3, sA4], NST, ORDER_A)

        # ================= PHASE B =================
        wpC = glob.enter_context(tc.tile_pool(name="wC", bufs=1))
        with ExitStack() as ph:
            work = ph.enter_context(tc.tile_pool(name="workB", bufs=2))
            ps_mm = ph.enter_context(
                tc.tile_pool(name="psB", bufs=1, space="PSUM")
            )
            wdi = prefetch["wdi"]
            wd1 = prefetch["wd1"]

            def sB0(i, st):
                if i == 1:
                    prefetch["wd2"] = load_w(wpC, wd2_d, H // P, 2 * H,
                                             "wd2")
                zeT = work.tile([P, L // P, NB], mmdt, name=f"zeB_{i}",
                                tag="zeB", bufs=3)
                nc.sync.dma_start(zeT[:], zenh_s[i])
                st["zeT"] = zeT

            def sB1(i, st):
                st["d1T"] = act_lrelu(
                    work, mmF(ps_mm, wdi, st.pop("zeT"), L // P, H // P,
                              f"di_{i}"),
                    H // P, f"di_{i}", "di",
                )

            def sB2(i, st):
                d2T = act_lrelu(
                    work, mmF(ps_mm, wd1, st.pop("d1T"), H // P, H // P,
                              f"d1_{i}"),
                    H // P, f"d1_{i}", "d1",
                )
                nc.sync.dma_start(d2t_s[i], d2T[:])
                st.clear()

            sw_pipeline([sB0, sB1, sB2], NST, ORDER_B)
        wrapB.close()  # release di/dec1 weights before phase C pools

        # ================= PHASE C =================
        wpC2 = glob.enter_context(tc.tile_pool(name="wC2", bufs=1))
        prefetch["wd3"] = load_w(wpC2, wd3_d, 2 * H // P, D, "wd3")
        with ExitStack() as ph:
            work = ph.enter_context(tc.tile_pool(name="workC", bufs=2))
            ps_mm = ph.enter_context(
                tc.tile_pool(name="psC", bufs=1, space="PSUM")
            )
            ps_x = ph.enter_context(
                tc.tile_pool(name="psCx", bufs=1, space="PSUM")
            )
            wd2 = prefetch["wd2"]
            wd3 = prefetch["wd3"]

            def sC0(i, st):
                d2T = work.tile([P, H // P, NB], mmdt, name=f"d2C_{i}",
                                tag="d2C", bufs=3)
                nc.sync.dma_start(d2T[:], d2t_s[i])
                st["d2T"] = d2T

            def sC1(i, st):
                chunks = mmF(ps_mm, wd2, st.pop("d2T"), H // P,
                             2 * H // P, f"d2_{i}")
                st["d3T"] = act_lrelu(work, chunks, 2 * H // P, f"d2_{i}",
                                      "d2")
                st["rsb4"] = rs_row(work, ps_x, chunks, 2 * H, f"d2_{i}")

            def sC2(i, st):
                chunks = mmF(ps_mm, wd3, st.pop("d3T"), 2 * H // P,
                             D // P, f"d3_{i}")
                rsb4 = st.pop("rsb4")
                recon = work.tile([P, D // P, NB], F32, name=f"rec_{i}",
                                  tag="rec", bufs=2)
                for ps, m0, msz in chunks:
                    nc.vector.tensor_tensor(
                        out=recon[:, m0 : m0 + msz, :].rearrange(
                            "p k c -> p (k c)"
                        ),
                        in0=ps[:, : msz * NB],
                        in1=rsb4[:].rearrange("p k c -> p (k c)")[
                            :, : msz * NB
                        ],
                        op=ALU.mult,
                    )
                rec_ap = recon_d.ap().rearrange(
                    "(c p) (nt b) -> p c nt b", p=P, b=NB
                )[:, :, i, :]
                nc.sync.dma_start(rec_ap, recon[:])
                st.clear()

            sw_pipeline([sC0, sC1, sC2], NST, ORDER_C)

    nc.finalize()
    return nc


_NC_CACHE = {}


def _get_nc(simple=True):
    key = ("simple" if simple else "general", str(MM_DTYPE))
    if key not in _NC_CACHE:
        _NC_CACHE[key] = (
            _build_v3() if simple else _build_v2(simple=False)
        )
    return _NC_CACHE[key]


def kernel(**inputs):
    i = {
        k: np.ascontiguousarray(np.asarray(v, dtype=np.float32))
        for k, v in inputs.items()
    }
    zeros = all(
        not np.any(i[k])
        for k in (
            "enc_b1", "enc_b2", "mu_b", "lv_b", "di_b", "dec_b1", "dec_b2",
            "dec_b3", "ln1_b", "ln2_b", "dln1_b", "dln2_b",
        )
    )
    units = all(
        np.all(i[k] == 1.0) for k in ("ln1_g", "ln2_g", "dln1_g", "dln2_g")
    )
    simple = zeros and units
    nc = _get_nc(simple=simple)

    def _ct(w):  # transpose + center over out-features (folds LN mean)
        wt = np.ascontiguousarray(w.T)
        return wt - wt.mean(axis=1, keepdims=True)

    shared = {
        "w1t": _ct(i["enc_w1"]) if simple
        else np.ascontiguousarray(i["enc_w1"].T),
        "b1": i["enc_b1"],
        "g1": i["ln1_g"],
        "be1": i["ln1_b"],
        "w2t": _ct(i["enc_w2"]) if simple
        else np.ascontiguousarray(i["enc_w2"].T),
        "b2": i["enc_b2"],
        "g2": i["ln2_g"],
        "be2": i["ln2_b"],
        "wmvt": np.ascontiguousarray(
            np.concatenate([i["mu_w"].T, i["lv_w"].T], axis=1)
        ),
        "bmv": np.concatenate([i["mu_b"], i["lv_b"]]),
        "ctxT": np.ascontiguousarray(i["ctx_mem"].T),
        "ctx": i["ctx_mem"],
        "wdit": np.ascontiguousarray(i["di_w"].T),
        "dib": i["di_b"],
        "wd1t": _ct(i["dec_w1"]) if simple
        else np.ascontiguousarray(i["dec_w1"].T),
        "db1": i["dec_b1"],
        "dg1": i["dln1_g"],
        "dbe1": i["dln1_b"],
        "wd2t": _ct(i["dec_w2"]) if simple
        else np.ascontiguousarray(i["dec_w2"].T),
        "db2": i["dec_b2"],
        "dg2": i["dln2_g"],
        "dbe2": i["dln2_b"],
        "wd3t": np.ascontiguousarray(i["dec_w3"].T),
        "db3": i["dec_b3"],
    }
    in_names = {
        alloc.memorylocations[0].name
        for alloc in nc.m.functions[0].allocations
        if isinstance(alloc, mybir.MemoryLocationSet)
        and alloc.kind == "ExternalInput"
    }
    shared = {k: v for k, v in shared.items() if k in in_names}
    in_maps = []
    for c in range(N_CORES):
        m = dict(shared)
        xc = i["x"][c * B_LOC : (c + 1) * B_LOC]
        ec = i["eps"][c * B_LOC : (c + 1) * B_LOC]
        m["x"] = np.ascontiguousarray(xc.T)
        m["eps"] = np.ascontiguousarray(ec.T) if simple else ec
        in_maps.append(m)

    res = run_bass_kernel_spmd(nc, in_maps, core_ids=list(range(N_CORES)))
    if simple:
        recon = np.concatenate(
            [r["recon"].T for r in res.results], axis=0
        )
        mu = np.concatenate([r["mu"].T for r in res.results], axis=0)
        lv = np.concatenate([r["lv"].T for r in res.results], axis=0)
    else:
        recon = np.concatenate([r["recon"] for r in res.results], axis=0)
        mu = np.concatenate([r["mu"] for r in res.results], axis=0)
        lv = np.concatenate([r["lv"] for r in res.results], axis=0)
    return recon, mu, lv


# revision 46
# speedup vs baseline: 1.9765x; 1.0159x over previous
"""Trainium2 Bass kernel for EnhancedMLPDenoisingVAE.

Pure data parallel over 8 NeuronCores (4096 rows each). Activations are
batch-major ([128 batch rows on partitions, features free]); each matmul
consumes a PE-transposed copy of its input (features on partitions) as
the stationary operand:

    psum = xT.T @ W            K-tiled fp32r matmuls, fp32 accumulate
    LN stats from PSUM         bn_stats/bn_aggr on DVE
    h = Prelu(psum*rs - mu*rs) one wide ScalarE op per 512-chunk
                               (LN normalize + leaky-relu fused, 0.2)
    hT = PE-transpose(h)       fp32r transposes into shared 512-wide
                               PSUM groups, wide DVE copies out

When LN gamma/beta are not (1, 0) or a layer bias is nonzero (never the
case for this model's setup_inputs), per-layer fallbacks reproduce the
general math: bias is added via a broadcast tile on DVE, and gamma/beta
are applied per transposed chunk on ScalarE where they are per-partition
scalars.

Weights live in SBUF in three sequential phases (encoder / di+dec1 /
dec2+dec3); activations stage through DRAM between phases. fp32r
matmuls give ~4e-4 max rel err end to end; MM_DTYPE=float32 is the
full-precision fallback at 4x PE cost.
"""

from contextlib import ExitStack

import numpy as np

import concourse.bass as bass
import concourse.tile as tile
from concourse import bacc, mybir
from concourse.bass_utils import run_bass_kernel_spmd
from concourse.masks import make_identity

F32 = mybir.dt.float32
F32R = mybir.dt.float32r
AF = mybir.ActivationFunctionType
ALU = mybir.AluOpType

B, D, H, L, M = 32768, 768, 1024, 256, 32
N_CORES = 8
B_LOC = B // N_CORES  # 4096
P = 128
NT = B_LOC // P  # 32 row tiles per core
LN_EPS = 1e-5
ALPHA = 0.2

MM_DTYPE = F32R


def _chunks(nf, sz=512):
    return [(s, min(sz, nf - s)) for s in range(0, nf, sz)]


def _build_v2(simple=True, mm_dtype=None, wbufs=(2, 2, 2), psbufs=6, trbufs=2,
           sbufs=4, ORDER_A=None, ORDER_B=None, ORDER_C=None):
    ORDER_A = ORDER_A or [0, 1, 2, 3, 4, 5, 6]
    ORDER_B = ORDER_B or [4, 3, 2, 1, 0]
    ORDER_C = ORDER_C or [0, 1, 2, 3]
    """simple=True assumes all biases zero and LN gamma=1/beta=0 (true for
    this model's setup_inputs); simple=False emits the general math."""
    if isinstance(wbufs, int):
        wbufs = (wbufs, wbufs, wbufs)
    mmdt = MM_DTYPE if mm_dtype is None else mm_dtype
    nc = bacc.Bacc(
        "TRN2", target_bir_lowering=False, debug=False, num_devices=N_CORES
    )

    dram = lambda name, shape, dt=F32, kind="ExternalInput": nc.dram_tensor(
        name, shape, dt, kind=kind
    )
    x_d = dram("x", [D, B_LOC])
    eps_d = dram("eps", [B_LOC, L])
    w1_d = dram("w1t", [D, H])
    b1_d = dram("b1", [H])
    g1_d = dram("g1", [H])
    be1_d = dram("be1", [H])
    w2_d = dram("w2t", [H, H])
    b2_d = dram("b2", [H])
    g2_d = dram("g2", [H])
    be2_d = dram("be2", [H])
    wmv_d = dram("wmvt", [H, 2 * L])
    bmv_d = dram("bmv", [2 * L])
    ctxT_d = dram("ctxT", [L, M])
    ctx_d = dram("ctx", [M, L])
    wdi_d = dram("wdit", [L, H])
    dib_d = dram("dib", [H])
    wd1_d = dram("wd1t", [H, H])
    db1_d = dram("db1", [H])
    dg1_d = dram("dg1", [H])
    dbe1_d = dram("dbe1", [H])
    wd2_d = dram("wd2t", [H, 2 * H])
    db2_d = dram("db2", [2 * H])
    dg2_d = dram("dg2", [2 * H])
    dbe2_d = dram("dbe2", [2 * H])
    wd3_d = dram("wd3t", [2 * H, D])
    db3_d = dram("db3", [D])

    recon_d = dram("recon", [B_LOC, D], kind="ExternalOutput")
    mu_d = dram("mu", [B_LOC, L], kind="ExternalOutput")
    lv_d = dram("lv", [B_LOC, L], kind="ExternalOutput")

    with tile.TileContext(nc, pool_alloc_mode="queue") as tc, ExitStack() as glob:
        const = glob.enter_context(tc.tile_pool(name="const", bufs=1))
        dstash = glob.enter_context(
            tc.tile_pool(name="dstash", bufs=1, space="DRAM")
        )
        ident = const.tile([P, P], F32)
        make_identity(nc, ident)
        identr = const.tile([P, P], F32R)
        nc.vector.tensor_copy(identr[:], ident[:])
        epsln = const.tile([P, 1], F32)
        nc.vector.memset(epsln, LN_EPS)

        zenh_s = dstash.tile([NT, P, L // P, P], mmdt)
        d2t_s = dstash.tile([NT, P, 8, P], mmdt)
        wpB = glob.enter_context(tc.tile_pool(name="wB", bufs=1))
        prefetch = {}

        # ---------- helpers ----------
        def load_w(pool, dram_t, kt, nf, name):
            t = pool.tile([P, kt, nf], mmdt, name=name)
            nc.gpsimd.dma_start(
                t[:], dram_t.ap().rearrange("(kt p) n -> p kt n", p=P)
            )
            return t

        def load_bcast(pool, dram_t, nf, name):
            t = pool.tile([P, nf], F32, name=name)
            src = bass.AP(
                tensor=dram_t.ap().tensor, offset=0, ap=[[0, P], [1, nf]]
            )
            nc.gpsimd.dma_start(t[:], src)
            return t

        def load_packed(pool, dram_t, kt, name):
            t = pool.tile([P, kt], F32, name=name)
            nc.sync.dma_start(t[:], dram_t.ap().rearrange("(c p) -> p c", p=P))
            return t

        def mm_chunks(ps_pool, xT, w_sb, kt, tag):
            """Returns [(psum_tile, n0, nsz)] for all 512-chunks."""
            nf = w_sb.shape[2]
            out = []
            for ci, (n0, nsz) in enumerate(_chunks(nf)):
                ps = ps_pool.tile(
                    [P, 512], F32, name=f"ps_{tag}_{ci}", tag="mmps",
                    bufs=psbufs,
                )
                for k in range(kt):
                    nc.tensor.matmul(
                        ps[:, :nsz],
                        xT[:, k, :],
                        w_sb[:, k, n0 : n0 + nsz],
                        start=(k == 0),
                        stop=(k == kt - 1),
                    )
                out.append((ps, n0, nsz))
            return out

        def ln_stats(work, srcs, tag):
            """bn stats over chunk aps -> (negmu [P,1], mv [P,2])."""
            nsub = len(srcs)
            stats = work.tile(
                [P, nsub, 6], F32, name=f"st_{tag}", tag="stats", bufs=sbufs
            )
            for s, (src, n0, nsz) in enumerate(srcs):
                nc.vector.bn_stats(out=stats[:, s, :], in_=src[:, :nsz])
            mv = work.tile([P, 2], F32, name=f"mv_{tag}", tag="mv", bufs=sbufs)
            nc.vector.bn_aggr(out=mv[:], in_=stats[:])
            nmu = work.tile(
                [P, 1], F32, name=f"nmu_{tag}", tag="nmu", bufs=sbufs
            )
            nc.vector.tensor_scalar_mul(out=nmu[:], in0=mv[:, 0:1],
                                        scalar1=-1.0)
            return nmu, mv

        def rsqrt_dve(work, mv, tag):
            """rs = 1/sqrt(var + eps) via bit-trick + 2 Newton iters (DVE
            only -- keeps Sqrt off ScalarE so its LUT set never swaps)."""
            I32 = mybir.dt.int32
            v1 = work.tile([P, 1], F32, name=f"v1_{tag}", tag="v1",
                           bufs=sbufs)
            nc.vector.tensor_scalar_add(out=v1[:], in0=mv[:, 1:2],
                                        scalar1=LN_EPS)
            ti = work.tile([P, 1], I32, name=f"ti_{tag}", tag="ti",
                           bufs=sbufs)
            nc.vector.tensor_scalar(
                out=ti[:], in0=v1[:].bitcast(I32), scalar1=1, scalar2=None,
                op0=ALU.logical_shift_right,
            )
            nc.vector.tensor_scalar(
                out=ti[:], in0=ti[:], scalar1=-1, scalar2=0x5F3759DF,
                op0=ALU.mult, op1=ALU.add,
            )
            y = work.tile([P, 1], F32, name=f"yq_{tag}", tag="yq",
                          bufs=sbufs)
            nc.vector.tensor_copy(y[:], ti[:].bitcast(F32))
            hv = work.tile([P, 1], F32, name=f"hv_{tag}", tag="hv",
                           bufs=sbufs)
            nc.vector.tensor_scalar_mul(out=hv[:], in0=v1[:], scalar1=0.5)
            tq = work.tile([P, 1], F32, name=f"tq_{tag}", tag="tq",
                           bufs=sbufs)
            for _ in range(2):
                nc.vector.tensor_tensor(out=tq[:], in0=y[:], in1=y[:],
                                        op=ALU.mult)
                nc.vector.tensor_tensor(out=tq[:], in0=tq[:], in1=hv[:],
                                        op=ALU.mult)
                nc.vector.tensor_scalar(
                    out=tq[:], in0=tq[:], scalar1=-1.0, scalar2=1.5,
                    op0=ALU.mult, op1=ALU.add,
                )
                nc.vector.tensor_tensor(out=y[:], in0=y[:], in1=tq[:],
                                        op=ALU.mult)
            return y

        def transpose_in(work, ps_pool, src_sb, kt, out_dt, idt, tag,
                         grp=None):
            grp = grp or tag
            """src [P, kt*128] -> [P, kt, 128] via PE transposes grouped
            into 512-wide PSUM tiles + wide DVE copies."""
            out = work.tile(
                [P, kt, P], out_dt, name=f"t_{tag}", tag=f"t_{grp}",
                bufs=(sbufs if kt <= 8 else 2),
            )
            for g0 in range(0, kt, 4):
                gn = min(4, kt - g0)
                pw = ps_pool.tile(
                    [P, 512], F32, name=f"tw_{tag}_{g0}", tag="trps",
                    bufs=trbufs,
                )
                for j in range(gn):
                    dst = pw[:, j * P : (j + 1) * P]
                    src_c = src_sb[:, (g0 + j) * P : (g0 + j + 1) * P]
                    if src_c.dtype == F32R:
                        dst = dst.bitcast(F32R)
                    nc.tensor.transpose(dst, src_c, idt)
                nc.vector.tensor_copy(
                    out[:, g0 : g0 + gn, :].rearrange("p k c -> p (k c)"),
                    pw[:, : gn * P],
                )
            return out

        def dense_act(work, ps_tr, srcs, nf, tag, grp, *, ln, need_rs,
                      out_kt):
            """activate (LN shift + lrelu) + transpose half of a layer."""
            h = work.tile(
                [P, nf], mmdt, name=f"h_{tag}", tag=f"h{nf}", bufs=2
            )
            rs = None
            if ln:
                nmu, mv = ln_stats(work, srcs, tag)
                if need_rs:
                    rs = rsqrt_dve(work, mv, tag)
                for src, n0, nsz in srcs:
                    nc.scalar.activation(
                        out=h[:, n0 : n0 + nsz], in_=src[:, :nsz],
                        func=AF.Prelu, bias=nmu[:], scale=1.0, alpha=ALPHA,
                    )
            else:
                for src, n0, nsz in srcs:
                    nc.scalar.activation(
                        out=h[:, n0 : n0 + nsz], in_=src[:, :nsz],
                        func=AF.Prelu, bias=0.0, scale=1.0, alpha=ALPHA,
                    )
            tT = transpose_in(
                work, ps_tr, h, out_kt, mmdt,
                identr if mmdt == F32R else ident, f"{tag}T",
                grp=f"{grp}T",
            )
            return (tT, rs) if need_rs else tT

        def dense_fast(work, ps_mm, ps_tr, xT, w_sb, kt, tag, *,
                       ln, lrelu, out_kt, need_rs=False, grp=None):
            grp = grp or tag
            """simple-path layer: matmul -> (LN shift) -> lrelu ->
            transposed fp32r copy. The LN 1/std factor is NOT applied
            here: leaky-relu is positively homogeneous and LN is
            scale-invariant per sample, so the factor cancels through
            the next LN; layers feeding non-LN consumers get it back
            via need_rs (folded into the consumer's PSUM copy)."""
            srcs = mm_chunks(ps_mm, xT, w_sb, kt, tag)
            nf = w_sb.shape[2]
            h = work.tile(
                [P, nf], mmdt, name=f"h_{tag}", tag=f"h{nf}", bufs=2
            )
            rs = None
            if ln:
                nmu, mv = ln_stats(work, srcs, tag)
                if need_rs:
                    rs = rsqrt_dve(work, mv, tag)
                for src, n0, nsz in srcs:
                    nc.scalar.activation(
                        out=h[:, n0 : n0 + nsz], in_=src[:, :nsz],
                        func=AF.Prelu, bias=nmu[:], scale=1.0, alpha=ALPHA,
                    )
            else:
                assert lrelu
                for src, n0, nsz in srcs:
                    nc.scalar.activation(
                        out=h[:, n0 : n0 + nsz], in_=src[:, :nsz],
                        func=AF.Prelu, bias=0.0, scale=1.0, alpha=ALPHA,
                    )
            tT = transpose_in(
                work, ps_tr, h, out_kt, mmdt,
                identr if mmdt == F32R else ident, f"{tag}T",
                grp=f"{grp}T",
            )
            return (tT, rs) if need_rs else tT

        def dense_general(work, ps_mm, ps_tr, xT, w_sb, kt, tag, *,
                          ln, lrelu, out_kt, bias_bc, gp, bep, grp=None):
            grp = grp or tag
            """general-path layer (nonzero bias / non-unit gamma):
            y = psum + bias; xhat = (y-mu)*rs; transpose; per-chunk
            ScalarE Prelu(xhat*g + beta)."""
            srcs = mm_chunks(ps_mm, xT, w_sb, kt, tag)
            nf = w_sb.shape[2]
            y = work.tile([P, nf], F32, name=f"y_{tag}", tag=f"y{nf}", bufs=2)
            for src, n0, nsz in srcs:
                if bias_bc is not None:
                    nc.vector.tensor_tensor(
                        out=y[:, n0 : n0 + nsz], in0=src[:, :nsz],
                        in1=bias_bc[:, n0 : n0 + nsz], op=ALU.add,
                    )
                else:
                    nc.vector.tensor_copy(y[:, n0 : n0 + nsz], src[:, :nsz])
            xh = y
            if ln:
                nmu, mv = ln_stats(
                    work,
                    [(y[:, n0 : n0 + nsz], n0, nsz) for _, n0, nsz in srcs],
                    tag,
                )
                rs = rsqrt_dve(work, mv, tag)
                xh = work.tile(
                    [P, nf], F32, name=f"xh_{tag}", tag=f"xh{nf}", bufs=2
                )
                nc.vector.tensor_scalar(
                    out=xh[:], in0=y[:], scalar1=nmu[:], scalar2=rs[:],
                    op0=ALU.add, op1=ALU.mult,
                )
            out = work.tile(
                [P, out_kt, P], mmdt, name=f"t_{tag}", tag=f"t_{grp}",
                bufs=2,
            )
            for g0 in range(0, out_kt, 4):
                gn = min(4, out_kt - g0)
                pw = ps_tr.tile(
                    [P, 512], F32, name=f"tw_{tag}_{g0}", tag="trps",
                    bufs=trbufs,
                )
                for j in range(gn):
                    nc.tensor.transpose(
                        pw[:, j * P : (j + 1) * P],
                        xh[:, (g0 + j) * P : (g0 + j + 1) * P],
                        ident,
                    )
                for j in range(gn):
                    k = g0 + j
                    nc.scalar.activation(
                        out=out[:, k, :], in_=pw[:, j * P : (j + 1) * P],
                        func=AF.Prelu if (ln or lrelu) else AF.Identity,
                        bias=bep[:, k : k + 1] if bep is not None else 0.0,
                        scale=gp[:, k : k + 1] if gp is not None else 1.0,
                        alpha=ALPHA,
                    )
            return out

        def raw_out(work, srcs, nf, tag, bias_bc=None, row_scale=None,
                    grp=None):
            o = work.tile([P, nf], F32, name=f"o_{tag}", tag=f"o_{grp or tag}",
                          bufs=2)
            for src, n0, nsz in srcs:
                if bias_bc is not None:
                    nc.vector.tensor_tensor(
                        out=o[:, n0 : n0 + nsz], in0=src[:, :nsz],
                        in1=bias_bc[:, n0 : n0 + nsz], op=ALU.add,
                    )
                elif row_scale is not None:
                    nc.vector.tensor_scalar(
                        out=o[:, n0 : n0 + nsz], in0=src[:, :nsz],
                        scalar1=row_scale[:], scalar2=None, op0=ALU.mult,
                    )
                else:
                    nc.vector.tensor_copy(o[:, n0 : n0 + nsz], src[:, :nsz])
            return o

        def sw_pipeline(stage_fns, n, order=None):
            """Software-pipelined emission: the Tile scheduler is a
            priority-list scheduler, so per-engine execution order tracks
            emission order -- interleaving stages of neighboring row-tiles
            here is what lets PE run tile j+1 matmuls while tile j's
            LN/softmax chain is on DVE/ScalarE. `order` sets the
            intra-tick stage emission order (default deepest-first)."""
            S = len(stage_fns)
            if order is None:
                order = list(range(S - 1, -1, -1))
            states = [dict() for _ in range(n)]
            for t in range(n + S - 1):
                for s in order:
                    j = t - s
                    if 0 <= j < n:
                        stage_fns[s](j, states[j])

        # ================= PHASE A: encoder =================
        with ExitStack() as ph:
            wp = ph.enter_context(tc.tile_pool(name="wA", bufs=1))
            work = ph.enter_context(tc.tile_pool(name="workA", bufs=wbufs[0]))
            ps_mm = ph.enter_context(
                tc.tile_pool(name="psA", bufs=1, space="PSUM")
            )
            ps_tr = ph.enter_context(
                tc.tile_pool(name="psAt", bufs=1, space="PSUM")
            )
            w1 = load_w(wp, w1_d, D // P, H, "w1")
            w2 = load_w(wp, w2_d, H // P, H, "w2")
            wmv = load_w(wp, wmv_d, H // P, 2 * L, "wmv")
            ctxT = wp.tile([P, 2, M], F32, name="ctxT")
            nc.sync.dma_start(
                ctxT[:], ctxT_d.ap().rearrange("(kt p) n -> p kt n", p=P)
            )
            ctxm = wp.tile([M, L], mmdt, name="ctxm")
            nc.gpsimd.dma_start(ctxm[:], ctx_d.ap())
            if not simple:
                b1c = load_bcast(wp, b1_d, H, "b1c")
                b2c = load_bcast(wp, b2_d, H, "b2c")
                bmvc = load_bcast(wp, bmv_d, 2 * L, "bmvc")
                g1p = load_packed(wp, g1_d, H // P, "g1p")
                be1p = load_packed(wp, be1_d, H // P, "be1p")
                g2p = load_packed(wp, g2_d, H // P, "g2p")
                be2p = load_packed(wp, be2_d, H // P, "be2p")

            def sA0(i, st):
                if i == 2:
                    prefetch["wdi"] = load_w(wpB, wdi_d, L // P, H, "wdi")
                    prefetch["wd1"] = load_w(wpB, wd1_d, H // P, H, "wd1")
                r0 = i * P
                xT = work.tile([P, D // P, P], mmdt, name=f"xT_{i}",
                               tag="xT", bufs=sbufs)
                nc.gpsimd.dma_start(
                    xT[:],
                    x_d.ap().rearrange("(kt p) (nt c) -> p kt nt c", p=P,
                                       c=P)[:, :, i, :],
                )
                st["xT"] = xT

            def sA1a(i, st):
                if simple:
                    st["l1s"] = mm_chunks(ps_mm, st["xT"], w1, D // P,
                                          f"l1_{i}")
                else:
                    st["h1T"] = dense_general(
                        work, ps_mm, ps_tr, st["xT"], w1, D // P, f"l1_{i}",
                        ln=True, lrelu=True, out_kt=H // P,
                        bias_bc=b1c, gp=g1p, bep=be1p, grp="l1",
                    )

            def sA1b(i, st):
                if simple:
                    st["h1T"] = dense_act(
                        work, ps_tr, st.pop("l1s"), H, f"l1_{i}", "l1",
                        ln=True, need_rs=False, out_kt=H // P,
                    )

            def sA2a(i, st):
                if simple:
                    st["l2s"] = mm_chunks(ps_mm, st["h1T"], w2, H // P,
                                          f"l2_{i}")
                else:
                    st["h2T"] = dense_general(
                        work, ps_mm, ps_tr, st["h1T"], w2, H // P, f"l2_{i}",
                        ln=True, lrelu=True, out_kt=H // P,
                        bias_bc=b2c, gp=g2p, bep=be2p, grp="l2",
                    )
                    st["rs2"] = None

            def sA2b(i, st):
                if simple:
                    st["h2T"], st["rs2"] = dense_act(
                        work, ps_tr, st.pop("l2s"), H, f"l2_{i}", "l2",
                        ln=True, need_rs=True, out_kt=H // P,
                    )

            def sA3(i, st):
                r0 = i * P
                if simple:
                    smv = raw_out(
                        work, mm_chunks(ps_mm, st["h2T"], wmv, H // P,
                                        f"mv_{i}"),
                        2 * L, f"mv_{i}", row_scale=st["rs2"], grp="mv",
                    )
                else:
                    smv = raw_out(
                        work, mm_chunks(ps_mm, st["h2T"], wmv, H // P,
                                        f"mv_{i}"),
                        2 * L, f"mv_{i}", bias_bc=bmvc, grp="mv",
                    )
                nc.sync.dma_start(mu_d[r0 : r0 + P, :], smv[:, :L])
                nc.sync.dma_start(lv_d[r0 : r0 + P, :], smv[:, L:])

                elv = work.tile([P, L], F32, name=f"elv_{i}", tag="elv",
                                bufs=sbufs)
                nc.scalar.activation(
                    out=elv[:], in_=smv[:, L:], func=AF.Exp, bias=0.0,
                    scale=0.5,
                )
                eps_sb = work.tile([P, L], F32, name=f"eps_{i}", tag="eps",
                                   bufs=sbufs)
                nc.sync.dma_start(eps_sb[:], eps_d[r0 : r0 + P, :])
                z_sb = work.tile([P, L], F32, name=f"z_{i}", tag="z",
                                 bufs=sbufs)
                nc.vector.tensor_tensor(
                    out=z_sb[:], in0=elv[:], in1=eps_sb[:], op=ALU.mult
                )
                nc.vector.tensor_tensor(
                    out=z_sb[:], in0=z_sb[:], in1=smv[:, :L], op=ALU.add
                )
                st["z"] = z_sb

            def sA4(i, st):
                z_sb = st["z"]
                zT = transpose_in(work, ps_tr, z_sb, L // P, F32, ident,
                                  f"zT{i}", grp="zT")
                s_ps = ps_mm.tile([P, 512], F32, name=f"sps_{i}", tag="mmps",
                                  bufs=psbufs)
                for k in range(L // P):
                    nc.tensor.matmul(
                        s_ps[:, :M], zT[:, k, :], ctxT[:, k, :],
                        start=(k == 0), stop=(k == L // P - 1),
                    )
                negmx = work.tile([P, 1], F32, name=f"nmx_{i}", tag="nmx",
                                  bufs=sbufs)
                nc.vector.tensor_reduce(
                    out=negmx[:], in_=s_ps[:, :M],
                    axis=mybir.AxisListType.X, op=ALU.max, negate=True,
                )
                e_sb = work.tile([P, M], F32, name=f"e_{i}", tag="e",
                                 bufs=sbufs)
                se = work.tile([P, 1], F32, name=f"se_{i}", tag="se",
                               bufs=sbufs)
                nc.scalar.activation(
                    out=e_sb[:], in_=s_ps[:, :M], func=AF.Exp,
                    bias=negmx[:], scale=1.0, accum_out=se[:],
                )
                rs01 = work.tile([P, 1], F32, name=f"r01_{i}", tag="r01",
                                 bufs=sbufs)
                nc.vector.reciprocal(out=rs01[:], in_=se[:])
                nc.vector.tensor_scalar_mul(
                    out=rs01[:], in0=rs01[:], scalar1=0.1
                )
                # e_n = e * (0.1 / sum): fold attn normalization here so the
                # transposed context matmul needs no per-column scale
                nc.vector.tensor_scalar(
                    out=e_sb[:], in0=e_sb[:], scalar1=rs01[:], scalar2=None,
                    op0=ALU.mult,
                )
                trE = ps_tr.tile([P, 512], F32, name=f"trE_{i}", tag="trps",
                                 bufs=trbufs)
                nc.tensor.transpose(trE[:M, :P], e_sb[:], ident[:])
                eT = work.tile([M, P], mmdt, name=f"eT_{i}", tag="eT",
                               bufs=sbufs)
                nc.vector.tensor_copy(eT[:], trE[:M, :P])
                # z_addT[l_chunk, b] = ctx[:, l_chunk].T @ e_n.T  (fp32r)
                za_ps = ps_tr.tile([P, 512], F32, name=f"zaps_{i}",
                                   tag="trps", bufs=trbufs)
                for c in range(L // P):
                    nc.tensor.matmul(
                        za_ps[:, c * P : (c + 1) * P],
                        ctxm[:, c * P : (c + 1) * P],
                        eT[:],
                        start=True, stop=True,
                    )
                zeT = work.tile([P, L // P, P], mmdt, name=f"zeT_{i}",
                                tag="zeTA", bufs=sbufs)
                nc.vector.tensor_tensor(
                    out=zeT[:].rearrange("p k c -> p (k c)"),
                    in0=zT[:].rearrange("p k c -> p (k c)"),
                    in1=za_ps[:, :L],
                    op=ALU.add,
                )
                nc.sync.dma_start(zenh_s[i], zeT[:])
                st.clear()

            sw_pipeline([sA0, sA1a, sA1b, sA2a, sA2b, sA3, sA4], NT,
                        order=ORDER_A)

        # ================= PHASE B: di + dec1 =================
        wpC = glob.enter_context(tc.tile_pool(name="wC", bufs=1))
        with ExitStack() as ph:
            work = ph.enter_context(tc.tile_pool(name="workB", bufs=wbufs[1]))
            ps_mm = ph.enter_context(
                tc.tile_pool(name="psB", bufs=1, space="PSUM")
            )
            ps_tr = ph.enter_context(
                tc.tile_pool(name="psBt", bufs=1, space="PSUM")
            )
            wdi = prefetch["wdi"]
            wd1 = prefetch["wd1"]
            if not simple:
                dibp = load_packed(wpB, dib_d, H // P, "dibp")
                db1c = load_bcast(wpB, db1_d, H, "db1c")
                dg1p = load_packed(wpB, dg1_d, H // P, "dg1p")
                dbe1p = load_packed(wpB, dbe1_d, H // P, "dbe1p")

            def sB0(i, st):
                if i == 2:
                    prefetch["wd2"] = load_w(wpC, wd2_d, H // P, 2 * H, "wd2")
                    prefetch["wd3"] = load_w(wpC, wd3_d, 2 * H // P, D, "wd3")
                zeT = work.tile([P, L // P, P], mmdt, name=f"zeB_{i}",
                                tag="zeB", bufs=sbufs)
                nc.sync.dma_start(zeT[:], zenh_s[i])
                st["zeT"] = zeT

            def sB1a(i, st):
                if simple:
                    st["dis"] = mm_chunks(ps_mm, st["zeT"], wdi, L // P,
                                          f"di_{i}")
                else:
                    st["d1T"] = dense_general(
                        work, ps_mm, ps_tr, st["zeT"], wdi, L // P, f"di_{i}",
                        ln=False, lrelu=True, out_kt=H // P,
                        bias_bc=None, gp=None, bep=dibp, grp="di",
                    )

            def sB1b(i, st):
                if simple:
                    st["d1T"] = dense_act(
                        work, ps_tr, st.pop("dis"), H, f"di_{i}", "di",
                        ln=False, need_rs=False, out_kt=H // P,
                    )

            def sB2a(i, st):
                if simple:
                    st["d1s"] = mm_chunks(ps_mm, st["d1T"], wd1, H // P,
                                          f"d1_{i}")
                else:
                    d2T = dense_general(
                        work, ps_mm, ps_tr, st["d1T"], wd1, H // P, f"d1_{i}",
                        ln=True, lrelu=True, out_kt=H // P,
                        bias_bc=db1c, gp=dg1p, bep=dbe1p, grp="d1",
                    )
                    nc.sync.dma_start(d2t_s[i], d2T[:])
                    st.clear()

            def sB2b(i, st):
                if simple:
                    d2T = dense_act(
                        work, ps_tr, st.pop("d1s"), H, f"d1_{i}", "d1",
                        ln=True, need_rs=False, out_kt=H // P,
                    )
                    nc.sync.dma_start(d2t_s[i], d2T[:])
                    st.clear()

            sw_pipeline([sB0, sB1a, sB1b, sB2a, sB2b], NT,
                        order=ORDER_B)

        # ================= PHASE C: dec2 + dec3 =================
        with ExitStack() as ph:
            work = ph.enter_context(tc.tile_pool(name="workC", bufs=wbufs[2]))
            ps_mm = ph.enter_context(
                tc.tile_pool(name="psC", bufs=1, space="PSUM")
            )
            ps_tr = ph.enter_context(
                tc.tile_pool(name="psCt", bufs=1, space="PSUM")
            )
            wd2 = prefetch["wd2"]
            wd3 = prefetch["wd3"]
            if not simple:
                db2c = load_bcast(wpC, db2_d, 2 * H, "db2c")
                dg2p = load_packed(wpC, dg2_d, 2 * H // P, "dg2p")
                dbe2p = load_packed(wpC, dbe2_d, 2 * H // P, "dbe2p")
                db3c = load_bcast(wpC, db3_d, D, "db3c")

            def sC0(i, st):
                d2T = work.tile([P, 8, P], mmdt, name=f"d2C_{i}", tag="d2C",
                                bufs=3)
                nc.sync.dma_start(d2T[:], d2t_s[i])
                st["d2T"] = d2T

            def sC1a(i, st):
                if simple:
                    st["d2s"] = mm_chunks(ps_mm, st["d2T"], wd2, H // P,
                                          f"d2_{i}")
                else:
                    st["d3T"] = dense_general(
                        work, ps_mm, ps_tr, st["d2T"], wd2, H // P, f"d2_{i}",
                        ln=True, lrelu=True, out_kt=2 * H // P,
                        bias_bc=db2c, gp=dg2p, bep=dbe2p, grp="d2",
                    )
                    st["rs4"] = None

            def sC1b(i, st):
                if simple:
                    st["d3T"], st["rs4"] = dense_act(
                        work, ps_tr, st.pop("d2s"), 2 * H, f"d2_{i}", "d2",
                        ln=True, need_rs=True, out_kt=2 * H // P,
                    )

            def sC2(i, st):
                r0 = i * P
                if simple:
                    recon_sb = raw_out(
                        work,
                        mm_chunks(ps_mm, st["d3T"], wd3, 2 * H // P,
                                  f"d3_{i}"),
                        D, f"d3_{i}", row_scale=st["rs4"], grp="d3",
                    )
                else:
                    recon_sb = raw_out(
                        work,
                        mm_chunks(ps_mm, st["d3T"], wd3, 2 * H // P,
                                  f"d3_{i}"),
                        D, f"d3_{i}", bias_bc=db3c, grp="d3",
                    )
                nc.sync.dma_start(recon_d[r0 : r0 + P, :], recon_sb[:])
                st.clear()

            sw_pipeline([sC0, sC1a, sC1b, sC2], NT, order=ORDER_C)

    nc.finalize()
    return nc


NB = 256  # batch columns per super-tile (moving-dim of fp32r matmuls)
NST = B_LOC // NB  # 16 super-tiles per core


def _build_v3(psbufs=4, trbufs=3, hbufs=2, sq_on_act=True,
              ORDER_A=None, ORDER_B=None, ORDER_C=None):
    """Feature-major dataflow: activations live transposed ([feature
    chunk on partitions, batch free]) end to end, weights are the
    stationary matmul operand, so no PE transposes of activations are
    needed. LN mean-subtraction is folded into host-centered weights
    (W' = W - mean_out(W)); the LN 1/std factor cancels through LN->LN
    chains (leaky-relu is positively homogeneous) and is only computed
    for ln2/dln2 via a ones-matmul over ScalarE-squared chunks, then
    applied per batch column through a PE-broadcast row. Assumes zero
    biases / unit gammas (checked by kernel())."""
    ORDER_A = ORDER_A or [0, 1, 2, 4, 3]
    ORDER_B = ORDER_B or [0, 1, 2]
    ORDER_C = ORDER_C or [0, 1, 2]
    mmdt = F32R
    nc = bacc.Bacc(
        "TRN2", target_bir_lowering=False, debug=False, num_devices=N_CORES
    )
    dram = lambda name, shape, dt=F32, kind="ExternalInput": nc.dram_tensor(
        name, shape, dt, kind=kind
    )
    x_d = dram("x", [D, B_LOC])        # host-transposed
    eps_d = dram("eps", [L, B_LOC])    # host-transposed
    w1_d = dram("w1t", [D, H])         # host-centered
    w2_d = dram("w2t", [H, H])         # host-centered
    wmv_d = dram("wmvt", [H, 2 * L])
    ctxT_d = dram("ctxT", [L, M])
    ctx_d = dram("ctx", [M, L])
    wdi_d = dram("wdit", [L, H])
    wd1_d = dram("wd1t", [H, H])       # host-centered
    wd2_d = dram("wd2t", [H, 2 * H])   # host-centered
    wd3_d = dram("wd3t", [2 * H, D])
    recon_d = dram("recon", [D, B_LOC], kind="ExternalOutput")  # host .T
    mu_d = dram("mu", [L, B_LOC], kind="ExternalOutput")        # host .T
    lv_d = dram("lv", [L, B_LOC], kind="ExternalOutput")        # host .T

    with tile.TileContext(nc) as tc, ExitStack() as glob:
        const = glob.enter_context(tc.tile_pool(name="const", bufs=1))
        dstash = glob.enter_context(
            tc.tile_pool(name="dstash", bufs=1, space="DRAM")
        )
        ident = const.tile([P, P], F32)
        make_identity(nc, ident)
        ones_f = const.tile([P, 1], F32)
        nc.vector.memset(ones_f, 1.0)
        onesr = const.tile([P, 1], F32R)
        nc.vector.tensor_copy(onesr[:], ones_f[:])
        ones_rf = const.tile([1, P], F32)
        nc.vector.memset(ones_rf, 1.0)
        ones_row = const.tile([1, P], F32R)
        nc.vector.tensor_copy(ones_row[:], ones_rf[:])

        zenh_s = dstash.tile([NST, P, L // P, NB], mmdt)
        d2t_s = dstash.tile([NST, P, H // P, NB], mmdt)

        bridge = glob.enter_context(tc.tile_pool(name="bridge", bufs=1))
        wrapB = glob.enter_context(ExitStack())
        wpB = wrapB.enter_context(
            tc.tile_pool(name="wB", bufs=1, side="right")
        )
        prefetch = {}

        def load_w(pool, dram_t, kt, nf, name):
            # one DMA per K-chunk so matmuls can start before the whole
            # weight tile has landed
            t = pool.tile([P, kt, nf], mmdt, name=name)
            src_ap = dram_t.ap().rearrange("(kt p) n -> p kt n", p=P)
            for k in range(kt):
                nc.gpsimd.dma_start(t[:, k : k + 1, :], src_ap[:, k : k + 1, :])
            return t

        def sw_pipeline(stage_fns, n, order):
            S = len(stage_fns)
            states = [dict() for _ in range(n)]
            for t in range(n + S - 1):
                for s in order:
                    j = t - s
                    if 0 <= j < n:
                        stage_fns[s](j, states[j])

        def mmF(ps_pool, w_sb, xT, kt, out_kt, tag):
            """feature-major layer: psum chunk m = sum_k W[:,k,m].T@xT[:,k].
            Two 256-wide chunks share one 512-wide PSUM tile (bank)."""
            chunks = []
            for mp in range((out_kt + 1) // 2):
                ps = ps_pool.tile([P, 512], F32, name=f"ps_{tag}_{mp}",
                                  tag="mmps", bufs=psbufs)
                msz = min(2, out_kt - 2 * mp)
                for sub in range(msz):
                    m = 2 * mp + sub
                    pslice = ps[:, sub * NB : (sub + 1) * NB]
                    for k in range(kt):
                        nc.tensor.matmul(
                            pslice,
                            w_sb[:, k, m * P : (m + 1) * P],
                            xT[:, k, :],
                            start=(k == 0),
                            stop=(k == kt - 1),
                        )
                chunks.append((ps, 2 * mp, msz))
            return chunks

        def act_lrelu(work, chunks, out_kt, tag, grp):
            h = work.tile([P, out_kt, NB], mmdt, name=f"h_{tag}",
                          tag=f"h_{grp}", bufs=hbufs)
            for ps, m0, msz in chunks:
                nc.scalar.activation(
                    out=h[:, m0 : m0 + msz, :].rearrange(
                        "p k c -> p (k c)"
                    ),
                    in_=ps[:, : msz * NB],
                    func=AF.Prelu, bias=0.0, scale=1.0, alpha=ALPHA,
                )
            return h

        def rs_row(work, ps_q, chunks, nf, tag):
            """rs = 1/sqrt(mean(y_c^2)+eps) per batch column, PE-broadcast
            to [P, NB] in SBUF (f32). chunks are this layer's psums."""
            q_ps = ps_q.tile([1, 2, NB], F32, name=f"q_{tag}", tag="qps",
                             bufs=1)
            for pi, (ps, m0, msz) in enumerate(chunks):
                sq = work.tile([P, 512], F32R, name=f"sq_{tag}_{m0}",
                               tag="sq", bufs=2)
                nc.scalar.activation(out=sq[:, : msz * NB],
                                     in_=ps[:, : msz * NB],
                                     func=AF.Square, bias=0.0, scale=1.0)
                nc.tensor.matmul(
                    q_ps[:].rearrange("o k c -> o (k c)")[:, : msz * NB],
                    onesr[:],
                    sq[:, : msz * NB],
                    start=(pi == 0), stop=(pi == len(chunks) - 1),
                )
            I32 = mybir.dt.int32
            q_sb = work.tile([1, 2, NB], F32, name=f"qs_{tag}", tag="qs",
                             bufs=1)
            nc.vector.tensor_copy(
                q_sb[:].rearrange("o k c -> o (k c)"),
                q_ps[:].rearrange("o k c -> o (k c)"),
            )
            qt = work.tile([1, NB], F32, name=f"qt_{tag}", tag="qt", bufs=2)
            nc.vector.tensor_tensor(out=qt[:], in0=q_sb[:, 0, :],
                                    in1=q_sb[:, 1, :], op=ALU.add)
            v1 = work.tile([1, NB], F32, name=f"v1_{tag}", tag="v1", bufs=1)
            nc.vector.tensor_scalar(
                out=v1[:], in0=qt[:], scalar1=1.0 / nf, scalar2=LN_EPS,
                op0=ALU.mult, op1=ALU.add,
            )
            ti = work.tile([1, NB], I32, name=f"ti_{tag}", tag="ti", bufs=1)
            nc.vector.tensor_scalar(
                out=ti[:], in0=v1[:].bitcast(I32), scalar1=1, scalar2=None,
                op0=ALU.logical_shift_right,
            )
            nc.vector.tensor_scalar(
                out=ti[:], in0=ti[:], scalar1=-1, scalar2=0x5F3759DF,
                op0=ALU.mult, op1=ALU.add,
            )
            y = work.tile([1, NB], F32, name=f"yq_{tag}", tag="yq", bufs=1)
            nc.vector.tensor_copy(y[:], ti[:].bitcast(F32))
            hv = work.tile([1, NB], F32, name=f"hv_{tag}", tag="hv", bufs=1)
            nc.vector.tensor_scalar_mul(out=hv[:], in0=v1[:], scalar1=0.5)
            tq = work.tile([1, NB], F32, name=f"tq_{tag}", tag="tq", bufs=1)
            for _ in range(2):
                nc.vector.tensor_tensor(out=tq[:], in0=y[:], in1=y[:],
                                        op=ALU.mult)
                nc.vector.tensor_tensor(out=tq[:], in0=tq[:], in1=hv[:],
                                        op=ALU.mult)
                nc.vector.tensor_scalar(
                    out=tq[:], in0=tq[:], scalar1=-1.0, scalar2=1.5,
                    op0=ALU.mult, op1=ALU.add,
                )
                nc.vector.tensor_tensor(out=y[:], in0=y[:], in1=tq[:],
                                        op=ALU.mult)
            yr = work.tile([1, NB], F32R, name=f"yr_{tag}", tag="yr", bufs=2)
            nc.vector.tensor_copy(yr[:], y[:])
            yr_b = bass.AP(
                tensor=yr.tensor, offset=yr.offset,
                ap=[list(yr.ap[0]), [0, 2], [1, NB]],
            )
            rb_ps = ps_q.tile([P, 2, NB], F32, name=f"rb_{tag}", tag="rbps",
                              bufs=1)
            nc.tensor.matmul(
                rb_ps[:].rearrange("p k c -> p (k c)"),
                ones_row[:],
                yr_b,
                start=True, stop=True,
            )
            rsb = work.tile([P, 2, NB], F32, name=f"rsb_{tag}", tag="rsb",
                            bufs=2)
            nc.vector.tensor_copy(
                rsb[:].rearrange("p k c -> p (k c)"),
                rb_ps[:].rearrange("p k c -> p (k c)"),
            )
            return rsb

        # ================= PHASE A =================
        with ExitStack() as ph:
            wp = ph.enter_context(tc.tile_pool(name="wA", bufs=1))
            work = ph.enter_context(tc.tile_pool(name="workA", bufs=2))
            ps_mm = ph.enter_context(
                tc.tile_pool(name="psA", bufs=1, space="PSUM")
            )
            ps_x = ph.enter_context(
                tc.tile_pool(name="psAx", bufs=1, space="PSUM")
            )
            weights = {}

            def sA0(i, st):
                xT = work.tile([P, D // P, NB], mmdt, name=f"xT_{i}",
                               tag="xT", bufs=2)
                nc.gpsimd.dma_start(
                    xT[:],
                    x_d.ap().rearrange("(kt p) (nt c) -> p kt nt c", p=P,
                                       c=NB)[:, :, i, :],
                )
                st["xT"] = xT
                if i == 0:
                    # weight loads go after the first input tile on the
                    # gpsimd queue so PE can start as soon as w1 lands;
                    # w1/w2 chunks interleave so L2 isn't starved either
                    w1 = wp.tile([P, D // P, H], mmdt, name="w1")
                    w2 = wp.tile([P, H // P, H], mmdt, name="w2")
                    s1 = w1_d.ap().rearrange("(kt p) n -> p kt n", p=P)
                    s2 = w2_d.ap().rearrange("(kt p) n -> p kt n", p=P)
                    for k in range(H // P):
                        if k < D // P:
                            nc.gpsimd.dma_start(
                                w1[:, k : k + 1, :], s1[:, k : k + 1, :]
                            )
                        nc.gpsimd.dma_start(
                            w2[:, k : k + 1, :], s2[:, k : k + 1, :]
                        )
                    weights["w1"] = w1
                    weights["w2"] = w2
                    weights["wmv"] = load_w(wp, wmv_d, H // P, 2 * L, "wmv")
                    ctxT = wp.tile([P, 2, M], F32, name="ctxT")
                    nc.sync.dma_start(
                        ctxT[:],
                        ctxT_d.ap().rearrange("(kt p) n -> p kt n", p=P),
                    )
                    weights["ctxT"] = ctxT
                    ctxm = wp.tile([M, L], mmdt, name="ctxm")
                    nc.gpsimd.dma_start(ctxm[:], ctx_d.ap())
                    weights["ctxm"] = ctxm
                if i == 1:
                    prefetch["wdi"] = load_w(wpB, wdi_d, L // P, H, "wdi")
                    prefetch["wd1"] = load_w(wpB, wd1_d, H // P, H, "wd1")

            def sA1(i, st):
                st["h1T"] = act_lrelu(
                    work, mmF(ps_mm, weights["w1"], st.pop("xT"), D // P, H // P,
                              f"l1_{i}"),
                    H // P, f"l1_{i}", "l1",
                )

            def sA2(i, st):
                chunks = mmF(ps_mm, weights["w2"], st.pop("h1T"), H // P, H // P,
                             f"l2_{i}")
                st["h2T"] = act_lrelu(work, chunks, H // P, f"l2_{i}", "l2")
                st["rsb2"] = rs_row(work, ps_x, chunks, H, f"l2_{i}")

            def sA3(i, st):
                chunks = mmF(ps_mm, weights["wmv"], st.pop("h2T"), H // P,
                             2 * L // P, f"mv_{i}")
                rsb2 = st.pop("rsb2")
                smv = work.tile([P, 2 * L // P, NB], F32, name=f"smv_{i}",
                                tag="smv", bufs=2)
                for ps, m0, msz in chunks:
                    nc.vector.tensor_tensor(
                        out=smv[:, m0 : m0 + msz, :].rearrange(
                            "p k c -> p (k c)"
                        ),
                        in0=ps[:, : msz * NB],
                        in1=rsb2[:].rearrange("p k c -> p (k c)")[
                            :, : msz * NB
                        ],
                        op=ALU.mult,
                    )
                mu_ap = mu_d.ap().rearrange(
                    "(c p) (nt b) -> p c nt b", p=P, b=NB
                )[:, :, i, :]
                lv_ap = lv_d.ap().rearrange(
                    "(c p) (nt b) -> p c nt b", p=P, b=NB
                )[:, :, i, :]
                nc.sync.dma_start(mu_ap, smv[:, 0 : L // P, :])
                nc.sync.dma_start(lv_ap, smv[:, L // P :, :])

                elv = work.tile([P, L // P, NB], F32, name=f"elv_{i}",
                                tag="elv", bufs=1)
                for c in range(L // P):
                    nc.scalar.activation(
                        out=elv[:, c, :], in_=smv[:, L // P + c, :],
                        func=AF.Exp, bias=0.0, scale=0.5,
                    )
                epsT = work.tile([P, L // P, NB], F32, name=f"epsT_{i}",
                                 tag="epsT", bufs=2)
                nc.sync.dma_start(
                    epsT[:],
                    eps_d.ap().rearrange("(kt p) (nt c) -> p kt nt c", p=P,
                                         c=NB)[:, :, i, :],
                )
                zT = work.tile([P, L // P, NB], F32, name=f"zT_{i}",
                               tag="zT", bufs=2)
                nc.vector.tensor_tensor(
                    out=zT[:].rearrange("p k c -> p (k c)"),
                    in0=elv[:].rearrange("p k c -> p (k c)"),
                    in1=epsT[:].rearrange("p k c -> p (k c)"),
                    op=ALU.mult,
                )
                nc.vector.tensor_tensor(
                    out=zT[:].rearrange("p k c -> p (k c)"),
                    in0=zT[:].rearrange("p k c -> p (k c)"),
                    in1=smv[:, 0 : L // P, :].rearrange("p k c -> p (k c)"),
                    op=ALU.add,
                )
                st["zT"] = zT

            def sA4(i, st):
                zT = st.pop("zT")
                eT = work.tile([M, NB], mmdt, name=f"eT_{i}", tag="eT",
                               bufs=1)
                for bc in range(NB // P):
                    s_ps = ps_x.tile([P, 512], F32, name=f"sps_{i}_{bc}",
                                     tag="sps", bufs=2)
                    for k in range(L // P):
                        nc.tensor.matmul(
                            s_ps[:, :M],
                            zT[:, k, bc * P : (bc + 1) * P],
                            weights["ctxT"][:, k, :],
                            start=(k == 0), stop=(k == L // P - 1),
                        )
                    negmx = work.tile([P, 1], F32, name=f"nmx_{i}_{bc}",
                                      tag="nmx", bufs=4)
                    nc.vector.tensor_reduce(
                        out=negmx[:], in_=s_ps[:, :M],
                        axis=mybir.AxisListType.X, op=ALU.max, negate=True,
                    )
                    e_sb = work.tile([P, M], F32, name=f"e_{i}_{bc}",
                                     tag="e", bufs=4)
                    se = work.tile([P, 1], F32, name=f"se_{i}_{bc}",
                                   tag="se", bufs=4)
                    nc.scalar.activation(
                        out=e_sb[:], in_=s_ps[:, :M], func=AF.Exp,
                        bias=negmx[:], scale=1.0, accum_out=se[:],
                    )
                    rs01 = work.tile([P, 1], F32, name=f"r01_{i}_{bc}",
                                     tag="r01", bufs=4)
                    nc.vector.reciprocal(out=rs01[:], in_=se[:])
                    nc.vector.tensor_scalar_mul(out=rs01[:], in0=rs01[:],
                                                scalar1=0.1)
                    nc.vector.tensor_scalar(
                        out=e_sb[:], in0=e_sb[:], scalar1=rs01[:],
                        scalar2=None, op0=ALU.mult,
                    )
                    trE = ps_x.tile([P, 512], F32, name=f"trE_{i}_{bc}",
                                    tag="sps", bufs=2)
                    nc.tensor.transpose(trE[:M, :P], e_sb[:], ident[:])
                    nc.vector.tensor_copy(
                        eT[:, bc * P : (bc + 1) * P], trE[:M, :P]
                    )
                za_ps = ps_x.tile([P, 2, NB], F32, name=f"za_{i}",
                                  tag="sps", bufs=2)
                for c in range(L // P):
                    nc.tensor.matmul(
                        za_ps[:, c, :],
                        weights["ctxm"][:, c * P : (c + 1) * P],
                        eT[:],
                        start=True, stop=True,
                    )
                zeT = work.tile([P, L // P, NB], mmdt, name=f"zeT_{i}",
                                tag="zeT", bufs=2)
                nc.vector.tensor_tensor(
                    out=zeT[:].rearrange("p k c -> p (k c)"),
                    in0=zT[:].rearrange("p k c -> p (k c)"),
                    in1=za_ps[:].rearrange("p k c -> p (k c)"),
                    op=ALU.add,
                )
                nc.sync.dma_start(zenh_s[i], zeT[:])
                st.clear()

            sw_pipeline([sA0, sA1, sA2, sA3, sA4], NST, ORDER_A)

        # ================= PHASE B =================
        wpC = glob.enter_context(tc.tile_pool(name="wC", bufs=1))
        with ExitStack() as ph:
            work = ph.enter_context(tc.tile_pool(name="workB", bufs=2))
            ps_mm = ph.enter_context(
                tc.tile_pool(name="psB", bufs=1, space="PSUM")
            )
            wdi = prefetch["wdi"]
            wd1 = prefetch["wd1"]

            def sB0(i, st):
                if i == 1:
                    prefetch["wd2"] = load_w(wpC, wd2_d, H // P, 2 * H,
                                             "wd2")
                zeT = bridge.tile([P, L // P, NB], mmdt, name=f"zeB_{i}",
                                  tag="zeB", bufs=3)
                nc.sync.dma_start(zeT[:], zenh_s[i])
                st["zeT"] = zeT

            def sB1(i, st):
                st["d1T"] = act_lrelu(
                    work, mmF(ps_mm, wdi, st.pop("zeT"), L // P, H // P,
                              f"di_{i}"),
                    H // P, f"di_{i}", "di",
                )

            def sB2(i, st):
                d2T = act_lrelu(
                    work, mmF(ps_mm, wd1, st.pop("d1T"), H // P, H // P,
                              f"d1_{i}"),
                    H // P, f"d1_{i}", "d1",
                )
                nc.sync.dma_start(d2t_s[i], d2T[:])
                st.clear()

            sw_pipeline([sB0, sB1, sB2], NST, ORDER_B)
        wrapB.close()  # release di/dec1 weights before phase C pools

        # ================= PHASE C =================
        wpC2 = glob.enter_context(tc.tile_pool(name="wC2", bufs=1))
        prefetch["wd3"] = load_w(wpC2, wd3_d, 2 * H // P, D, "wd3")
        with ExitStack() as ph:
            work = ph.enter_context(tc.tile_pool(name="workC", bufs=2))
            ps_mm = ph.enter_context(
                tc.tile_pool(name="psC", bufs=1, space="PSUM")
            )
            ps_x = ph.enter_context(
                tc.tile_pool(name="psCx", bufs=1, space="PSUM")
            )
            wd2 = prefetch["wd2"]
            wd3 = prefetch["wd3"]

            def sC0(i, st):
                d2T = work.tile([P, H // P, NB], mmdt, name=f"d2C_{i}",
                                tag="d2C", bufs=3)
                nc.sync.dma_start(d2T[:], d2t_s[i])
                st["d2T"] = d2T

            def sC1(i, st):
                chunks = mmF(ps_mm, wd2, st.pop("d2T"), H // P,
                             2 * H // P, f"d2_{i}")
                st["d3T"] = act_lrelu(work, chunks, 2 * H // P, f"d2_{i}",
                                      "d2")
                st["rsb4"] = rs_row(work, ps_x, chunks, 2 * H, f"d2_{i}")

            def sC2(i, st):
                chunks = mmF(ps_mm, wd3, st.pop("d3T"), 2 * H // P,
                             D // P, f"d3_{i}")
                rsb4 = st.pop("rsb4")
                recon = work.tile([P, D // P, NB], F32, name=f"rec_{i}",
                                  tag="rec", bufs=2)
                for ps, m0, msz in chunks:
                    nc.vector.tensor_tensor(
                        out=recon[:, m0 : m0 + msz, :].rearrange(
                            "p k c -> p (k c)"
                        ),
                        in0=ps[:, : msz * NB],
                        in1=rsb4[:].rearrange("p k c -> p (k c)")[
                            :, : msz * NB
                        ],
                        op=ALU.mult,
                    )
                rec_ap = recon_d.ap().rearrange(
                    "(c p) (nt b) -> p c nt b", p=P, b=NB
                )[:, :, i, :]
                nc.sync.dma_start(rec_ap, recon[:])
                st.clear()

            sw_pipeline([sC0, sC1, sC2], NST, ORDER_C)

    nc.finalize()
    return nc


_NC_CACHE = {}


def _get_nc(simple=True):
    key = ("simple" if simple else "general", str(MM_DTYPE))
    if key not in _NC_CACHE:
        _NC_CACHE[key] = (
            _build_v3() if simple else _build_v2(simple=False)
        )
    return _NC_CACHE[key]


def kernel(**inputs):
    i = {
        k: np.ascontiguousarray(np.asarray(v, dtype=np.float32))
        for k, v in inputs.items()
    }
    zeros = all(
        not np.any(i[k])
        for k in (
            "enc_b1", "enc_b2", "mu_b", "lv_b", "di_b", "dec_b1", "dec_b2",
            "dec_b3", "ln1_b", "ln2_b", "dln1_b", "dln2_b",
        )
    )
    units = all(
        np.all(i[k] == 1.0) for k in ("ln1_g", "ln2_g", "dln1_g", "dln2_g")
    )
    simple = zeros and units
    nc = _get_nc(simple=simple)

    def _ct(w):  # transpose + center over out-features (folds LN mean)
        wt = np.ascontiguousarray(w.T)
        return wt - wt.mean(axis=1, keepdims=True)

    shared = {
        "w1t": _ct(i["enc_w1"]) if simple
        else np.ascontiguousarray(i["enc_w1"].T),
        "b1": i["enc_b1"],
        "g1": i["ln1_g"],
        "be1": i["ln1_b"],
        "w2t": _ct(i["enc_w2"]) if simple
        else np.ascontiguousarray(i["enc_w2"].T),
        "b2": i["enc_b2"],
        "g2": i["ln2_g"],
        "be2": i["ln2_b"],
        "wmvt": np.ascontiguousarray(
            np.concatenate([i["mu_w"].T, i["lv_w"].T], axis=1)
        ),
        "bmv": np.concatenate([i["mu_b"], i["lv_b"]]),
        "ctxT": np.ascontiguousarray(i["ctx_mem"].T),
        "ctx": i["ctx_mem"],
        "wdit": np.ascontiguousarray(i["di_w"].T),
        "dib": i["di_b"],
        "wd1t": _ct(i["dec_w1"]) if simple
        else np.ascontiguousarray(i["dec_w1"].T),
        "db1": i["dec_b1"],
        "dg1": i["dln1_g"],
        "dbe1": i["dln1_b"],
        "wd2t": _ct(i["dec_w2"]) if simple
        else np.ascontiguousarray(i["dec_w2"].T),
        "db2": i["dec_b2"],
        "dg2": i["dln2_g"],
        "dbe2": i["dln2_b"],
        "wd3t": np.ascontiguousarray(i["dec_w3"].T),
        "db3": i["dec_b3"],
    }
    in_names = {
        alloc.memorylocations[0].name
        for alloc in nc.m.functions[0].allocations
        if isinstance(alloc, mybir.MemoryLocationSet)
        and alloc.kind == "ExternalInput"
    }
    shared = {k: v for k, v in shared.items() if k in in_names}
    in_maps = []
    for c in range(N_CORES):
        m = dict(shared)
        xc = i["x"][c * B_LOC : (c + 1) * B_LOC]
        ec = i["eps"][c * B_LOC : (c + 1) * B_LOC]
        m["x"] = np.ascontiguousarray(xc.T)
        m["eps"] = np.ascontiguousarray(ec.T) if simple else ec
        in_maps.append(m)

    res = run_bass_kernel_spmd(nc, in_maps, core_ids=list(range(N_CORES)))
    if simple:
        recon = np.concatenate(
            [r["recon"].T for r in res.results], axis=0
        )
        mu = np.concatenate([r["mu"].T for r in res.results], axis=0)
        lv = np.concatenate([r["lv"].T for r in res.results], axis=0)
    else:
        recon = np.concatenate([r["recon"] for r in res.results], axis=0)
        mu = np.concatenate([r["mu"] for r in res.results], axis=0)
        lv = np.concatenate([r["lv"] for r in res.results], axis=0)
    return recon, mu, lv
